# revision 5
# baseline (speedup 1.0000x reference)
"""nn_Block_21062519619681 fully on-device: hybrid Mamba2 + MQA + RWKV-CMix
block as ONE Bass/Tile SPMD kernel on 8 trn2 NeuronCores.

Sharding: 8 cores = 4 batches x 2 token-halves (512 own tokens/core).
 - mamba: token-sharded; chunked-SSD scan (L=128); cross-half state carry via
   a pairwise AllGather applied as a linear correction pass.
 - attention: q-head-split (8 heads/core over ALL 1024 tokens; per-core
   permuted q/proj weights keep the SPMD graph rank-uniform); k/v + q halves
   exchanged via pairwise AllGather; softmax without max-subtraction (scores
   bounded); colsum ridden as a ones-column in the av matmul.
 - cmix: token-sharded, replicated weights, erf/sigmoid fused into PSUM evac.
All matmuls bf16 (weights pre-cast on host), fp32 PSUM accumulate, fp32
residual stream. Rank-dependent selection uses host-fed 0/1 masks (masked
sums) - the instruction graph is identical on all cores.
"""
import os
import sys

sys.path.insert(0, "/opt/trn_rl_repo")
import numpy as np
import ml_dtypes

B_, T_, C_ = 4, 1024, 1024
NH, HD = 16, 64
DS, DCONV, EXP, PHD = 64, 4, 2, 64
DIN = EXP * C_
NHM = DIN // PHD
CONVD = DIN + 2 * DS
FFN = 4 * C_
EPS = 1e-5
N_CORES = 8
TOK = 512
TH = TOK + 3
L = 128
NCH = TOK // L
NEG = -1e30

BF16 = ml_dtypes.bfloat16
DEBUG = bool(int(os.environ.get("BASSK_DEBUG", "0")))
PHASES = int(os.environ.get("BASSK_PHASES", "6"))

_CACHE = {}


def _build():
    import contextlib
    import concourse.mybir as mybir
    import concourse.bacc as bacc
    import concourse.tile as tile
    from concourse.masks import make_identity

    f32 = mybir.dt.float32
    bf16 = mybir.dt.bfloat16
    AF = mybir.ActivationFunctionType
    OP = mybir.AluOpType

    nc = bacc.Bacc("TRN2", target_bir_lowering=False, debug=False,
                   num_devices=N_CORES)

    def din(name, shape, dt=bf16):
        return nc.dram_tensor(name, shape, dt, kind="ExternalInput").ap()

    xin = din("xin", [C_, TH], f32)
    w_inproj = din("w_inproj", [C_, 4256])
    convw = din("convw", [128, 17, DCONV], f32)
    convb = din("convb", [128, 17], f32)
    dtb = din("dtb", [NHM, 1], f32)
    aneg = din("aneg", [NHM, 1], f32)
    drep = din("drep", [128, 16], f32)
    mnw = din("mnw", [128, 16], f32)
    w_outproj = din("w_outproj", [DIN, C_])
    w_att = din("w_att", [C_, 1024 + 128])
    w_proj = din("w_proj", [C_, C_])
    mk = din("mk", [128, 8], f32)
    mk1 = din("mk1", [128, 8], f32)
    mr = din("mr", [128, 8], f32)
    mr1 = din("mr1", [128, 8], f32)
    w_key = din("w_key", [C_, FFN])
    w_val = din("w_val", [FFN, C_])
    vbias = din("vbias", [128, 8], f32)
    w_rec = din("w_rec", [C_, C_])
    msk = din("msk", [128, 2], f32)
    efull = din("efull", [NHM, DIN])

    out_d = nc.dram_tensor("out", [C_, TOK], f32, kind="ExternalOutput").ap()

    dbg_outs = {}

    def dbg_dump(name, ap_or_tile, shape, dt=None):
        if not DEBUG:
            return
        d = nc.dram_tensor(name, shape, dt or mybir.dt.float32,
                           kind="ExternalOutput").ap()
        dbg_outs[name] = d
        nc.sync.dma_start(d, ap_or_tile)

    RG = [[0, 1], [2, 3], [4, 5], [6, 7]]

    with tile.TileContext(nc) as tc:
        _open = []

        def stack():
            s = contextlib.ExitStack()
            _open.append(s)
            return s

        whole = contextlib.ExitStack()
        consts = whole.enter_context(tc.tile_pool(name="consts", bufs=1))
        resid = whole.enter_context(tc.tile_pool(name="resid", bufs=1))
        wsl = whole.enter_context(tc.tile_pool(name="wsl", bufs=3))
        dram = whole.enter_context(tc.tile_pool(name="dram", bufs=1,
                                                space="DRAM"))

        # ---------------- constants ----------------
        ident_b = consts.tile([128, 128], bf16, name="ident_b")
        make_identity(nc, ident_b)
        ident_f = consts.tile([128, 128], f32, name="ident_f")
        make_identity(nc, ident_f)
        tri01 = consts.tile([128, 128], bf16, name="tri01")
        nc.vector.memset(tri01, 1.0)
        nc.gpsimd.affine_select(out=tri01, in_=tri01, compare_op=OP.is_ge,
                                fill=0.0, base=0, channel_multiplier=-1,
                                pattern=[[1, 128]])
        onesb = consts.tile([128, 1], bf16, name="onesb")
        nc.vector.memset(onesb, 1.0)
        ones32 = consts.tile([NHM, L], f32, name="ones32")
        nc.vector.memset(ones32, 1.0)
        eps_c = consts.tile([128, 1], f32, name="eps_c")
        nc.vector.memset(eps_c, EPS)
        _MU = float(np.sqrt(0.5))
        _DEN = float(np.sqrt(1.0 / (4.0 * np.pi)) * np.sqrt(2.0))
        erfb_c = consts.tile([128, 1], f32, name="erfb_c")
        nc.vector.memset(erfb_c, -_MU / _DEN)

        def cin(name, shape, src, dt=f32):
            t = consts.tile(list(shape), dt, name=name)
            nc.sync.dma_start(t[:], src)
            return t

        convw_s = cin("convw_s", [128, 17, DCONV], convw)
        convb_s = cin("convb_s", [128, 17], convb)
        dtb_s = cin("dtb_s", [NHM, 1], dtb)
        aneg_s = cin("aneg_s", [NHM, 1], aneg)
        drep_s = cin("drep_s", [128, 16], drep)
        mnw_s = cin("mnw_s", [128, 16], mnw)
        mk_s = cin("mk_s", [128, 8], mk)
        mk1_s = cin("mk1_s", [128, 8], mk1)
        mr_s = cin("mr_s", [128, 8], mr)
        mr1_s = cin("mr1_s", [128, 8], mr1)
        vbias_s = cin("vbias_s", [128, 8], vbias)
        msk_s = cin("msk_s", [128, 2], msk)
        efull_s = cin("efull_s", [NHM, DIN], efull, dt=bf16)
        is_first = msk_s[:, 0:1]
        is_second = msk_s[:, 1:2]

        zsil_d = dram.tile([DIN, TOK], bf16, name="zsil_d")
        x1_d = dram.tile([C_, TOK], f32, name="x1_d")
        x2_d = dram.tile([C_, TOK], f32, name="x2_d")

        def wslab(wt, m0, mw, kt, name, pool=None, tag="wslab", bufs=None):
            s = (pool or wsl).tile([128, kt, mw], bf16, name=name, tag=tag,
                                   bufs=bufs)
            nc.sync.dma_start(
                s[:], wt[:, m0:m0 + mw].rearrange("(t p) m -> p t m", p=128))
            return s

        def rmsnorm_cm(src_aps, dst_aps, width, pool, psp, nfeat, tag):
            ssq = psp.tile([1, width], f32, name=f"ssq_{tag}",
                           tag=f"ssq{tag}", bufs=1)
            n = len(src_aps)
            for i, sap in enumerate(src_aps):
                sq = pool.tile([128, width], bf16, name=f"sq_{tag}",
                               tag=f"sq{tag}", bufs=2)
                nc.vector.tensor_mul(sq[:], sap, sap)
                nc.tensor.matmul(ssq[:], onesb[:], sq[:], start=(i == 0),
                                 stop=(i == n - 1))
            rms = pool.tile([1, width], f32, name=f"rms_{tag}",
                            tag=f"rms{tag}", bufs=1)
            nc.scalar.activation(rms[:], ssq[:], AF.Sqrt,
                                 bias=eps_c[0:1, :], scale=1.0 / nfeat)
            rinv = pool.tile([1, width], f32, name=f"rinv_{tag}",
                             tag=f"rinv{tag}", bufs=1)
            nc.vector.reciprocal(rinv[:], rms[:])
            rbc = pool.tile([128, width], f32, name=f"rbc_{tag}",
                            tag=f"rbc{tag}", bufs=1)
            nc.gpsimd.partition_broadcast(rbc[:], rinv[:], channels=128)
            for i, sap in enumerate(src_aps):
                nc.vector.tensor_mul(dst_aps[i], sap, rbc[:])

        # pool nesting (open early -> close late):
        stD = stack()
        pD = stD.enter_context(tc.tile_pool(name="pD", bufs=1))   # ..P5
        stY = stack()
        pY = stY.enter_context(tc.tile_pool(name="pY", bufs=1))   # ..P4
        stB = stack()
        pB = stB.enter_context(tc.tile_pool(name="pB", bufs=1))   # ..P3
        stC = stack()
        pC = stC.enter_context(tc.tile_pool(name="pC", bufs=1))   # ..P3

        # ================= P0 + P1: rmsnorm + in_proj =================
        st01 = stack()
        pA = st01.enter_context(tc.tile_pool(name="pA", bufs=1))
        ps01 = st01.enter_context(tc.tile_pool(name="ps01", bufs=1,
                                               space="PSUM"))
        xn = [pA.tile([128, TH], bf16, name=f"xn{i}") for i in range(8)]
        # streaming rmsnorm over x (full TH width, stats on own 512 cols)
        ssqx = ps01.tile([1, 512], f32, name="ssqx", tag="ssqx", bufs=1)
        ssqh = ps01.tile([1, 3], f32, name="ssqh", tag="ssqh", bufs=1)
        for i in range(8):
            xt = pA.tile([128, TH], f32, name="xt", tag="xt", bufs=3)
            nc.sync.dma_start(xt[:], xin[128 * i:128 * (i + 1), :])
            sqx = pA.tile([128, TH], bf16, name="sqx", tag="sqx", bufs=2)
            nc.vector.tensor_mul(sqx[:], xt[:], xt[:])
            nc.tensor.matmul(ssqx[:], onesb[:], sqx[:, 3:TH],
                             start=(i == 0), stop=(i == 7))
            nc.tensor.matmul(ssqh[:], onesb[:], sqx[:, 0:3],
                             start=(i == 0), stop=(i == 7))
        rmsx = pA.tile([1, TH], f32, name="rmsx")
        nc.scalar.activation(rmsx[:, 3:TH], ssqx[:], AF.Sqrt,
                             bias=eps_c[0:1, :], scale=1.0 / C_)
        nc.scalar.activation(rmsx[:, 0:3], ssqh[:], AF.Sqrt,
                             bias=eps_c[0:1, :], scale=1.0 / C_)
        rinvx = pA.tile([1, TH], f32, name="rinvx")
        nc.vector.reciprocal(rinvx[:], rmsx[:])
        rbcx = pA.tile([128, TH], f32, name="rbcx")
        nc.gpsimd.partition_broadcast(rbcx[:], rinvx[:], channels=128)
        for i in range(8):
            xt = pA.tile([128, TH], f32, name="xt", tag="xt", bufs=3)
            nc.sync.dma_start(xt[:], xin[128 * i:128 * (i + 1), :])
            nc.vector.tensor_mul(xn[i][:], xt[:], rbcx[:])
        dbg_dump("d_xn0", xn[0][:], [128, TH], mybir.dt.bfloat16)

        xbc = [pB.tile([128, TH], bf16, name=f"xbc{i}") for i in range(17)]
        dtraw = pB.tile([NHM, TOK], f32, name="dtraw")

        for mb in range(16):
            sl = wslab(w_inproj, 128 * mb, 128, 8, f"wz{mb}")
            pz = ps01.tile([128, TOK], f32, name="pz", tag="pbig", bufs=3)
            for k in range(8):
                nc.tensor.matmul(pz[:], sl[:, k, :], xn[k][:, 3:TH],
                                 start=(k == 0), stop=(k == 7))
            zst = pA.tile([128, TOK], bf16, name="zst", tag="zst", bufs=3)
            nc.scalar.activation(zst[:], pz[:], AF.Silu)
            nc.sync.dma_start(zsil_d[128 * mb:128 * (mb + 1), :], zst[:])
        for mb in range(17):
            sl = wslab(w_inproj, DIN + 128 * mb, 128, 8, f"wxbc{mb}")
            pb_ = ps01.tile([128, TOK], f32, name="pb", tag="pbig", bufs=3)
            ph = ps01.tile([128, 3], f32, name="ph", tag="phalo", bufs=2)
            for k in range(8):
                nc.tensor.matmul(pb_[:], sl[:, k, :], xn[k][:, 3:TH],
                                 start=(k == 0), stop=(k == 7))
                nc.tensor.matmul(ph[:], sl[:, k, :], xn[k][:, 0:3],
                                 start=(k == 0), stop=(k == 7))
            nc.vector.tensor_copy(xbc[mb][:, 3:TH], pb_[:])
            nc.vector.tensor_copy(xbc[mb][:, 0:3], ph[:])
        sl = wslab(w_inproj, 4224, 32, 8, "wdtp")
        pdt = ps01.tile([NHM, TOK], f32, name="pdt", tag="pdt", bufs=1)
        for k in range(8):
            nc.tensor.matmul(pdt[:], sl[:, k, :], xn[k][:, 3:TH],
                             start=(k == 0), stop=(k == 7))
        nc.vector.tensor_copy(dtraw[:], pdt[:])
        dbg_dump("d_xbc0", xbc[0][:], [128, TH], mybir.dt.bfloat16)
        st01.close()

        if PHASES >= 2:
            # ============ P2: conv + dt pipeline + transposes ============
            ps2 = stack()
            ps2p = ps2.enter_context(tc.tile_pool(name="ps2", bufs=1,
                                                  space="PSUM"))
            xs_cm = [pC.tile([128, TOK], bf16, name=f"xs_cm{i}")
                     for i in range(17)]
            for i in range(17):
                tmp = pC.tile([128, TOK], f32, name="ctmp", tag="ctmp",
                              bufs=2)
                nc.vector.tensor_scalar(tmp[:], xbc[i][:, 0:TOK],
                                        convw_s[:, i, 0:1], None,
                                        op0=OP.mult)
                for j in range(1, DCONV):
                    nc.vector.scalar_tensor_tensor(
                        tmp[:], xbc[i][:, j:j + TOK], convw_s[:, i, j:j + 1],
                        tmp[:], op0=OP.mult, op1=OP.add)
                nc.scalar.activation(xs_cm[i][:], tmp[:], AF.Silu,
                                     bias=convb_s[:, i:i + 1])
            C_cm = pC.tile([64, TOK], bf16, name="C_cm")
            nc.vector.tensor_copy(C_cm[:], xs_cm[16][64:128, :])
            dbg_dump("d_xs0", xs_cm[0][:], [128, TOK], mybir.dt.bfloat16)

            dt_f = pC.tile([NHM, TOK], f32, name="dt_f")
            Lc = pC.tile([NHM, TOK], f32, name="Lc")
            wdt = pC.tile([NHM, TOK], f32, name="wdt", tag="scr", bufs=2)
            u = pC.tile([NHM, TOK], f32, name="u")
            ex = pC.tile([NHM, TOK], f32, name="ex", tag="scr", bufs=2)
            nc.vector.tensor_scalar(u[:], dtraw[:], dtb_s[:], None,
                                    op0=OP.add)
            ab = pC.tile([NHM, TOK], f32, name="ab", tag="scr", bufs=2)
            nc.vector.tensor_scalar(ab[:], u[:], -1.0, None, op0=OP.mult)
            nc.vector.tensor_max(ab[:], ab[:], u[:])
            nc.scalar.activation(ex[:], ab[:], AF.Exp, scale=-1.0)
            nc.scalar.activation(ex[:], ex[:], AF.Ln, bias=1.0)
            nc.vector.tensor_scalar(dt_f[:], u[:], 0.0, None, op0=OP.max)
            nc.vector.tensor_add(dt_f[:], dt_f[:], ex[:])
            dta = u
            nc.vector.tensor_scalar(dta[:], dt_f[:], aneg_s[:], None,
                                    op0=OP.mult)
            for c in range(NCH):
                cs = slice(L * c, L * (c + 1))
                nc.vector.tensor_tensor_scan(Lc[:, cs], ones32[:],
                                             dta[:, cs], 0.0, op0=OP.mult,
                                             op1=OP.add)
                nc.scalar.activation(wdt[:, cs], Lc[:, cs], AF.Exp,
                                     scale=-1.0,
                                     bias=Lc[:, L * (c + 1) - 1:L * (c + 1)])
            nc.vector.tensor_mul(wdt[:], wdt[:], dt_f[:])

            dbg_dump("d_dt", dt_f[:], [NHM, TOK])
            dbg_dump("d_Lc", Lc[:], [NHM, TOK])

            eLcE = pC.tile([NHM, NCH], bf16, name="eLcE")
            lce = pC.tile([NHM, NCH], f32, name="lce")
            for c in range(NCH):
                nc.vector.tensor_copy(lce[:, c:c + 1],
                                      Lc[:, L * (c + 1) - 1:L * (c + 1)])
            nc.scalar.activation(eLcE[:], lce[:], AF.Exp)
            arep = [pC.tile([128, NCH], f32, name=f"arep{k}")
                    for k in range(16)]
            carep = [pC.tile([128, NCH], f32, name=f"carep{k}")
                     for k in range(16)]
            for k in range(16):
                pa = ps2p.tile([128, NCH], f32, name="pa", tag="pa", bufs=2)
                nc.tensor.matmul(pa[:], efull_s[:, 128 * k:128 * (k + 1)],
                                 eLcE[:], start=True, stop=True)
                nc.vector.tensor_copy(arep[k][:], pa[:])
                nc.vector.memset(carep[k][:, 0:1], 1.0)
                for c in range(1, NCH):
                    nc.vector.tensor_mul(carep[k][:, c:c + 1],
                                         carep[k][:, c - 1:c],
                                         arep[k][:, c - 1:c])

            tmv = [pC.tile([128, 96], f32, name=f"tmv{c}")
                   for c in range(NCH)]
            stk = pC.tile([96, TOK], f32, name="stk")
            nc.vector.tensor_scalar(stk[0:NHM, :], Lc[:], -1.0, None,
                                    op0=OP.mult)
            nc.vector.tensor_copy(stk[NHM:2 * NHM, :], dt_f[:])
            nc.vector.tensor_copy(stk[2 * NHM:3 * NHM, :], wdt[:])
            for c in range(NCH):
                pt = ps2p.tile([128, 96], f32, name="pt", tag="ptr", bufs=2)
                nc.tensor.transpose(pt[:], stk[:, L * c:L * (c + 1)],
                                    ident_f[0:96, 0:96])
                nc.vector.tensor_copy(tmv[c][:], pt[:])

            xs_tm = [pC.tile([128, 2176], bf16, name=f"xs_tm{c}")
                     for c in range(NCH)]
            for c in range(NCH):
                for i in range(17):
                    ptb = ps2p.tile([128, 128], bf16, name="ptb", tag="ptrb",
                                    bufs=3)
                    nc.tensor.transpose(ptb[:],
                                        xs_cm[i][:, L * c:L * (c + 1)],
                                        ident_b[:])
                    nc.vector.tensor_copy(
                        xs_tm[c][:, 128 * i:128 * (i + 1)], ptb[:])
            dbg_dump("d_xstm0", xs_tm[0][:], [128, 2176], mybir.dt.bfloat16)
            ps2.close()

        if PHASES >= 3:
            # ========== P3: scan (interleaved per chunk) ==========
            ps3 = stack()
            ps3p = ps3.enter_context(tc.tile_pool(name="ps3", bufs=1,
                                                  space="PSUM"))
            state = [pC.tile([128, PHD], f32, name=f"state{k}")
                     for k in range(16)]
            for k in range(16):
                nc.vector.memset(state[k][:], 0.0)
            stateb = [pC.tile([64, PHD], bf16, name=f"stateb{h}")
                      for h in range(NHM)]
            ycm = [pY.tile([128, TOK], bf16, name=f"ycm{k}")
                   for k in range(16)]

            def stage_bcast(lcf, hh, with_exp=True):
                lba = pC.tile([128, 16 * L], f32, name="lba", tag="lball",
                              bufs=1)
                nc.gpsimd.partition_broadcast(
                    lba[:], lcf[0:1, 16 * L * hh:16 * L * (hh + 1)],
                    channels=128)
                eba = None
                if with_exp:
                    eba = pC.tile([64, 16 * L], bf16, name="eba",
                                  tag="eball", bufs=1)
                    nc.scalar.activation(eba[:], lba[0:64, :], AF.Exp)
                return lba, eba

            def make_cdec(eba, h, cs):
                off = L * (h % 16)
                cd = pC.tile([64, L], bf16, name="cd", tag="cdec", bufs=3)
                nc.vector.tensor_mul(cd[:], C_cm[:, cs],
                                     eba[:, off:off + L])
                return cd

            def stage_lc(c):
                t = pC.tile([1, NHM * L], f32, name=f"LcFc{c}",
                            tag="lcf", bufs=2)
                nc.sync.dma_start(t[0:1, :], Lc[:, L * c:L * (c + 1)])
                return t

            for c in range(NCH):
                cs = slice(L * c, L * (c + 1))
                lcf = stage_lc(c)
                if c > 0:
                    for h in range(NHM):
                        nc.vector.tensor_copy(
                            stateb[h][:],
                            state[h // 2][64 * (h % 2):64 * (h % 2) + 64, :])
                pg = ps3p.tile([128, L], f32, name="pg", tag="pg", bufs=1)
                nc.tensor.matmul(pg[:], xs_cm[16][0:64, cs], C_cm[:, cs],
                                 start=True, stop=True)
                gts = pC.tile([128, L], bf16, name="gts", tag="gts", bufs=2)
                nc.vector.tensor_mul(gts[:], pg[:], tri01[:])
                lba = eba = None
                for h in range(NHM):
                    k = h // 2
                    rows = slice(64 * (h % 2), 64 * (h % 2) + 64)
                    if h % 16 == 0:
                        lba, eba = stage_bcast(lcf, h // 16,
                                               with_exp=(c > 0))
                    darg = pC.tile([128, L], f32, name="darg", tag="darg",
                                   bufs=4)
                    nc.vector.tensor_scalar(darg[:],
                                            lba[:, L * (h % 16):
                                                L * (h % 16) + L],
                                            tmv[c][:, h:h + 1], 0.0,
                                            op0=OP.add, op1=OP.min)
                    expd = pC.tile([128, L], f32, name="expd", tag="expd",
                                   bufs=4)
                    nc.scalar.activation(expd[:], darg[:], AF.Exp)
                    mt = pC.tile([128, L], bf16, name="mt", tag="mt", bufs=4)
                    nc.vector.scalar_tensor_tensor(
                        mt[:], gts[:], tmv[c][:, 32 + h:33 + h], expd[:],
                        op0=OP.mult, op1=OP.mult)
                    py = ps3p.tile([64, L], f32, name="py", tag="py", bufs=2)
                    nc.tensor.matmul(py[:],
                                     xs_tm[c][:, PHD * h:PHD * (h + 1)],
                                     mt[:], start=True, stop=(c == 0))
                    if c > 0:
                        cd = make_cdec(eba, h, cs)
                        nc.tensor.matmul(py[:], stateb[h][:], cd[:],
                                         start=False, stop=True)
                    nc.vector.scalar_tensor_tensor(
                        ycm[k][rows, cs], xs_cm[k][rows, cs],
                        drep_s[rows, k:k + 1], py[:], op0=OP.mult,
                        op1=OP.add)
                    bw = pC.tile([128, DS], bf16, name="bw", tag="bw",
                                 bufs=3)
                    nc.vector.tensor_scalar(
                        bw[:], xs_tm[c][:, DIN:DIN + DS],
                        tmv[c][:, 64 + h:65 + h], None, op0=OP.mult)
                    psc = ps3p.tile([64, PHD], f32, name="psc", tag="psc",
                                    bufs=2)
                    nc.tensor.matmul(psc[:], bw[:],
                                     xs_tm[c][:, PHD * h:PHD * (h + 1)],
                                     start=True, stop=True)
                    nc.vector.scalar_tensor_tensor(
                        state[k][rows, :], state[k][rows, :],
                        arep[k][rows, c:c + 1], psc[:], op0=OP.mult,
                        op1=OP.add)

            b1_in = dram.tile([128, 16 * PHD], bf16, name="b1_in")
            b1_out = dram.tile([256, 16 * PHD], bf16, name="b1_out")
            steb = pC.tile([128, 16 * PHD], bf16, name="steb")
            for k in range(16):
                nc.vector.tensor_copy(steb[:, PHD * k:PHD * (k + 1)],
                                      state[k][:])
            nc.sync.dma_start(b1_in[:], steb[:])
            nc.gpsimd.collective_compute(
                "AllGather", OP.bypass, replica_groups=RG,
                ins=[b1_in.opt()], outs=[b1_out.opt()])
            dbg_dump("d_st0", state[0][:], [128, PHD])

            h0bf2 = [pC.tile([64, PHD], bf16, name=f"h0bf2{h}")
                     for h in range(NHM)]
            for k in range(16):
                rcv = pC.tile([128, PHD], bf16, name="rcv", tag="rcv",
                              bufs=2)
                nc.sync.dma_start(rcv[:],
                                  b1_out[0:128, PHD * k:PHD * (k + 1)])
                for j in (0, 1):
                    nc.vector.tensor_scalar(
                        h0bf2[2 * k + j][:], rcv[64 * j:64 * j + 64, :],
                        is_second[0:64, :], None, op0=OP.mult)
            for c in range(NCH):
                cs = slice(L * c, L * (c + 1))
                lcf2 = stage_lc(c)
                eba2 = None
                for h in range(NHM):
                    k = h // 2
                    rows = slice(64 * (h % 2), 64 * (h % 2) + 64)
                    if h % 16 == 0:
                        _, eba2 = stage_bcast(lcf2, h // 16)
                    cd = make_cdec(eba2, h, cs)
                    pyc = ps3p.tile([64, L], f32, name="pyc", tag="pyc",
                                    bufs=3)
                    nc.tensor.matmul(pyc[:], h0bf2[h][:], cd[:], start=True,
                                     stop=True)
                    # ycm += cumalpha * (h0^T @ Cdec)
                    nc.vector.scalar_tensor_tensor(
                        ycm[k][rows, cs], pyc[:],
                        carep[k][rows, c:c + 1], ycm[k][rows, cs],
                        op0=OP.mult, op1=OP.add)
            dbg_dump("d_y0", ycm[0][:], [128, TOK], mybir.dt.bfloat16)
            ps3.close()
            stC.close()
            stB.close()

        if PHASES >= 4:
            # ======== P4: gated norm + out_proj + x1 + rmsnorm2 ========
            st4 = stack()
            p4 = st4.enter_context(tc.tile_pool(name="p4", bufs=1))
            ps4s = stack()
            ps4 = ps4s.enter_context(tc.tile_pool(name="ps4", bufs=1,
                                                  space="PSUM"))
            g = [p4.tile([128, TOK], bf16, name=f"g{k}") for k in range(16)]
            for k in range(16):
                zs = p4.tile([128, TOK], bf16, name="zs", tag="zs", bufs=3)
                nc.sync.dma_start(zs[:], zsil_d[128 * k:128 * (k + 1), :])
                nc.vector.tensor_mul(g[k][:], ycm[k][:], zs[:])
            ssq = ps4.tile([1, TOK], f32, name="ssqg", tag="ssqg", bufs=1)
            for k in range(16):
                sq = p4.tile([128, TOK], bf16, name="gsq", tag="gsq", bufs=2)
                nc.vector.tensor_mul(sq[:], g[k][:], g[k][:])
                nc.tensor.matmul(ssq[:], onesb[:], sq[:], start=(k == 0),
                                 stop=(k == 15))
            rms = p4.tile([1, TOK], f32, name="grms")
            nc.scalar.activation(rms[:], ssq[:], AF.Sqrt,
                                 bias=eps_c[0:1, :], scale=1.0 / DIN)
            rinv = p4.tile([1, TOK], f32, name="grinv")
            nc.vector.reciprocal(rinv[:], rms[:])
            rbc = p4.tile([128, TOK], f32, name="grbc")
            nc.gpsimd.partition_broadcast(rbc[:], rinv[:], channels=128)
            for k in range(16):
                nc.vector.scalar_tensor_tensor(g[k][:], g[k][:],
                                               mnw_s[:, k:k + 1], rbc[:],
                                               op0=OP.mult, op1=OP.mult)
            dbg_dump("d_g0", g[0][:], [128, TOK], mybir.dt.bfloat16)

            x1 = [p4.tile([128, TOK], f32, name=f"x1_{i}") for i in range(8)]
            x1pb = pD.tile([128, 8], bf16, name="x1pb")
            for mb in range(8):
                sl = wslab(w_outproj, 128 * mb, 128, 16, f"wop{mb}")
                po = ps4.tile([128, TOK], f32, name="po", tag="pbig4",
                              bufs=3)
                for k in range(16):
                    nc.tensor.matmul(po[:], sl[:, k, :], g[k][:],
                                     start=(k == 0), stop=(k == 15))
                xre = p4.tile([128, TOK], f32, name="xre", tag="xre", bufs=2)
                nc.sync.dma_start(xre[:],
                                  xin[128 * mb:128 * (mb + 1), 3:TH])
                nc.vector.scalar_tensor_tensor(x1[mb][:], xre[:], 1.0,
                                               po[:], op0=OP.mult,
                                               op1=OP.add)
                nc.sync.dma_start(x1_d[128 * mb:128 * (mb + 1), :],
                                  x1[mb][:])
                nc.vector.tensor_copy(x1pb[:, mb:mb + 1],
                                      x1[mb][:, TOK - 1:TOK])
            x1n = [pD.tile([128, TOK], bf16, name=f"x1n{i}")
                   for i in range(8)]
            rmsnorm_cm([x1[i][:] for i in range(8)],
                       [x1n[i][:] for i in range(8)], TOK, p4, ps4, C_, "n1")
            dbg_dump("d_x1_0", x1[0][:], [128, TOK])
            ps4s.close()
            st4.close()
            stY.close()

        if PHASES >= 5:
            # ================= P5: attention =================
            st5 = stack()
            p5 = st5.enter_context(tc.tile_pool(name="p5", bufs=1))
            ps5s = stack()
            ps5 = ps5s.enter_context(tc.tile_pool(name="ps5", bufs=1,
                                                  space="PSUM"))
            amask = []
            for r in range(4):
                # keep when t >= s: f - p + (512*qb - 128*sb) >= 0,
                # variant j = sb - 4*qb in {0..3} -> base = -128*j
                m = p5.tile([128, 512], bf16, name=f"amask{r}")
                nc.vector.memset(m, 0.0)
                nc.gpsimd.affine_select(out=m, in_=m, compare_op=OP.is_ge,
                                        fill=NEG, base=-128 * r,
                                        channel_multiplier=-1,
                                        pattern=[[1, 512]])
                amask.append(m)
            qloc = [p5.tile([128, TOK], bf16, name=f"qloc{i}")
                    for i in range(8)]
            kloc = p5.tile([64, TOK], bf16, name="kloc")
            for mb in range(8):
                sl = wslab(w_att, 128 * mb, 128, 8, f"wq{mb}")
                pq = ps5.tile([128, TOK], f32, name="pq", tag="pbig5",
                              bufs=2)
                for k in range(8):
                    nc.tensor.matmul(pq[:], sl[:, k, :], x1n[k][:],
                                     start=(k == 0), stop=(k == 7))
                nc.vector.tensor_copy(qloc[mb][:], pq[:])
            slk = wslab(w_att, 1024, 64, 8, "wkp")
            pk = ps5.tile([64, TOK], f32, name="pk", tag="psx", bufs=2)
            for k in range(8):
                nc.tensor.matmul(pk[:], slk[:, k, :], x1n[k][:],
                                 start=(k == 0), stop=(k == 7))
            nc.vector.tensor_copy(kloc[:], pk[:])
            vloc = [p5.tile([128, 65], bf16, name=f"vloc{tb}")
                    for tb in range(4)]
            slv = wsl.tile([128, 8, 64], bf16, name="wvp", tag="wslab")
            nc.sync.dma_start(
                slv[:],
                w_att[:, 1088:1152].rearrange("(t p) m -> p t m", p=128))
            for tb in range(4):
                pv = ps5.tile([128, 64], f32, name="pv", tag="psx", bufs=2)
                for k in range(8):
                    nc.tensor.matmul(pv[:],
                                     x1n[k][:, 128 * tb:128 * (tb + 1)],
                                     slv[:, k, :], start=(k == 0),
                                     stop=(k == 7))
                nc.vector.tensor_copy(vloc[tb][:, 0:64], pv[:])
                nc.vector.memset(vloc[tb][:, 64:65], 1.0)
            dbg_dump("d_q0", qloc[0][:], [128, TOK], mybir.dt.bfloat16)

            b2_in = dram.tile([652, TOK], bf16, name="b2_in")
            b2_out = dram.tile([1304, TOK], bf16, name="b2_out")
            for i in range(4):
                nc.sync.dma_start(b2_in[128 * i:128 * (i + 1), :],
                                  qloc[4 + i][:])
            nc.sync.dma_start(b2_in[512:576, :], kloc[:])
            for tb in range(4):
                nc.sync.dma_start(
                    b2_in[576:641, 128 * tb:128 * (tb + 1)]
                    .rearrange("r c -> c r"), vloc[tb][:])
            nc.sync.dma_start(
                b2_in[644:652, 0:128].rearrange("f p -> p f"), x1pb[:])
            nc.gpsimd.collective_compute(
                "AllGather", OP.bypass, replica_groups=RG,
                ins=[b2_in.opt()], outs=[b2_out.opt()])

            def masked2(dst, local_ap, recv_ap, local_is_first):
                # dst/recv must share a base partition; local may be shifted.
                P = local_ap.shape[0]
                ma = is_first if local_is_first else is_second
                mb_ = is_second if local_is_first else is_first
                nc.vector.tensor_scalar(dst, local_ap, ma[0:P, :], None,
                                        op0=OP.mult)
                nc.vector.scalar_tensor_tensor(dst, recv_ap, mb_[0:P, :],
                                               dst, op0=OP.mult, op1=OP.add)

            qall = [p5.tile([64, T_], bf16, name=f"qall{h}")
                    for h in range(8)]
            kall = p5.tile([64, T_], bf16, name="kall")
            vall = [p5.tile([128, 65], bf16, name=f"vall{gb}")
                    for gb in range(8)]
            for h in range(8):
                t = h // 2
                ro = 128 * t + 64 * (h % 2)
                rows = slice(64 * (h % 2), 64 * (h % 2) + 64)
                for half in (0, 1):
                    rcv = p5.tile([64, TOK], bf16, name="qr", tag="qrcv",
                                  bufs=2)
                    nc.sync.dma_start(
                        rcv[:],
                        b2_out[652 * half + ro:652 * half + ro + 64, :])
                    masked2(qall[h][:, TOK * half:TOK * (half + 1)],
                            qloc[t][rows, :], rcv[:],
                            local_is_first=(half == 0))
            for half in (0, 1):
                rcv = p5.tile([64, TOK], bf16, name="kr", tag="krcv", bufs=2)
                nc.sync.dma_start(
                    rcv[:], b2_out[652 * half + 512:652 * half + 576, :])
                masked2(kall[:, TOK * half:TOK * (half + 1)], kloc[:],
                        rcv[:], local_is_first=(half == 0))
            for gb in range(8):
                half, tb = gb // 4, gb % 4
                rcv = p5.tile([128, 65], bf16, name="vr", tag="vrcv", bufs=2)
                nc.sync.dma_start(
                    rcv[:], b2_out[652 * half + 576:652 * half + 641,
                                   128 * tb:128 * (tb + 1)]
                    .rearrange("r c -> c r"))
                masked2(vall[gb][:], vloc[tb][:], rcv[:],
                        local_is_first=(half == 0))
            x1p = p5.tile([128, 8], bf16, name="x1p")
            rx = p5.tile([128, 8], bf16, name="rx")
            nc.sync.dma_start(
                rx[:], b2_out[644:652, 0:128].rearrange("f p -> p f"))
            nc.vector.tensor_scalar(x1p[:], rx[:], is_second, None,
                                    op0=OP.mult)
            dbg_dump("d_qall0", qall[0][:], [64, T_], mybir.dt.bfloat16)
            dbg_dump("d_kall", kall[:], [64, T_], mybir.dt.bfloat16)

            yall = [p5.tile([64, T_], bf16, name=f"yall{h}")
                    for h in range(8)]
            for h in range(8):
                for qb in range(2):
                    qcols = slice(TOK * qb, TOK * (qb + 1))
                    pav = ps5.tile([65, TOK], f32, name="pav", tag="pav",
                                   bufs=2)
                    nsb = 4 * (qb + 1)
                    for sb in range(nsb):
                        psx = ps5.tile([128, TOK], f32, name="psx",
                                       tag="psx", bufs=2)
                        nc.tensor.matmul(psx[:],
                                         kall[:, 128 * sb:128 * (sb + 1)],
                                         qall[h][:, qcols], start=True,
                                         stop=True)
                        r = sb - 4 * qb
                        if 0 <= r <= 3:
                            nc.vector.tensor_add(psx[:], psx[:],
                                                 amask[r][:])
                        pexp = p5.tile([128, TOK], bf16, name="pexp",
                                       tag="pexp", bufs=3)
                        nc.scalar.activation(pexp[:], psx[:], AF.Exp)
                        nc.tensor.matmul(pav[:], vall[sb][:], pexp[:],
                                         start=(sb == 0),
                                         stop=(sb == nsb - 1))
                    rc = p5.tile([1, TOK], f32, name="rcs", tag="rcs",
                                 bufs=2)
                    nc.vector.reciprocal(rc[:], pav[64:65, :])
                    rcb = p5.tile([64, TOK], f32, name="rcb", tag="rcb",
                                  bufs=2)
                    nc.gpsimd.partition_broadcast(rcb[:], rc[:],
                                                  channels=64)
                    nc.vector.tensor_mul(yall[h][:, qcols], pav[0:64, :],
                                         rcb[:])
            dbg_dump("d_yall0", yall[0][:], [64, T_], mybir.dt.bfloat16)

            # exchange 3 + proj rhs assembly (per-head base-0 builds)
            wph = [p5.tile([64, TOK + 1], bf16, name=f"wph{h}", tag="wph",
                           bufs=8) for h in range(8)]
            yown = [p5.tile([128, TOK + 1], bf16, name=f"yown{t}")
                    for t in range(4)]
            for h in range(8):
                t = h // 2
                rows = slice(64 * (h % 2), 64 * (h % 2) + 64)
                nc.vector.tensor_scalar(wph[h][:, :],
                                        yall[h][:, TOK - 1:T_],
                                        is_first[0:64, :], None,
                                        op0=OP.mult)
                nc.vector.scalar_tensor_tensor(
                    wph[h][:, 1:TOK + 1], yall[h][:, 0:TOK],
                    is_second[0:64, :], wph[h][:, 1:TOK + 1],
                    op0=OP.mult, op1=OP.add)
                yoh = p5.tile([64, TOK + 1], bf16, name="yoh", tag="yoh",
                              bufs=2)
                nc.vector.tensor_scalar(yoh[:, :],
                                        yall[h][:, TOK - 1:T_],
                                        is_second[0:64, :], None,
                                        op0=OP.mult)
                nc.vector.scalar_tensor_tensor(
                    yoh[:, 1:TOK + 1], yall[h][:, 0:TOK],
                    is_first[0:64, :], yoh[:, 1:TOK + 1],
                    op0=OP.mult, op1=OP.add)
                nc.vector.tensor_copy(yown[t][rows, :], yoh[:])
            b3_in = dram.tile([512, TOK + 1], bf16, name="b3_in")
            b3_out = dram.tile([1024, TOK + 1], bf16, name="b3_out")
            for h in range(8):
                nc.sync.dma_start(b3_in[64 * h:64 * (h + 1), :], wph[h][:])
            nc.gpsimd.collective_compute(
                "AllGather", OP.bypass, replica_groups=RG,
                ins=[b3_in.opt()], outs=[b3_out.opt()])

            yfull = yown + [p5.tile([128, TOK + 1], bf16, name=f"yfp{t}")
                            for t in range(4)]
            for t in range(4):
                r0 = p5.tile([128, TOK + 1], bf16, name="yr0", tag="yr0",
                             bufs=2)
                r1 = p5.tile([128, TOK + 1], bf16, name="yr1", tag="yr1",
                             bufs=2)
                nc.sync.dma_start(r0[:], b3_out[128 * t:128 * (t + 1), :])
                nc.sync.dma_start(
                    r1[:], b3_out[512 + 128 * t:512 + 128 * (t + 1), :])
                nc.vector.tensor_scalar(yfull[4 + t][:], r0[:], is_second,
                                        None, op0=OP.mult)
                nc.vector.scalar_tensor_tensor(yfull[4 + t][:], r1[:],
                                               is_first, yfull[4 + t][:],
                                               op0=OP.mult, op1=OP.add)

            x2 = [p5.tile([128, TOK], f32, name=f"x2_{i}")
                  for i in range(8)]
            x2p = resid.tile([128, 8], f32, name="x2p")
            for mb in range(8):
                sl = wslab(w_proj, 128 * mb, 128, 8, f"wpj{mb}")
                pp = ps5.tile([128, TOK], f32, name="pp", tag="pbig5",
                              bufs=2)
                pp1 = ps5.tile([128, 1], f32, name="pp1", tag="pp1", bufs=1)
                for k in range(8):
                    nc.tensor.matmul(pp[:], sl[:, k, :],
                                     yfull[k][:, 1:TOK + 1],
                                     start=(k == 0), stop=(k == 7))
                    nc.tensor.matmul(pp1[:], sl[:, k, :], yfull[k][:, 0:1],
                                     start=(k == 0), stop=(k == 7))
                x1l = p5.tile([128, TOK], f32, name="x1l", tag="x1l",
                              bufs=2)
                nc.sync.dma_start(x1l[:], x1_d[128 * mb:128 * (mb + 1), :])
                nc.vector.scalar_tensor_tensor(x2[mb][:], x1l[:], 1.0,
                                               pp[:], op0=OP.mult,
                                               op1=OP.add)
                nc.sync.dma_start(x2_d[128 * mb:128 * (mb + 1), :],
                                  x2[mb][:])
                tpv = p5.tile([128, 1], f32, name="tpv", tag="tpv", bufs=2)
                nc.vector.tensor_add(tpv[:], x1p[:, mb:mb + 1], pp1[:])
                nc.vector.tensor_scalar(x2p[:, mb:mb + 1], tpv[:],
                                        is_second, None, op0=OP.mult)
            dbg_dump("d_x2_0", x2[0][:], [128, TOK])
            ps5s.close()
            st5.close()
            stD.close()

        if PHASES >= 6:
            # ================= P6: cmix =================
            st6 = stack()
            p6 = st6.enter_context(tc.tile_pool(name="p6", bufs=1))
            ps6s = stack()
            ps6 = ps6s.enter_context(tc.tile_pool(name="ps6", bufs=1,
                                                  space="PSUM"))
            x2l = [p6.tile([128, TOK], f32, name=f"x2l{i}")
                   for i in range(8)]
            for i in range(8):
                nc.sync.dma_start(x2l[i][:],
                                  x2_d[128 * i:128 * (i + 1), :])
            z3 = [p6.tile([128, TOK + 1], bf16, name=f"z3_{i}")
                  for i in range(8)]
            rmsnorm_cm([x2l[i][:] for i in range(8)],
                       [z3[i][:, 1:TOK + 1] for i in range(8)], TOK, p6,
                       ps6, C_, "n2")
            sqp = p6.tile([128, 8], bf16, name="sqp")
            nc.vector.tensor_mul(sqp[:], x2p[:], x2p[:])
            psp = ps6.tile([1, 8], f32, name="psp", tag="psp", bufs=1)
            nc.tensor.matmul(psp[:], onesb[:], sqp[:], start=True,
                             stop=True)
            ssp = p6.tile([1, 1], f32, name="ssp")
            nc.vector.tensor_reduce(ssp[:], psp[:],
                                    axis=mybir.AxisListType.X, op=OP.add)
            nc.scalar.activation(ssp[:], ssp[:], AF.Sqrt,
                                 bias=eps_c[0:1, :], scale=1.0 / C_)
            nc.vector.reciprocal(ssp[:], ssp[:])
            rpb = p6.tile([128, 1], f32, name="rpb")
            nc.gpsimd.partition_broadcast(rpb[:], ssp[:], channels=128)
            for i in range(8):
                nc.vector.scalar_tensor_tensor(z3[i][:, 0:1],
                                               x2p[:, i:i + 1], 1.0,
                                               rpb[:], op0=OP.mult,
                                               op1=OP.mult)
            dbg_dump("d_z3_0", z3[0][:], [128, TOK + 1], mybir.dt.bfloat16)

            xk = [p6.tile([128, TOK], bf16, name=f"xk{i}")
                  for i in range(8)]
            xr = [p6.tile([128, TOK], bf16, name=f"xr{i}")
                  for i in range(8)]
            for i in range(8):
                nc.vector.tensor_scalar(xk[i][:], z3[i][:, 1:TOK + 1],
                                        mk1_s[:, i:i + 1], None,
                                        op0=OP.mult)
                nc.vector.scalar_tensor_tensor(xk[i][:], z3[i][:, 0:TOK],
                                               mk_s[:, i:i + 1], xk[i][:],
                                               op0=OP.mult, op1=OP.add)
                nc.vector.tensor_scalar(xr[i][:], z3[i][:, 1:TOK + 1],
                                        mr1_s[:, i:i + 1], None,
                                        op0=OP.mult)
                nc.vector.scalar_tensor_tensor(xr[i][:], z3[i][:, 0:TOK],
                                               mr_s[:, i:i + 1], xr[i][:],
                                               op0=OP.mult, op1=OP.add)

            r_sb = [p6.tile([128, TOK], bf16, name=f"r_sb{i}")
                    for i in range(8)]
            for mb in range(8):
                sl = wslab(w_rec, 128 * mb, 128, 8, f"wrc{mb}")
                pr = ps6.tile([128, TOK], f32, name="pr", tag="pbig6",
                              bufs=3)
                for k in range(8):
                    nc.tensor.matmul(pr[:], sl[:, k, :], xr[k][:],
                                     start=(k == 0), stop=(k == 7))
                nc.scalar.activation(r_sb[mb][:], pr[:], AF.Sigmoid)
            kE = [p6.tile([128, TOK], bf16, name=f"kE{i}")
                  for i in range(32)]
            for mb in range(32):
                sl = wslab(w_key, 128 * mb, 128, 8, f"wky{mb}")
                pky = ps6.tile([128, TOK], f32, name="pky", tag="pbig6",
                               bufs=3)
                for k in range(8):
                    nc.tensor.matmul(pky[:], sl[:, k, :], xk[k][:],
                                     start=(k == 0), stop=(k == 7))
                nc.scalar.activation(kE[mb][:], pky[:], AF.Erf,
                                     scale=1.0 / _DEN, bias=erfb_c[:, :])
            dbg_dump("d_kE0", kE[0][:], [128, TOK], mybir.dt.bfloat16)
            dbg_dump("d_r0", r_sb[0][:], [128, TOK], mybir.dt.bfloat16)

            for mb in range(8):
                slab = wslab(w_val, 128 * mb, 128, 32, f"wvl{mb}", pool=p6,
                             tag="wslab_v", bufs=2)
                pvv = ps6.tile([128, TOK], f32, name="pvv", tag="pbig6",
                               bufs=3)
                for k in range(32):
                    nc.tensor.matmul(pvv[:], slab[:, k, :], kE[k][:],
                                     start=(k == 0), stop=(k == 31))
                tmpv = p6.tile([128, TOK], f32, name="tmpv", tag="tmpv",
                               bufs=2)
                nc.vector.tensor_scalar(tmpv[:], pvv[:],
                                        vbias_s[:, mb:mb + 1], None,
                                        op0=OP.add)
                nc.vector.tensor_mul(tmpv[:], tmpv[:], r_sb[mb][:])
                outt = p6.tile([128, TOK], f32, name="outt", tag="outt",
                               bufs=2)
                nc.vector.tensor_add(outt[:], x2l[mb][:], tmpv[:])
                nc.sync.dma_start(out_d[128 * mb:128 * (mb + 1), :],
                                  outt[:])
            ps6s.close()
            st6.close()

        for s in reversed(_open):
            s.close()
        whole.close()

    nc.compile()
    return nc, dbg_outs


# ================= host glue =================

def _prep_inputs(x, in_proj_w, conv_w, conv_b, dt_bias, A_log, D, mnorm_w,
                 out_proj_w, attn_w, proj_w, time_maa_k, time_maa_r, key_w,
                 recept_w, value_w):
    f32 = np.float32

    def b(a):
        return np.ascontiguousarray(np.asarray(a, f32).astype(BF16))

    x = np.asarray(x, f32)
    shared = {
        "w_inproj": b(in_proj_w),
        "convw": np.ascontiguousarray(
            np.asarray(conv_w, f32).reshape(17, 128, DCONV)
            .transpose(1, 0, 2)),
        "convb": np.ascontiguousarray(
            np.asarray(conv_b, f32).reshape(17, 128).T),
        "dtb": np.ascontiguousarray(
            np.asarray(dt_bias, f32).reshape(NHM, 1)),
        "aneg": np.ascontiguousarray(
            (-np.exp(np.asarray(A_log, f32))).reshape(NHM, 1)),
        # drep[p, k] = D[2k + (p >= 64)]
        "drep": np.ascontiguousarray(np.stack(
            [np.concatenate([np.full(64, D2[0]), np.full(64, D2[1])])
             for D2 in np.asarray(D, f32).reshape(16, 2)], axis=1)
            .astype(f32)),
        "mnw": np.ascontiguousarray(
            np.asarray(mnorm_w, f32).reshape(16, 128).T),
        "w_outproj": b(out_proj_w),
        "mk": np.ascontiguousarray(
            np.asarray(time_maa_k, f32).reshape(8, 128).T),
        "mk1": np.ascontiguousarray(
            (1.0 - np.asarray(time_maa_k, f32)).reshape(8, 128).T),
        "mr": np.ascontiguousarray(
            np.asarray(time_maa_r, f32).reshape(8, 128).T),
        "mr1": np.ascontiguousarray(
            (1.0 - np.asarray(time_maa_r, f32)).reshape(8, 128).T),
        "w_key": b(key_w),
        "w_val": b(0.5 * np.asarray(value_w, f32)),
        "vbias": np.ascontiguousarray(
            (0.5 * np.asarray(value_w, f32).sum(0)).reshape(8, 128).T),
        "w_rec": b(recept_w),
    }
    ef = np.zeros((NHM, DIN), f32)
    for k in range(16):
        ef[2 * k, 128 * k:128 * k + 64] = 1.0
        ef[2 * k + 1, 128 * k + 64:128 * k + 128] = 1.0
    shared["efull"] = ef

    attn_w = np.asarray(attn_w, f32)
    proj_w = np.asarray(proj_w, f32)
    scale = 1.0 / np.sqrt(np.float32(HD))
    in_maps = []
    for core in range(N_CORES):
        bi, half = core // 2, core % 2
        start = half * TOK
        xcm = x[bi].T
        xs = np.zeros((C_, TH), f32)
        xs[:, 3:] = xcm[:, start:start + TOK]
        if start >= 3:
            xs[:, 0:3] = xcm[:, start - 3:start]
        myh = np.arange(8 * half, 8 * half + 8)
        oth = np.arange(8 * (1 - half), 8 * (1 - half) + 8)
        qcols = attn_w[:, :C_].reshape(C_, NH, HD)
        wq_perm = np.concatenate(
            [qcols[:, myh].reshape(C_, 512),
             qcols[:, oth].reshape(C_, 512)], axis=1) * scale
        w_att_c = np.concatenate([wq_perm, attn_w[:, C_:]], axis=1)
        prows = proj_w.reshape(NH, HD, C_)
        w_proj_c = np.concatenate(
            [prows[myh].reshape(512, C_), prows[oth].reshape(512, C_)],
            axis=0)
        mskc = np.zeros((128, 2), f32)
        mskc[:, 0] = 1.0 - half
        mskc[:, 1] = half
        m = dict(shared)
        m["xin"] = np.ascontiguousarray(xs)
        m["w_att"] = np.ascontiguousarray(w_att_c.astype(BF16))
        m["w_proj"] = np.ascontiguousarray(w_proj_c.astype(BF16))
        m["msk"] = mskc
        in_maps.append(m)
    return in_maps


def kernel(**inputs):
    from concourse.bass_utils import run_bass_kernel_spmd

    if "nc" not in _CACHE:
        _CACHE["nc"], _CACHE["dbg"] = _build()
    nc = _CACHE["nc"]
    in_maps = _prep_inputs(**inputs)
    res = run_bass_kernel_spmd(nc, in_maps, core_ids=list(range(N_CORES)))
    _CACHE["results"] = res
    out = np.empty((B_, T_, C_), np.float32)
    for core in range(N_CORES):
        bi, half = core // 2, core % 2
        out[bi, half * TOK:(half + 1) * TOK, :] = \
            np.asarray(res.results[core]["out"], np.float32).T
    return out


# revision 19
# speedup vs baseline: 1.0611x; 1.0611x over previous
"""nn_Block_21062519619681 fully on-device: hybrid Mamba2 + MQA + RWKV-CMix
block as ONE Bass/Tile SPMD kernel on 8 trn2 NeuronCores.

Sharding: 8 cores = 4 batches x 2 token-halves (512 own tokens/core).
 - mamba: token-sharded; chunked-SSD scan (L=128); cross-half state carry via
   a pairwise AllGather applied as a linear correction pass.
 - attention: q-head-split (8 heads/core over ALL 1024 tokens; per-core
   permuted q/proj weights keep the SPMD graph rank-uniform); k/v + q halves
   exchanged via pairwise AllGather; softmax without max-subtraction (scores
   bounded); colsum ridden as a ones-column in the av matmul.
 - cmix: token-sharded, replicated weights, erf/sigmoid fused into PSUM evac.
All matmuls bf16 (weights pre-cast on host), fp32 PSUM accumulate, fp32
residual stream. Rank-dependent selection uses host-fed 0/1 masks (masked
sums) - the instruction graph is identical on all cores.
"""
import os
import sys

sys.path.insert(0, "/opt/trn_rl_repo")
import numpy as np
import ml_dtypes

B_, T_, C_ = 4, 1024, 1024
NH, HD = 16, 64
DS, DCONV, EXP, PHD = 64, 4, 2, 64
DIN = EXP * C_
NHM = DIN // PHD
CONVD = DIN + 2 * DS
FFN = 4 * C_
EPS = 1e-5
N_CORES = 8
TOK = 512
TH = TOK + 3
L = 128
NCH = TOK // L
NEG = -1e30

BF16 = ml_dtypes.bfloat16
DEBUG = bool(int(os.environ.get("BASSK_DEBUG", "0")))
PHASES = int(os.environ.get("BASSK_PHASES", "6"))

_CACHE = {}


def _build():
    import contextlib
    import concourse.mybir as mybir
    import concourse.bacc as bacc
    import concourse.tile as tile
    from concourse.masks import make_identity

    f32 = mybir.dt.float32
    bf16 = mybir.dt.bfloat16
    AF = mybir.ActivationFunctionType
    OP = mybir.AluOpType

    nc = bacc.Bacc("TRN2", target_bir_lowering=False, debug=False,
                   num_devices=N_CORES)

    def din(name, shape, dt=bf16):
        return nc.dram_tensor(name, shape, dt, kind="ExternalInput").ap()

    xin = din("xin", [C_, TH], f32)
    w_inproj = din("w_inproj", [C_, 4256])
    convw = din("convw", [128, 17, DCONV], f32)
    convb = din("convb", [128, 17], f32)
    dtb = din("dtb", [NHM, 1], f32)
    aneg = din("aneg", [NHM, 1], f32)
    drep = din("drep", [128, 16], f32)
    mnw = din("mnw", [128, 16], f32)
    w_outproj = din("w_outproj", [DIN, C_])
    w_att = din("w_att", [C_, 1024 + 128])
    w_proj = din("w_proj", [C_, C_])
    mk = din("mk", [128, 8], f32)
    mk1 = din("mk1", [128, 8], f32)
    mr = din("mr", [128, 8], f32)
    mr1 = din("mr1", [128, 8], f32)
    w_key = din("w_key", [C_, FFN])
    w_val = din("w_val", [FFN, C_])
    vbias = din("vbias", [128, 8], f32)
    w_rec = din("w_rec", [C_, C_])
    msk = din("msk", [128, 2], f32)
    efull = din("efull", [NHM, DIN])

    out_d = nc.dram_tensor("out", [C_, TOK], f32, kind="ExternalOutput").ap()

    dbg_outs = {}

    def dbg_dump(name, ap_or_tile, shape, dt=None):
        if not DEBUG:
            return
        d = nc.dram_tensor(name, shape, dt or mybir.dt.float32,
                           kind="ExternalOutput").ap()
        dbg_outs[name] = d
        nc.sync.dma_start(d, ap_or_tile)

    RG = [[0, 1], [2, 3], [4, 5], [6, 7]]

    with tile.TileContext(nc) as tc:
        _open = []

        def stack():
            s = contextlib.ExitStack()
            _open.append(s)
            return s

        whole = contextlib.ExitStack()
        consts = whole.enter_context(tc.tile_pool(name="consts", bufs=1))
        resid = whole.enter_context(tc.tile_pool(name="resid", bufs=1))
        wsl = whole.enter_context(tc.tile_pool(name="wsl", bufs=3))
        dram = whole.enter_context(tc.tile_pool(name="dram", bufs=1,
                                                space="DRAM"))

        # ---------------- constants ----------------
        ident_b = consts.tile([128, 128], bf16, name="ident_b")
        make_identity(nc, ident_b)
        ident_f = consts.tile([128, 128], f32, name="ident_f")
        make_identity(nc, ident_f)
        tri01 = consts.tile([128, 128], bf16, name="tri01")
        nc.vector.memset(tri01, 1.0)
        nc.gpsimd.affine_select(out=tri01, in_=tri01, compare_op=OP.is_ge,
                                fill=0.0, base=0, channel_multiplier=-1,
                                pattern=[[1, 128]])
        onesb = consts.tile([128, 1], bf16, name="onesb")
        nc.vector.memset(onesb, 1.0)
        onesf_r = consts.tile([1, 64], f32, name="onesf_r")
        nc.vector.memset(onesf_r, 1.0)
        ones32 = consts.tile([NHM, L], f32, name="ones32")
        nc.vector.memset(ones32, 1.0)
        eps_c = consts.tile([128, 1], f32, name="eps_c")
        nc.vector.memset(eps_c, EPS)
        _MU = float(np.sqrt(0.5))
        _DEN = float(np.sqrt(1.0 / (4.0 * np.pi)) * np.sqrt(2.0))
        erfb_c = consts.tile([128, 1], f32, name="erfb_c")
        nc.vector.memset(erfb_c, -_MU / _DEN)

        def cin(name, shape, src, dt=f32):
            t = consts.tile(list(shape), dt, name=name)
            nc.sync.dma_start(t[:], src)
            return t

        convw_s = cin("convw_s", [128, 17, DCONV], convw)
        convb_s = cin("convb_s", [128, 17], convb)
        dtb_s = cin("dtb_s", [NHM, 1], dtb)
        aneg_s = cin("aneg_s", [NHM, 1], aneg)
        drep_s = cin("drep_s", [128, 16], drep)
        mnw_s = cin("mnw_s", [128, 16], mnw)
        mk_s = cin("mk_s", [128, 8], mk)
        mk1_s = cin("mk1_s", [128, 8], mk1)
        mr_s = cin("mr_s", [128, 8], mr)
        mr1_s = cin("mr1_s", [128, 8], mr1)
        vbias_s = cin("vbias_s", [128, 8], vbias)
        msk_s = cin("msk_s", [128, 2], msk)
        efull_s = cin("efull_s", [NHM, DIN], efull, dt=bf16)
        is_first = msk_s[:, 0:1]
        is_second = msk_s[:, 1:2]

        zsil_d = dram.tile([DIN, TOK], bf16, name="zsil_d")
        x1_d = dram.tile([C_, TOK], f32, name="x1_d")
        x2_d = dram.tile([C_, TOK], f32, name="x2_d")

        def wslab(wt, m0, mw, kt, name, pool=None, tag="wslab", bufs=None,
                  r0=0):
            s = (pool or wsl).tile([128, kt, mw], bf16, name=name, tag=tag,
                                   bufs=bufs)
            nc.sync.dma_start(
                s[:], wt[r0:r0 + 128 * kt, m0:m0 + mw]
                .rearrange("(t p) m -> p t m", p=128))
            return s

        def rmsnorm_cm(src_aps, dst_aps, width, pool, psp, nfeat, tag):
            ssq = psp.tile([1, width], f32, name=f"ssq_{tag}",
                           tag=f"ssq{tag}", bufs=1)
            n = len(src_aps)
            for i, sap in enumerate(src_aps):
                sq = pool.tile([128, width], bf16, name=f"sq_{tag}",
                               tag=f"sq{tag}", bufs=2)
                nc.vector.tensor_mul(sq[:], sap, sap)
                nc.tensor.matmul(ssq[:], onesb[:], sq[:], start=(i == 0),
                                 stop=(i == n - 1))
            rms = pool.tile([1, width], f32, name=f"rms_{tag}",
                            tag=f"rms{tag}", bufs=1)
            nc.scalar.activation(rms[:], ssq[:], AF.Sqrt,
                                 bias=eps_c[0:1, :], scale=1.0 / nfeat)
            rinv = pool.tile([1, width], f32, name=f"rinv_{tag}",
                             tag=f"rinv{tag}", bufs=1)
            nc.vector.reciprocal(rinv[:], rms[:])
            rbc = pool.tile([128, width], f32, name=f"rbc_{tag}",
                            tag=f"rbc{tag}", bufs=1)
            nc.gpsimd.partition_broadcast(rbc[:], rinv[:], channels=128)
            for i, sap in enumerate(src_aps):
                nc.vector.tensor_mul(dst_aps[i], sap, rbc[:])

        # pool nesting (open early -> close late):
        stD = stack()
        pD = stD.enter_context(tc.tile_pool(name="pD", bufs=1))   # ..P5
        stY = stack()
        pY = stY.enter_context(tc.tile_pool(name="pY", bufs=1))   # ..P4
        stB = stack()
        pB = stB.enter_context(tc.tile_pool(name="pB", bufs=1))   # ..P3
        stC = stack()
        pC = stC.enter_context(tc.tile_pool(name="pC", bufs=1))   # ..P3

        # ================= P0 + P1: rmsnorm + in_proj =================
        st01 = stack()
        pA = st01.enter_context(tc.tile_pool(name="pA", bufs=1))
        ps01 = st01.enter_context(tc.tile_pool(name="ps01", bufs=1,
                                               space="PSUM"))
        xn = [pA.tile([128, TH], bf16, name=f"xn{i}") for i in range(8)]
        # streaming rmsnorm over x (full TH width, stats on own 512 cols)
        ssqx = ps01.tile([1, 512], f32, name="ssqx", tag="ssqx", bufs=1)
        ssqh = ps01.tile([1, 3], f32, name="ssqh", tag="ssqh", bufs=1)
        for i in range(8):
            xt = pA.tile([128, TH], f32, name="xt", tag="xt", bufs=3)
            nc.sync.dma_start(xt[:], xin[128 * i:128 * (i + 1), :])
            sqx = pA.tile([128, TH], bf16, name="sqx", tag="sqx", bufs=2)
            nc.vector.tensor_mul(sqx[:], xt[:], xt[:])
            nc.tensor.matmul(ssqx[:], onesb[:], sqx[:, 3:TH],
                             start=(i == 0), stop=(i == 7))
            nc.tensor.matmul(ssqh[:], onesb[:], sqx[:, 0:3],
                             start=(i == 0), stop=(i == 7))
        rmsx = pA.tile([1, TH], f32, name="rmsx")
        nc.scalar.activation(rmsx[:, 3:TH], ssqx[:], AF.Sqrt,
                             bias=eps_c[0:1, :], scale=1.0 / C_)
        nc.scalar.activation(rmsx[:, 0:3], ssqh[:], AF.Sqrt,
                             bias=eps_c[0:1, :], scale=1.0 / C_)
        rinvx = pA.tile([1, TH], f32, name="rinvx")
        nc.vector.reciprocal(rinvx[:], rmsx[:])
        rbcx = pA.tile([128, TH], f32, name="rbcx")
        nc.gpsimd.partition_broadcast(rbcx[:], rinvx[:], channels=128)
        for i in range(8):
            xt = pA.tile([128, TH], f32, name="xt", tag="xt", bufs=3)
            nc.sync.dma_start(xt[:], xin[128 * i:128 * (i + 1), :])
            nc.vector.tensor_mul(xn[i][:], xt[:], rbcx[:])
        dbg_dump("d_xn0", xn[0][:], [128, TH], mybir.dt.bfloat16)

        xbc = [pB.tile([128, TH], bf16, name=f"xbc{i}") for i in range(17)]
        dtraw = pB.tile([NHM, TOK], f32, name="dtraw")

        for mb in range(16):
            sl = wslab(w_inproj, 128 * mb, 128, 8, f"wz{mb}")
            pz = ps01.tile([128, TOK], f32, name="pz", tag="pbig", bufs=3)
            for k in range(8):
                nc.tensor.matmul(pz[:], sl[:, k, :], xn[k][:, 3:TH],
                                 start=(k == 0), stop=(k == 7))
            zst = pA.tile([128, TOK], bf16, name="zst", tag="zst", bufs=3)
            nc.scalar.activation(zst[:], pz[:], AF.Silu)
            nc.sync.dma_start(zsil_d[128 * mb:128 * (mb + 1), :], zst[:])
        for mb in range(17):
            sl = wslab(w_inproj, DIN + 128 * mb, 128, 8, f"wxbc{mb}")
            pb_ = ps01.tile([128, TOK], f32, name="pb", tag="pbig", bufs=3)
            ph = ps01.tile([128, 3], f32, name="ph", tag="phalo", bufs=2)
            for k in range(8):
                nc.tensor.matmul(pb_[:], sl[:, k, :], xn[k][:, 3:TH],
                                 start=(k == 0), stop=(k == 7))
                nc.tensor.matmul(ph[:], sl[:, k, :], xn[k][:, 0:3],
                                 start=(k == 0), stop=(k == 7))
            nc.scalar.copy(xbc[mb][:, 3:TH], pb_[:])
            nc.vector.tensor_copy(xbc[mb][:, 0:3], ph[:])
        sl = wslab(w_inproj, 4224, 32, 8, "wdtp")
        pdt = ps01.tile([NHM, TOK], f32, name="pdt", tag="pdt", bufs=1)
        for k in range(8):
            nc.tensor.matmul(pdt[:], sl[:, k, :], xn[k][:, 3:TH],
                             start=(k == 0), stop=(k == 7))
        nc.vector.tensor_copy(dtraw[:], pdt[:])
        dbg_dump("d_xbc0", xbc[0][:], [128, TH], mybir.dt.bfloat16)
        st01.close()

        if PHASES >= 2:
            # ============ P2: conv + dt pipeline + transposes ============
            ps2 = stack()
            ps2p = ps2.enter_context(tc.tile_pool(name="ps2", bufs=1,
                                                  space="PSUM"))
            xs_cm = [pC.tile([128, TOK], bf16, name=f"xs_cm{i}")
                     for i in range(17)]
            for i in range(17):
                tmp = pC.tile([128, TOK], f32, name="ctmp", tag="ctmp",
                              bufs=3)
                nc.vector.tensor_scalar(tmp[:], xbc[i][:, 0:TOK],
                                        convw_s[:, i, 0:1], None,
                                        op0=OP.mult)
                for j in range(1, DCONV):
                    nc.vector.scalar_tensor_tensor(
                        tmp[:], xbc[i][:, j:j + TOK], convw_s[:, i, j:j + 1],
                        tmp[:], op0=OP.mult, op1=OP.add)
                nc.scalar.activation(xs_cm[i][:], tmp[:], AF.Silu,
                                     bias=convb_s[:, i:i + 1])
            C_cm = pC.tile([64, TOK], bf16, name="C_cm")
            nc.vector.tensor_copy(C_cm[:], xs_cm[16][64:128, :])
            dbg_dump("d_xs0", xs_cm[0][:], [128, TOK], mybir.dt.bfloat16)

            dt_f = pC.tile([NHM, TOK], f32, name="dt_f")
            Lc = pC.tile([NHM, TOK], f32, name="Lc")
            wdt = pC.tile([NHM, TOK], f32, name="wdt", tag="scr", bufs=2)
            u = pC.tile([NHM, TOK], f32, name="u")
            ex = pC.tile([NHM, TOK], f32, name="ex", tag="scr", bufs=2)
            nc.vector.tensor_scalar(u[:], dtraw[:], dtb_s[:], None,
                                    op0=OP.add)
            ab = pC.tile([NHM, TOK], f32, name="ab", tag="scr", bufs=2)
            nc.vector.tensor_scalar(ab[:], u[:], -1.0, None, op0=OP.mult)
            nc.vector.tensor_max(ab[:], ab[:], u[:])
            nc.scalar.activation(ex[:], ab[:], AF.Exp, scale=-1.0)
            nc.scalar.activation(ex[:], ex[:], AF.Ln, bias=1.0)
            nc.vector.tensor_scalar(dt_f[:], u[:], 0.0, None, op0=OP.max)
            nc.vector.tensor_add(dt_f[:], dt_f[:], ex[:])
            dta = u
            nc.vector.tensor_scalar(dta[:], dt_f[:], aneg_s[:], None,
                                    op0=OP.mult)
            for c in range(NCH):
                cs = slice(L * c, L * (c + 1))
                nc.vector.tensor_tensor_scan(Lc[:, cs], ones32[:],
                                             dta[:, cs], 0.0, op0=OP.mult,
                                             op1=OP.add)
                nc.scalar.activation(wdt[:, cs], Lc[:, cs], AF.Exp,
                                     scale=-1.0,
                                     bias=Lc[:, L * (c + 1) - 1:L * (c + 1)])
            nc.vector.tensor_mul(wdt[:], wdt[:], dt_f[:])

            dbg_dump("d_dt", dt_f[:], [NHM, TOK])
            dbg_dump("d_Lc", Lc[:], [NHM, TOK])

            eLcE = pC.tile([NHM, NCH], bf16, name="eLcE")
            lce = pC.tile([NHM, NCH], f32, name="lce")
            for c in range(NCH):
                nc.vector.tensor_copy(lce[:, c:c + 1],
                                      Lc[:, L * (c + 1) - 1:L * (c + 1)])
            nc.scalar.activation(eLcE[:], lce[:], AF.Exp)
            arep = [pC.tile([128, NCH], f32, name=f"arep{k}")
                    for k in range(16)]
            carep = [pC.tile([128, NCH], f32, name=f"carep{k}")
                     for k in range(16)]
            for k in range(16):
                pa = ps2p.tile([128, NCH], f32, name="pa", tag="pa", bufs=2)
                nc.tensor.matmul(pa[:], efull_s[:, 128 * k:128 * (k + 1)],
                                 eLcE[:], start=True, stop=True)
                nc.vector.tensor_copy(arep[k][:], pa[:])
                nc.vector.memset(carep[k][:, 0:1], 1.0)
                for c in range(1, NCH):
                    nc.vector.tensor_mul(carep[k][:, c:c + 1],
                                         carep[k][:, c - 1:c],
                                         arep[k][:, c - 1:c])

            tmv = [pC.tile([128, 96], f32, name=f"tmv{c}")
                   for c in range(NCH)]
            stk = pC.tile([96, TOK], f32, name="stk")
            nc.vector.tensor_scalar(stk[0:NHM, :], Lc[:], -1.0, None,
                                    op0=OP.mult)
            nc.vector.tensor_copy(stk[NHM:2 * NHM, :], dt_f[:])
            nc.vector.tensor_copy(stk[2 * NHM:3 * NHM, :], wdt[:])
            for c in range(NCH):
                pt = ps2p.tile([128, 96], f32, name="pt", tag="ptr", bufs=2)
                nc.tensor.transpose(pt[:], stk[:, L * c:L * (c + 1)],
                                    ident_f[0:96, 0:96])
                nc.vector.tensor_copy(tmv[c][:], pt[:])

            xs_tm = [pC.tile([128, 2176], bf16, name=f"xs_tm{c}")
                     for c in range(NCH)]
            for c in range(NCH):
                for i in range(17):
                    ptb = ps2p.tile([128, 128], bf16, name="ptb", tag="ptrb",
                                    bufs=3)
                    nc.tensor.transpose(ptb[:],
                                        xs_cm[i][:, L * c:L * (c + 1)],
                                        ident_b[:])
                    nc.vector.tensor_copy(
                        xs_tm[c][:, 128 * i:128 * (i + 1)], ptb[:])
            dbg_dump("d_xstm0", xs_tm[0][:], [128, 2176], mybir.dt.bfloat16)
            ps2.close()

        if PHASES >= 3:
            # ========== P3: scan (interleaved per chunk) ==========
            ps3 = stack()
            ps3p = ps3.enter_context(tc.tile_pool(name="ps3", bufs=1,
                                                  space="PSUM"))
            state = [pC.tile([128, PHD], f32, name=f"state{k}")
                     for k in range(16)]
            for k in range(16):
                nc.vector.memset(state[k][:], 0.0)
            stateb = [pC.tile([64, PHD], bf16, name=f"stateb{h}")
                      for h in range(NHM)]
            ycm = [pY.tile([128, TOK], bf16, name=f"ycm{k}")
                   for k in range(16)]

            def stage_bcast(lcf, hh, with_exp=True, channels=128):
                lba = pC.tile([channels, 16 * L], f32, name="lba",
                              tag="lball", bufs=1)
                nc.gpsimd.partition_broadcast(
                    lba[:], lcf[0:1, 16 * L * hh:16 * L * (hh + 1)],
                    channels=channels)
                eba = None
                if with_exp:
                    eba = pC.tile([64, 16 * L], bf16, name="eba",
                                  tag="eball", bufs=1)
                    nc.scalar.activation(eba[:], lba[0:64, :], AF.Exp)
                return lba, eba

            def make_cdec_dve(eba, h, cs):
                off = L * (h % 16)
                cd = pC.tile([64, L], bf16, name="cd", tag="cdec", bufs=4)
                nc.vector.tensor_mul(cd[:], C_cm[:, cs],
                                     eba[:, off:off + L])
                return cd

            def make_cdec(eba, h, cs):
                off = L * (h % 16)
                cd = pC.tile([64, L], bf16, name="cd", tag="cdec", bufs=4)
                nc.gpsimd.tensor_mul(cd[:], C_cm[:, cs],
                                     eba[:, off:off + L])
                return cd

            def stage_lc(c):
                t = pC.tile([1, NHM * L], f32, name=f"LcFc{c}",
                            tag="lcf", bufs=2)
                nc.sync.dma_start(t[0:1, :], Lc[:, L * c:L * (c + 1)])
                return t

            for c in range(NCH):
                cs = slice(L * c, L * (c + 1))
                lcf = stage_lc(c)
                if c > 0:
                    for h in range(NHM):
                        nc.gpsimd.tensor_copy(
                            stateb[h][:],
                            state[h // 2][64 * (h % 2):64 * (h % 2) + 64, :])
                pg = ps3p.tile([128, L], f32, name="pg", tag="pg", bufs=1)
                nc.tensor.matmul(pg[:], xs_cm[16][0:64, cs], C_cm[:, cs],
                                 start=True, stop=True)
                gts = pC.tile([128, L], bf16, name="gts", tag="gts", bufs=2)
                nc.vector.tensor_mul(gts[:], pg[:], tri01[:])
                lba = eba = None
                for h in range(NHM):
                    k = h // 2
                    rows = slice(64 * (h % 2), 64 * (h % 2) + 64)
                    if h % 16 == 0:
                        lba, eba = stage_bcast(lcf, h // 16,
                                               with_exp=(c > 0))
                    darg = pC.tile([128, L], f32, name="darg", tag="darg",
                                   bufs=4)
                    nc.vector.tensor_scalar(darg[:],
                                            lba[:, L * (h % 16):
                                                L * (h % 16) + L],
                                            tmv[c][:, h:h + 1], 0.0,
                                            op0=OP.add, op1=OP.min)
                    expd = pC.tile([128, L], f32, name="expd", tag="expd",
                                   bufs=4)
                    nc.scalar.activation(expd[:], darg[:], AF.Exp)
                    mt = pC.tile([128, L], bf16, name="mt", tag="mt", bufs=4)
                    nc.vector.scalar_tensor_tensor(
                        mt[:], gts[:], tmv[c][:, 32 + h:33 + h], expd[:],
                        op0=OP.mult, op1=OP.mult)
                    py = ps3p.tile([64, L], f32, name="py", tag="py", bufs=2)
                    nc.tensor.matmul(py[:],
                                     xs_tm[c][:, PHD * h:PHD * (h + 1)],
                                     mt[:], start=True, stop=(c == 0))
                    if c > 0:
                        cd = make_cdec(eba, h, cs)
                        nc.tensor.matmul(py[:], stateb[h][:], cd[:],
                                         start=False, stop=True)
                    nc.vector.scalar_tensor_tensor(
                        ycm[k][rows, cs], xs_cm[k][rows, cs],
                        drep_s[rows, k:k + 1], py[:], op0=OP.mult,
                        op1=OP.add)
                    bw = pC.tile([128, DS], bf16, name="bw", tag="bw",
                                 bufs=3)
                    nc.gpsimd.tensor_scalar(
                        bw[:], xs_tm[c][:, DIN:DIN + DS],
                        tmv[c][:, 64 + h:65 + h], None, op0=OP.mult)
                    psc = ps3p.tile([64, PHD], f32, name="psc", tag="psc",
                                    bufs=2)
                    nc.tensor.matmul(psc[:], bw[:],
                                     xs_tm[c][:, PHD * h:PHD * (h + 1)],
                                     start=True, stop=True)
                    nc.vector.scalar_tensor_tensor(
                        state[k][rows, :], state[k][rows, :],
                        arep[k][rows, c:c + 1], psc[:], op0=OP.mult,
                        op1=OP.add)

            b1_in = dram.tile([128, 16 * PHD], bf16, name="b1_in")
            b1_out = dram.tile([256, 16 * PHD], bf16, name="b1_out")
            steb = pC.tile([128, 16 * PHD], bf16, name="steb")
            for k in range(16):
                nc.vector.tensor_copy(steb[:, PHD * k:PHD * (k + 1)],
                                      state[k][:])
            nc.sync.dma_start(b1_in[:], steb[:])
            nc.gpsimd.collective_compute(
                "AllGather", OP.bypass, replica_groups=RG,
                ins=[b1_in.opt()], outs=[b1_out.opt()])
            dbg_dump("d_st0", state[0][:], [128, PHD])

            h0bf2 = [pC.tile([64, PHD], bf16, name=f"h0bf2{h}")
                     for h in range(NHM)]
            for k in range(16):
                rcv = pC.tile([128, PHD], bf16, name="rcv", tag="rcv",
                              bufs=2)
                nc.sync.dma_start(rcv[:],
                                  b1_out[0:128, PHD * k:PHD * (k + 1)])
                for j in (0, 1):
                    nc.vector.tensor_scalar(
                        h0bf2[2 * k + j][:], rcv[64 * j:64 * j + 64, :],
                        is_second[0:64, :], None, op0=OP.mult)
            for c in range(NCH):
                cs = slice(L * c, L * (c + 1))
                lcf2 = stage_lc(c)
                eba2 = None
                for h in range(NHM):
                    k = h // 2
                    rows = slice(64 * (h % 2), 64 * (h % 2) + 64)
                    if h % 16 == 0:
                        _, eba2 = stage_bcast(lcf2, h // 16)
                    cd = make_cdec(eba2, h, cs)
                    pyc = ps3p.tile([64, L], f32, name="pyc", tag="pyc",
                                    bufs=3)
                    nc.tensor.matmul(pyc[:], h0bf2[h][:], cd[:], start=True,
                                     stop=True)
                    # ycm += cumalpha * (h0^T @ Cdec)
                    nc.vector.scalar_tensor_tensor(
                        ycm[k][rows, cs], pyc[:],
                        carep[k][rows, c:c + 1], ycm[k][rows, cs],
                        op0=OP.mult, op1=OP.add)
            dbg_dump("d_y0", ycm[0][:], [128, TOK], mybir.dt.bfloat16)
            ps3.close()
            stC.close()
            stB.close()

        if PHASES >= 4:
            # ======== P4: gated norm + out_proj + x1 + rmsnorm2 ========
            st4 = stack()
            p4 = st4.enter_context(tc.tile_pool(name="p4", bufs=1))
            ps4s = stack()
            ps4 = ps4s.enter_context(tc.tile_pool(name="ps4", bufs=1,
                                                  space="PSUM"))
            g = [p4.tile([128, TOK], bf16, name=f"g{k}") for k in range(16)]
            for k in range(16):
                zs = p4.tile([128, TOK], bf16, name="zs", tag="zs", bufs=3)
                nc.sync.dma_start(zs[:], zsil_d[128 * k:128 * (k + 1), :])
                nc.vector.tensor_mul(g[k][:], ycm[k][:], zs[:])
            ssq = ps4.tile([1, TOK], f32, name="ssqg", tag="ssqg", bufs=1)
            for k in range(16):
                sq = p4.tile([128, TOK], bf16, name="gsq", tag="gsq", bufs=2)
                nc.vector.tensor_mul(sq[:], g[k][:], g[k][:])
                nc.tensor.matmul(ssq[:], onesb[:], sq[:], start=(k == 0),
                                 stop=(k == 15))
            rms = p4.tile([1, TOK], f32, name="grms")
            nc.scalar.activation(rms[:], ssq[:], AF.Sqrt,
                                 bias=eps_c[0:1, :], scale=1.0 / DIN)
            rinv = p4.tile([1, TOK], f32, name="grinv")
            nc.vector.reciprocal(rinv[:], rms[:])
            rbc = p4.tile([128, TOK], f32, name="grbc")
            nc.gpsimd.partition_broadcast(rbc[:], rinv[:], channels=128)
            for k in range(16):
                nc.vector.scalar_tensor_tensor(g[k][:], g[k][:],
                                               mnw_s[:, k:k + 1], rbc[:],
                                               op0=OP.mult, op1=OP.mult)
            dbg_dump("d_g0", g[0][:], [128, TOK], mybir.dt.bfloat16)

            x1 = [p4.tile([128, TOK], f32, name=f"x1_{i}") for i in range(8)]
            x1pb = pD.tile([128, 8], bf16, name="x1pb")
            for mb in range(8):
                sla = wslab(w_outproj, 128 * mb, 128, 8, f"wopa{mb}")
                slb = wslab(w_outproj, 128 * mb, 128, 8, f"wopb{mb}",
                            r0=1024)
                po = ps4.tile([128, TOK], f32, name="po", tag="pbig4",
                              bufs=3)
                for k in range(16):
                    sl_, kk = (sla, k) if k < 8 else (slb, k - 8)
                    nc.tensor.matmul(po[:], sl_[:, kk, :], g[k][:],
                                     start=(k == 0), stop=(k == 15))
                xre = p4.tile([128, TOK], f32, name="xre", tag="xre", bufs=2)
                nc.sync.dma_start(xre[:],
                                  xin[128 * mb:128 * (mb + 1), 3:TH])
                nc.vector.scalar_tensor_tensor(x1[mb][:], xre[:], 1.0,
                                               po[:], op0=OP.mult,
                                               op1=OP.add)
                nc.sync.dma_start(x1_d[128 * mb:128 * (mb + 1), :],
                                  x1[mb][:])
                nc.vector.tensor_copy(x1pb[:, mb:mb + 1],
                                      x1[mb][:, TOK - 1:TOK])
            x1n = [pD.tile([128, TOK], bf16, name=f"x1n{i}")
                   for i in range(8)]
            rmsnorm_cm([x1[i][:] for i in range(8)],
                       [x1n[i][:] for i in range(8)], TOK, p4, ps4, C_, "n1")
            dbg_dump("d_x1_0", x1[0][:], [128, TOK])
            ps4s.close()
            st4.close()
            stY.close()

        if PHASES >= 5:
            # ================= P5: attention =================
            st5 = stack()
            p5 = st5.enter_context(tc.tile_pool(name="p5", bufs=1))
            ps5s = stack()
            ps5 = ps5s.enter_context(tc.tile_pool(name="ps5", bufs=1,
                                                  space="PSUM"))
            amask = []
            for r in range(4):
                # keep when t >= s: f - p + (512*qb - 128*sb) >= 0,
                # variant j = sb - 4*qb in {0..3} -> base = -128*j
                m = p5.tile([128, 512], bf16, name=f"amask{r}")
                nc.vector.memset(m, 0.0)
                nc.gpsimd.affine_select(out=m, in_=m, compare_op=OP.is_ge,
                                        fill=NEG, base=-128 * r,
                                        channel_multiplier=-1,
                                        pattern=[[1, 512]])
                amask.append(m)
            qloc = [p5.tile([128, TOK], bf16, name=f"qloc{i}")
                    for i in range(8)]
            kloc = p5.tile([64, TOK], bf16, name="kloc")
            for mb in range(8):
                sl = wslab(w_att, 128 * mb, 128, 8, f"wq{mb}")
                pq = ps5.tile([128, TOK], f32, name="pq", tag="pbig5",
                              bufs=2)
                for k in range(8):
                    nc.tensor.matmul(pq[:], sl[:, k, :], x1n[k][:],
                                     start=(k == 0), stop=(k == 7))
                nc.vector.tensor_copy(qloc[mb][:], pq[:])
            slk = wslab(w_att, 1024, 64, 8, "wkp")
            pk = ps5.tile([64, TOK], f32, name="pk", tag="psx", bufs=3)
            for k in range(8):
                nc.tensor.matmul(pk[:], slk[:, k, :], x1n[k][:],
                                 start=(k == 0), stop=(k == 7))
            nc.vector.tensor_copy(kloc[:], pk[:])
            vloc = [p5.tile([128, 65], bf16, name=f"vloc{tb}")
                    for tb in range(4)]
            slv = wsl.tile([128, 8, 64], bf16, name="wvp", tag="wslab")
            nc.sync.dma_start(
                slv[:],
                w_att[:, 1088:1152].rearrange("(t p) m -> p t m", p=128))
            for tb in range(4):
                pv = ps5.tile([128, 64], f32, name="pv", tag="psx", bufs=3)
                for k in range(8):
                    nc.tensor.matmul(pv[:],
                                     x1n[k][:, 128 * tb:128 * (tb + 1)],
                                     slv[:, k, :], start=(k == 0),
                                     stop=(k == 7))
                nc.vector.tensor_copy(vloc[tb][:, 0:64], pv[:])
                nc.vector.memset(vloc[tb][:, 64:65], 1.0)
            dbg_dump("d_q0", qloc[0][:], [128, TOK], mybir.dt.bfloat16)

            b2_in = dram.tile([652, TOK], bf16, name="b2_in")
            b2_out = dram.tile([1304, TOK], bf16, name="b2_out")
            for i in range(4):
                nc.sync.dma_start(b2_in[128 * i:128 * (i + 1), :],
                                  qloc[4 + i][:])
            nc.sync.dma_start(b2_in[512:576, :], kloc[:])
            for tb in range(4):
                nc.sync.dma_start(
                    b2_in[576:641, 128 * tb:128 * (tb + 1)]
                    .rearrange("r c -> c r"), vloc[tb][:])
            nc.sync.dma_start(
                b2_in[644:652, 0:128].rearrange("f p -> p f"), x1pb[:])
            nc.gpsimd.collective_compute(
                "AllGather", OP.bypass, replica_groups=RG,
                ins=[b2_in.opt()], outs=[b2_out.opt()])

            def masked2(dst, local_ap, recv_ap, local_is_first):
                # dst/recv must share a base partition; local may be shifted.
                P = local_ap.shape[0]
                ma = is_first if local_is_first else is_second
                mb_ = is_second if local_is_first else is_first
                nc.vector.tensor_scalar(dst, local_ap, ma[0:P, :], None,
                                        op0=OP.mult)
                nc.vector.scalar_tensor_tensor(dst, recv_ap, mb_[0:P, :],
                                               dst, op0=OP.mult, op1=OP.add)

            qall = [p5.tile([64, T_], bf16, name=f"qall{h}")
                    for h in range(8)]
            kall = p5.tile([64, T_], bf16, name="kall")
            vall = [p5.tile([128, 65], bf16, name=f"vall{gb}")
                    for gb in range(8)]
            for h in range(8):
                t = h // 2
                ro = 128 * t + 64 * (h % 2)
                rows = slice(64 * (h % 2), 64 * (h % 2) + 64)
                for half in (0, 1):
                    rcv = p5.tile([64, TOK], bf16, name="qr", tag="qrcv",
                                  bufs=2)
                    nc.sync.dma_start(
                        rcv[:],
                        b2_out[652 * half + ro:652 * half + ro + 64, :])
                    masked2(qall[h][:, TOK * half:TOK * (half + 1)],
                            qloc[t][rows, :], rcv[:],
                            local_is_first=(half == 0))
            for half in (0, 1):
                rcv = p5.tile([64, TOK], bf16, name="kr", tag="krcv", bufs=2)
                nc.sync.dma_start(
                    rcv[:], b2_out[652 * half + 512:652 * half + 576, :])
                masked2(kall[:, TOK * half:TOK * (half + 1)], kloc[:],
                        rcv[:], local_is_first=(half == 0))
            for gb in range(8):
                half, tb = gb // 4, gb % 4
                rcv = p5.tile([128, 65], bf16, name="vr", tag="vrcv", bufs=2)
                nc.sync.dma_start(
                    rcv[:], b2_out[652 * half + 576:652 * half + 641,
                                   128 * tb:128 * (tb + 1)]
                    .rearrange("r c -> c r"))
                masked2(vall[gb][:], vloc[tb][:], rcv[:],
                        local_is_first=(half == 0))
            x1p = p5.tile([128, 8], bf16, name="x1p")
            rx = p5.tile([128, 8], bf16, name="rx")
            nc.sync.dma_start(
                rx[:], b2_out[644:652, 0:128].rearrange("f p -> p f"))
            nc.vector.tensor_scalar(x1p[:], rx[:], is_second, None,
                                    op0=OP.mult)
            dbg_dump("d_qall0", qall[0][:], [64, T_], mybir.dt.bfloat16)
            dbg_dump("d_kall", kall[:], [64, T_], mybir.dt.bfloat16)

            yall = [p5.tile([64, T_], bf16, name=f"yall{h}")
                    for h in range(8)]
            for h in range(8):
                for qb in range(2):
                    qcols = slice(TOK * qb, TOK * (qb + 1))
                    pav = ps5.tile([65, TOK], f32, name="pav", tag="pav",
                                   bufs=2)
                    nsb = 4 * (qb + 1)
                    for sb in range(nsb):
                        psx = ps5.tile([128, TOK], f32, name="psx",
                                       tag="psx", bufs=3)
                        nc.tensor.matmul(psx[:],
                                         kall[:, 128 * sb:128 * (sb + 1)],
                                         qall[h][:, qcols], start=True,
                                         stop=True)
                        r = sb - 4 * qb
                        if 0 <= r <= 3:
                            nc.vector.tensor_add(psx[:], psx[:],
                                                 amask[r][:])
                        pexp = p5.tile([128, TOK], bf16, name="pexp",
                                       tag="pexp", bufs=4)
                        nc.scalar.activation(pexp[:], psx[:], AF.Exp)
                        nc.tensor.matmul(pav[:], vall[sb][:], pexp[:],
                                         start=(sb == 0),
                                         stop=(sb == nsb - 1))
                    rc = p5.tile([1, TOK], f32, name="rcs", tag="rcs",
                                 bufs=2)
                    nc.vector.reciprocal(rc[:], pav[64:65, :])
                    rcb = p5.tile([64, TOK], f32, name="rcb", tag="rcb",
                                  bufs=2)
                    nc.gpsimd.partition_broadcast(rcb[:], rc[:],
                                                  channels=64)
                    nc.vector.tensor_mul(yall[h][:, qcols], pav[0:64, :],
                                         rcb[:])
            dbg_dump("d_yall0", yall[0][:], [64, T_], mybir.dt.bfloat16)

            # exchange 3 + proj rhs assembly (per-head base-0 builds)
            wph = [p5.tile([64, TOK + 1], bf16, name=f"wph{h}", tag="wph",
                           bufs=8) for h in range(8)]
            yown = [p5.tile([128, TOK + 1], bf16, name=f"yown{t}")
                    for t in range(4)]
            for h in range(8):
                t = h // 2
                rows = slice(64 * (h % 2), 64 * (h % 2) + 64)
                nc.vector.tensor_scalar(wph[h][:, :],
                                        yall[h][:, TOK - 1:T_],
                                        is_first[0:64, :], None,
                                        op0=OP.mult)
                nc.vector.scalar_tensor_tensor(
                    wph[h][:, 1:TOK + 1], yall[h][:, 0:TOK],
                    is_second[0:64, :], wph[h][:, 1:TOK + 1],
                    op0=OP.mult, op1=OP.add)
                yoh = p5.tile([64, TOK + 1], bf16, name="yoh", tag="yoh",
                              bufs=2)
                nc.vector.tensor_scalar(yoh[:, :],
                                        yall[h][:, TOK - 1:T_],
                                        is_second[0:64, :], None,
                                        op0=OP.mult)
                nc.vector.scalar_tensor_tensor(
                    yoh[:, 1:TOK + 1], yall[h][:, 0:TOK],
                    is_first[0:64, :], yoh[:, 1:TOK + 1],
                    op0=OP.mult, op1=OP.add)
                nc.vector.tensor_copy(yown[t][rows, :], yoh[:])
            b3_in = dram.tile([512, TOK + 1], bf16, name="b3_in")
            b3_out = dram.tile([1024, TOK + 1], bf16, name="b3_out")
            for h in range(8):
                nc.sync.dma_start(b3_in[64 * h:64 * (h + 1), :], wph[h][:])
            nc.gpsimd.collective_compute(
                "AllGather", OP.bypass, replica_groups=RG,
                ins=[b3_in.opt()], outs=[b3_out.opt()])

            yfull = yown + [p5.tile([128, TOK + 1], bf16, name=f"yfp{t}")
                            for t in range(4)]
            for t in range(4):
                r0 = p5.tile([128, TOK + 1], bf16, name="yr0", tag="yr0",
                             bufs=2)
                r1 = p5.tile([128, TOK + 1], bf16, name="yr1", tag="yr1",
                             bufs=2)
                nc.sync.dma_start(r0[:], b3_out[128 * t:128 * (t + 1), :])
                nc.sync.dma_start(
                    r1[:], b3_out[512 + 128 * t:512 + 128 * (t + 1), :])
                nc.vector.tensor_scalar(yfull[4 + t][:], r0[:], is_second,
                                        None, op0=OP.mult)
                nc.vector.scalar_tensor_tensor(yfull[4 + t][:], r1[:],
                                               is_first, yfull[4 + t][:],
                                               op0=OP.mult, op1=OP.add)

            x2 = [p5.tile([128, TOK], f32, name=f"x2_{i}")
                  for i in range(8)]
            x2p = resid.tile([128, 8], f32, name="x2p")
            for mb in range(8):
                sl = wslab(w_proj, 128 * mb, 128, 8, f"wpj{mb}")
                pp = ps5.tile([128, TOK], f32, name="pp", tag="pbig5",
                              bufs=2)
                pp1 = ps5.tile([128, 1], f32, name="pp1", tag="pp1", bufs=1)
                for k in range(8):
                    nc.tensor.matmul(pp[:], sl[:, k, :],
                                     yfull[k][:, 1:TOK + 1],
                                     start=(k == 0), stop=(k == 7))
                    nc.tensor.matmul(pp1[:], sl[:, k, :], yfull[k][:, 0:1],
                                     start=(k == 0), stop=(k == 7))
                x1l = p5.tile([128, TOK], f32, name="x1l", tag="x1l",
                              bufs=2)
                nc.sync.dma_start(x1l[:], x1_d[128 * mb:128 * (mb + 1), :])
                nc.vector.scalar_tensor_tensor(x2[mb][:], x1l[:], 1.0,
                                               pp[:], op0=OP.mult,
                                               op1=OP.add)
                nc.sync.dma_start(x2_d[128 * mb:128 * (mb + 1), :],
                                  x2[mb][:])
                tpv = p5.tile([128, 1], f32, name="tpv", tag="tpv", bufs=2)
                nc.vector.tensor_add(tpv[:], x1p[:, mb:mb + 1], pp1[:])
                nc.vector.tensor_scalar(x2p[:, mb:mb + 1], tpv[:],
                                        is_second, None, op0=OP.mult)
            dbg_dump("d_x2_0", x2[0][:], [128, TOK])
            ps5s.close()
            st5.close()
            stD.close()

        if PHASES >= 6:
            # ================= P6: cmix =================
            st6 = stack()
            p6 = st6.enter_context(tc.tile_pool(name="p6", bufs=1))
            ps6s = stack()
            ps6 = ps6s.enter_context(tc.tile_pool(name="ps6", bufs=1,
                                                  space="PSUM"))
            x2l = [p6.tile([128, TOK], f32, name=f"x2l{i}")
                   for i in range(8)]
            for i in range(8):
                nc.sync.dma_start(x2l[i][:],
                                  x2_d[128 * i:128 * (i + 1), :])
            z3 = [p6.tile([128, TOK + 1], bf16, name=f"z3_{i}")
                  for i in range(8)]
            rmsnorm_cm([x2l[i][:] for i in range(8)],
                       [z3[i][:, 1:TOK + 1] for i in range(8)], TOK, p6,
                       ps6, C_, "n2")
            sqp = p6.tile([128, 8], bf16, name="sqp")
            nc.vector.tensor_mul(sqp[:], x2p[:], x2p[:])
            psp = ps6.tile([1, 8], f32, name="psp", tag="psp", bufs=1)
            nc.tensor.matmul(psp[:], onesb[:], sqp[:], start=True,
                             stop=True)
            ssp = p6.tile([1, 1], f32, name="ssp")
            nc.vector.tensor_reduce(ssp[:], psp[:],
                                    axis=mybir.AxisListType.X, op=OP.add)
            nc.scalar.activation(ssp[:], ssp[:], AF.Sqrt,
                                 bias=eps_c[0:1, :], scale=1.0 / C_)
            nc.vector.reciprocal(ssp[:], ssp[:])
            rpb = p6.tile([128, 1], f32, name="rpb")
            nc.gpsimd.partition_broadcast(rpb[:], ssp[:], channels=128)
            for i in range(8):
                nc.vector.scalar_tensor_tensor(z3[i][:, 0:1],
                                               x2p[:, i:i + 1], 1.0,
                                               rpb[:], op0=OP.mult,
                                               op1=OP.mult)
            dbg_dump("d_z3_0", z3[0][:], [128, TOK + 1], mybir.dt.bfloat16)

            xk = [p6.tile([128, TOK], bf16, name=f"xk{i}")
                  for i in range(8)]
            xr = [p6.tile([128, TOK], bf16, name=f"xr{i}")
                  for i in range(8)]
            for i in range(8):
                nc.vector.tensor_scalar(xk[i][:], z3[i][:, 1:TOK + 1],
                                        mk1_s[:, i:i + 1], None,
                                        op0=OP.mult)
                nc.vector.scalar_tensor_tensor(xk[i][:], z3[i][:, 0:TOK],
                                               mk_s[:, i:i + 1], xk[i][:],
                                               op0=OP.mult, op1=OP.add)
                nc.vector.tensor_scalar(xr[i][:], z3[i][:, 1:TOK + 1],
                                        mr1_s[:, i:i + 1], None,
                                        op0=OP.mult)
                nc.vector.scalar_tensor_tensor(xr[i][:], z3[i][:, 0:TOK],
                                               mr_s[:, i:i + 1], xr[i][:],
                                               op0=OP.mult, op1=OP.add)

            kE = [p6.tile([128, TOK], bf16, name=f"kE{i}")
                  for i in range(32)]
            for mb in range(32):
                sl = wslab(w_key, 128 * mb, 128, 8, f"wky{mb}")
                pky = ps6.tile([128, TOK], f32, name="pky", tag="pbig6",
                               bufs=3)
                for k in range(8):
                    nc.tensor.matmul(pky[:], sl[:, k, :], xk[k][:],
                                     start=(k == 0), stop=(k == 7))
                nc.scalar.activation(kE[mb][:], pky[:], AF.Erf,
                                     scale=1.0 / _DEN, bias=erfb_c[:, :])
            r_sb = [p6.tile([128, TOK], bf16, name=f"r_sb{i}")
                    for i in range(8)]
            for mb in range(8):
                sl = wslab(w_rec, 128 * mb, 128, 8, f"wrc{mb}")
                pr = ps6.tile([128, TOK], f32, name="pr", tag="pbig6",
                              bufs=3)
                for k in range(8):
                    nc.tensor.matmul(pr[:], sl[:, k, :], xr[k][:],
                                     start=(k == 0), stop=(k == 7))
                nc.scalar.activation(r_sb[mb][:], pr[:], AF.Sigmoid)
            dbg_dump("d_kE0", kE[0][:], [128, TOK], mybir.dt.bfloat16)
            dbg_dump("d_r0", r_sb[0][:], [128, TOK], mybir.dt.bfloat16)

            for mb in range(8):
                slab = wslab(w_val, 128 * mb, 128, 32, f"wvl{mb}", pool=p6,
                             tag="wslab_v", bufs=2)
                pvv = ps6.tile([128, TOK], f32, name="pvv", tag="pbig6",
                               bufs=3)
                for k in range(32):
                    nc.tensor.matmul(pvv[:], slab[:, k, :], kE[k][:],
                                     start=(k == 0), stop=(k == 31))
                tmpv = p6.tile([128, TOK], f32, name="tmpv", tag="tmpv",
                               bufs=2)
                nc.vector.tensor_scalar(tmpv[:], pvv[:],
                                        vbias_s[:, mb:mb + 1], None,
                                        op0=OP.add)
                nc.vector.tensor_mul(tmpv[:], tmpv[:], r_sb[mb][:])
                outt = p6.tile([128, TOK], f32, name="outt", tag="outt",
                               bufs=2)
                nc.vector.tensor_add(outt[:], x2l[mb][:], tmpv[:])
                nc.sync.dma_start(out_d[128 * mb:128 * (mb + 1), :],
                                  outt[:])
            ps6s.close()
            st6.close()

        for s in reversed(_open):
            s.close()
        whole.close()

    nc.compile()
    return nc, dbg_outs


# ================= host glue =================

def _prep_inputs(x, in_proj_w, conv_w, conv_b, dt_bias, A_log, D, mnorm_w,
                 out_proj_w, attn_w, proj_w, time_maa_k, time_maa_r, key_w,
                 recept_w, value_w):
    f32 = np.float32

    def b(a):
        return np.ascontiguousarray(np.asarray(a, f32).astype(BF16))

    x = np.asarray(x, f32)
    shared = {
        "w_inproj": b(in_proj_w),
        "convw": np.ascontiguousarray(
            np.asarray(conv_w, f32).reshape(17, 128, DCONV)
            .transpose(1, 0, 2)),
        "convb": np.ascontiguousarray(
            np.asarray(conv_b, f32).reshape(17, 128).T),
        "dtb": np.ascontiguousarray(
            np.asarray(dt_bias, f32).reshape(NHM, 1)),
        "aneg": np.ascontiguousarray(
            (-np.exp(np.asarray(A_log, f32))).reshape(NHM, 1)),
        # drep[p, k] = D[2k + (p >= 64)]
        "drep": np.ascontiguousarray(np.stack(
            [np.concatenate([np.full(64, D2[0]), np.full(64, D2[1])])
             for D2 in np.asarray(D, f32).reshape(16, 2)], axis=1)
            .astype(f32)),
        "mnw": np.ascontiguousarray(
            np.asarray(mnorm_w, f32).reshape(16, 128).T),
        "w_outproj": b(out_proj_w),
        "mk": np.ascontiguousarray(
            np.asarray(time_maa_k, f32).reshape(8, 128).T),
        "mk1": np.ascontiguousarray(
            (1.0 - np.asarray(time_maa_k, f32)).reshape(8, 128).T),
        "mr": np.ascontiguousarray(
            np.asarray(time_maa_r, f32).reshape(8, 128).T),
        "mr1": np.ascontiguousarray(
            (1.0 - np.asarray(time_maa_r, f32)).reshape(8, 128).T),
        "w_key": b(key_w),
        "w_val": b(0.5 * np.asarray(value_w, f32)),
        "vbias": np.ascontiguousarray(
            (0.5 * np.asarray(value_w, f32).sum(0)).reshape(8, 128).T),
        "w_rec": b(recept_w),
    }
    ef = np.zeros((NHM, DIN), f32)
    for k in range(16):
        ef[2 * k, 128 * k:128 * k + 64] = 1.0
        ef[2 * k + 1, 128 * k + 64:128 * k + 128] = 1.0
    shared["efull"] = ef

    attn_w = np.asarray(attn_w, f32)
    proj_w = np.asarray(proj_w, f32)
    scale = 1.0 / np.sqrt(np.float32(HD))
    in_maps = []
    for core in range(N_CORES):
        bi, half = core // 2, core % 2
        start = half * TOK
        xcm = x[bi].T
        xs = np.zeros((C_, TH), f32)
        xs[:, 3:] = xcm[:, start:start + TOK]
        if start >= 3:
            xs[:, 0:3] = xcm[:, start - 3:start]
        myh = np.arange(8 * half, 8 * half + 8)
        oth = np.arange(8 * (1 - half), 8 * (1 - half) + 8)
        qcols = attn_w[:, :C_].reshape(C_, NH, HD)
        wq_perm = np.concatenate(
            [qcols[:, myh].reshape(C_, 512),
             qcols[:, oth].reshape(C_, 512)], axis=1) * scale
        w_att_c = np.concatenate([wq_perm, attn_w[:, C_:]], axis=1)
        prows = proj_w.reshape(NH, HD, C_)
        w_proj_c = np.concatenate(
            [prows[myh].reshape(512, C_), prows[oth].reshape(512, C_)],
            axis=0)
        mskc = np.zeros((128, 2), f32)
        mskc[:, 0] = 1.0 - half
        mskc[:, 1] = half
        m = dict(shared)
        m["xin"] = np.ascontiguousarray(xs)
        m["w_att"] = np.ascontiguousarray(w_att_c.astype(BF16))
        m["w_proj"] = np.ascontiguousarray(w_proj_c.astype(BF16))
        m["msk"] = mskc
        in_maps.append(m)
    return in_maps


def kernel(**inputs):
    from concourse.bass_utils import run_bass_kernel_spmd

    if "nc" not in _CACHE:
        _CACHE["nc"], _CACHE["dbg"] = _build()
    nc = _CACHE["nc"]
    in_maps = _prep_inputs(**inputs)
    res = run_bass_kernel_spmd(nc, in_maps, core_ids=list(range(N_CORES)))
    _CACHE["results"] = res
    out = np.empty((B_, T_, C_), np.float32)
    for core in range(N_CORES):
        bi, half = core // 2, core % 2
        out[bi, half * TOK:(half + 1) * TOK, :] = \
            np.asarray(res.results[core]["out"], np.float32).T
    return out


# revision 25
# speedup vs baseline: 1.0667x; 1.0053x over previous
"""nn_Block_21062519619681 fully on-device: hybrid Mamba2 + MQA + RWKV-CMix
block as ONE Bass/Tile SPMD kernel on 8 trn2 NeuronCores.

Sharding: 8 cores = 4 batches x 2 token-halves (512 own tokens/core).
 - mamba: token-sharded; chunked-SSD scan (L=128); cross-half state carry via
   a pairwise AllGather applied as a linear correction pass.
 - attention: q-head-split (8 heads/core over ALL 1024 tokens; per-core
   permuted q/proj weights keep the SPMD graph rank-uniform); k/v + q halves
   exchanged via pairwise AllGather; softmax without max-subtraction (scores
   bounded); colsum ridden as a ones-column in the av matmul.
 - cmix: token-sharded, replicated weights, erf/sigmoid fused into PSUM evac.
All matmuls bf16 (weights pre-cast on host), fp32 PSUM accumulate, fp32
residual stream. Rank-dependent selection uses host-fed 0/1 masks (masked
sums) - the instruction graph is identical on all cores.
"""
import os
import sys

sys.path.insert(0, "/opt/trn_rl_repo")
import numpy as np
import ml_dtypes

B_, T_, C_ = 4, 1024, 1024
NH, HD = 16, 64
DS, DCONV, EXP, PHD = 64, 4, 2, 64
DIN = EXP * C_
NHM = DIN // PHD
CONVD = DIN + 2 * DS
FFN = 4 * C_
EPS = 1e-5
N_CORES = 8
TOK = 512
TH = TOK + 3
L = 128
NCH = TOK // L
NEG = -1e30

BF16 = ml_dtypes.bfloat16
DEBUG = bool(int(os.environ.get("BASSK_DEBUG", "0")))
PHASES = int(os.environ.get("BASSK_PHASES", "6"))

_CACHE = {}


def _build():
    import contextlib
    import concourse.mybir as mybir
    import concourse.bacc as bacc
    import concourse.tile as tile
    from concourse.masks import make_identity

    f32 = mybir.dt.float32
    bf16 = mybir.dt.bfloat16
    AF = mybir.ActivationFunctionType
    OP = mybir.AluOpType

    nc = bacc.Bacc("TRN2", target_bir_lowering=False, debug=False,
                   num_devices=N_CORES)

    def din(name, shape, dt=bf16):
        return nc.dram_tensor(name, shape, dt, kind="ExternalInput").ap()

    xin = din("xin", [C_, TH], f32)
    w_inproj = din("w_inproj", [C_, 4256])
    convw = din("convw", [128, 17, DCONV], f32)
    convb = din("convb", [128, 17], f32)
    dtb = din("dtb", [NHM, 1], f32)
    aneg = din("aneg", [NHM, 1], f32)
    drep = din("drep", [128, 16], f32)
    mnw = din("mnw", [128, 16], f32)
    w_outproj = din("w_outproj", [DIN, C_])
    w_att = din("w_att", [C_, 1024 + 128])
    w_proj = din("w_proj", [C_, C_])
    mk = din("mk", [128, 8], f32)
    mk1 = din("mk1", [128, 8], f32)
    mr = din("mr", [128, 8], f32)
    mr1 = din("mr1", [128, 8], f32)
    w_key = din("w_key", [C_, FFN])
    w_val = din("w_val", [FFN, C_])
    vbias = din("vbias", [128, 8], f32)
    w_rec = din("w_rec", [C_, C_])
    msk = din("msk", [128, 2], f32)
    efull = din("efull", [NHM, DIN])

    out_d = nc.dram_tensor("out", [C_, TOK], f32, kind="ExternalOutput").ap()

    dbg_outs = {}

    def dbg_dump(name, ap_or_tile, shape, dt=None):
        if not DEBUG:
            return
        d = nc.dram_tensor(name, shape, dt or mybir.dt.float32,
                           kind="ExternalOutput").ap()
        dbg_outs[name] = d
        nc.sync.dma_start(d, ap_or_tile)

    RG = [[0, 1], [2, 3], [4, 5], [6, 7]]

    with tile.TileContext(nc) as tc:
        _open = []

        def stack():
            s = contextlib.ExitStack()
            _open.append(s)
            return s

        whole = contextlib.ExitStack()
        consts = whole.enter_context(tc.tile_pool(name="consts", bufs=1))
        resid = whole.enter_context(tc.tile_pool(name="resid", bufs=1))
        wsl = whole.enter_context(tc.tile_pool(name="wsl", bufs=3))
        dram = whole.enter_context(tc.tile_pool(name="dram", bufs=1,
                                                space="DRAM"))

        # ---------------- constants ----------------
        ident_b = consts.tile([128, 128], bf16, name="ident_b")
        make_identity(nc, ident_b)
        ident_f = consts.tile([128, 128], f32, name="ident_f")
        make_identity(nc, ident_f)
        tri01 = consts.tile([128, 128], bf16, name="tri01")
        nc.vector.memset(tri01, 1.0)
        nc.gpsimd.affine_select(out=tri01, in_=tri01, compare_op=OP.is_ge,
                                fill=0.0, base=0, channel_multiplier=-1,
                                pattern=[[1, 128]])
        onesb = consts.tile([128, 1], bf16, name="onesb")
        nc.vector.memset(onesb, 1.0)
        onesf_r = consts.tile([1, 64], f32, name="onesf_r")
        nc.vector.memset(onesf_r, 1.0)
        ones32 = consts.tile([NHM, L], f32, name="ones32")
        nc.vector.memset(ones32, 1.0)
        eps_c = consts.tile([128, 1], f32, name="eps_c")
        nc.vector.memset(eps_c, EPS)
        _MU = float(np.sqrt(0.5))
        _DEN = float(np.sqrt(1.0 / (4.0 * np.pi)) * np.sqrt(2.0))
        erfb_c = consts.tile([128, 1], f32, name="erfb_c")
        nc.vector.memset(erfb_c, -_MU / _DEN)

        def cin(name, shape, src, dt=f32):
            t = consts.tile(list(shape), dt, name=name)
            nc.sync.dma_start(t[:], src)
            return t

        convw_s = cin("convw_s", [128, 17, DCONV], convw)
        convb_s = cin("convb_s", [128, 17], convb)
        dtb_s = cin("dtb_s", [NHM, 1], dtb)
        aneg_s = cin("aneg_s", [NHM, 1], aneg)
        drep_s = cin("drep_s", [128, 16], drep)
        mnw_s = cin("mnw_s", [128, 16], mnw)
        mk_s = cin("mk_s", [128, 8], mk)
        mk1_s = cin("mk1_s", [128, 8], mk1)
        mr_s = cin("mr_s", [128, 8], mr)
        mr1_s = cin("mr1_s", [128, 8], mr1)
        vbias_s = cin("vbias_s", [128, 8], vbias)
        msk_s = cin("msk_s", [128, 2], msk)
        efull_s = cin("efull_s", [NHM, DIN], efull, dt=bf16)
        is_first = msk_s[:, 0:1]
        is_second = msk_s[:, 1:2]

        zsil_d = dram.tile([DIN, TOK], bf16, name="zsil_d")
        rinv1_dd = dram.tile([1, TOK], f32, name="rinv1_dd")
        x1_d = dram.tile([C_, TOK], f32, name="x1_d")
        x2_d = dram.tile([C_, TOK], f32, name="x2_d")

        def wslab(wt, m0, mw, kt, name, pool=None, tag="wslab", bufs=None,
                  r0=0):
            s = (pool or wsl).tile([128, kt, mw], bf16, name=name, tag=tag,
                                   bufs=bufs)
            nc.sync.dma_start(
                s[:], wt[r0:r0 + 128 * kt, m0:m0 + mw]
                .rearrange("(t p) m -> p t m", p=128))
            return s

        def rmsnorm_cm(src_aps, dst_aps, width, pool, psp, nfeat, tag):
            ssq = psp.tile([1, width], f32, name=f"ssq_{tag}",
                           tag=f"ssq{tag}", bufs=1)
            n = len(src_aps)
            for i, sap in enumerate(src_aps):
                sq = pool.tile([128, width], bf16, name=f"sq_{tag}",
                               tag=f"sq{tag}", bufs=2)
                nc.vector.tensor_mul(sq[:], sap, sap)
                nc.tensor.matmul(ssq[:], onesb[:], sq[:], start=(i == 0),
                                 stop=(i == n - 1))
            rms = pool.tile([1, width], f32, name=f"rms_{tag}",
                            tag=f"rms{tag}", bufs=1)
            nc.scalar.activation(rms[:], ssq[:], AF.Sqrt,
                                 bias=eps_c[0:1, :], scale=1.0 / nfeat)
            rinv = pool.tile([1, width], f32, name=f"rinv_{tag}",
                             tag=f"rinv{tag}", bufs=1)
            nc.vector.reciprocal(rinv[:], rms[:])
            rbc = pool.tile([128, width], f32, name=f"rbc_{tag}",
                            tag=f"rbc{tag}", bufs=1)
            nc.gpsimd.partition_broadcast(rbc[:], rinv[:], channels=128)
            for i, sap in enumerate(src_aps):
                nc.vector.tensor_mul(dst_aps[i], sap, rbc[:])

        # pool nesting (open early -> close late):
        stD = stack()
        pD = stD.enter_context(tc.tile_pool(name="pD", bufs=1))   # ..P5
        stY = stack()
        pY = stY.enter_context(tc.tile_pool(name="pY", bufs=1))   # ..P4
        stB = stack()
        pB = stB.enter_context(tc.tile_pool(name="pB", bufs=1))   # ..P3
        stC = stack()
        pC = stC.enter_context(tc.tile_pool(name="pC", bufs=1))   # ..P3

        # ================= P0 + P1: rmsnorm + in_proj =================
        st01 = stack()
        pA = st01.enter_context(tc.tile_pool(name="pA", bufs=1))
        ps01 = st01.enter_context(tc.tile_pool(name="ps01", bufs=1,
                                               space="PSUM"))
        xn = [pA.tile([128, TH], bf16, name=f"xn{i}") for i in range(8)]
        # streaming rmsnorm over x (full TH width, stats on own 512 cols)
        ssqx = ps01.tile([1, 512], f32, name="ssqx", tag="ssqx", bufs=1)
        ssqh = ps01.tile([1, 3], f32, name="ssqh", tag="ssqh", bufs=1)
        for i in range(8):
            xt = pA.tile([128, TH], f32, name="xt", tag="xt", bufs=3)
            nc.sync.dma_start(xt[:], xin[128 * i:128 * (i + 1), :])
            sqx = pA.tile([128, TH], bf16, name="sqx", tag="sqx", bufs=2)
            nc.vector.tensor_mul(sqx[:], xt[:], xt[:])
            nc.tensor.matmul(ssqx[:], onesb[:], sqx[:, 3:TH],
                             start=(i == 0), stop=(i == 7))
            nc.tensor.matmul(ssqh[:], onesb[:], sqx[:, 0:3],
                             start=(i == 0), stop=(i == 7))
        rmsx = pA.tile([1, TH], f32, name="rmsx")
        nc.scalar.activation(rmsx[:, 3:TH], ssqx[:], AF.Sqrt,
                             bias=eps_c[0:1, :], scale=1.0 / C_)
        nc.scalar.activation(rmsx[:, 0:3], ssqh[:], AF.Sqrt,
                             bias=eps_c[0:1, :], scale=1.0 / C_)
        rinvx = pA.tile([1, TH], f32, name="rinvx")
        nc.vector.reciprocal(rinvx[:], rmsx[:])
        rbcx = pA.tile([128, TH], f32, name="rbcx")
        nc.gpsimd.partition_broadcast(rbcx[:], rinvx[:], channels=128)
        for i in range(8):
            xt = pA.tile([128, TH], f32, name="xt", tag="xt", bufs=3)
            nc.sync.dma_start(xt[:], xin[128 * i:128 * (i + 1), :])
            nc.vector.tensor_mul(xn[i][:], xt[:], rbcx[:])
        dbg_dump("d_xn0", xn[0][:], [128, TH], mybir.dt.bfloat16)

        xbc = [pB.tile([128, TH], bf16, name=f"xbc{i}") for i in range(17)]
        dtraw = pB.tile([NHM, TOK], f32, name="dtraw")

        for mb in range(16):
            sl = wslab(w_inproj, 128 * mb, 128, 8, f"wz{mb}")
            pz = ps01.tile([128, TOK], f32, name="pz", tag="pbig", bufs=3)
            for k in range(8):
                nc.tensor.matmul(pz[:], sl[:, k, :], xn[k][:, 3:TH],
                                 start=(k == 0), stop=(k == 7))
            zst = pA.tile([128, TOK], bf16, name="zst", tag="zst", bufs=3)
            nc.scalar.activation(zst[:], pz[:], AF.Silu)
            nc.sync.dma_start(zsil_d[128 * mb:128 * (mb + 1), :], zst[:])
        for mb in range(17):
            sl = wslab(w_inproj, DIN + 128 * mb, 128, 8, f"wxbc{mb}")
            pb_ = ps01.tile([128, TOK], f32, name="pb", tag="pbig", bufs=3)
            ph = ps01.tile([128, 3], f32, name="ph", tag="phalo", bufs=2)
            for k in range(8):
                nc.tensor.matmul(pb_[:], sl[:, k, :], xn[k][:, 3:TH],
                                 start=(k == 0), stop=(k == 7))
                nc.tensor.matmul(ph[:], sl[:, k, :], xn[k][:, 0:3],
                                 start=(k == 0), stop=(k == 7))
            nc.scalar.copy(xbc[mb][:, 3:TH], pb_[:])
            nc.vector.tensor_copy(xbc[mb][:, 0:3], ph[:])
        sl = wslab(w_inproj, 4224, 32, 8, "wdtp")
        pdt = ps01.tile([NHM, TOK], f32, name="pdt", tag="pdt", bufs=1)
        for k in range(8):
            nc.tensor.matmul(pdt[:], sl[:, k, :], xn[k][:, 3:TH],
                             start=(k == 0), stop=(k == 7))
        nc.vector.tensor_copy(dtraw[:], pdt[:])
        dbg_dump("d_xbc0", xbc[0][:], [128, TH], mybir.dt.bfloat16)
        st01.close()

        if PHASES >= 2:
            # ============ P2: conv + dt pipeline + transposes ============
            ps2 = stack()
            ps2p = ps2.enter_context(tc.tile_pool(name="ps2", bufs=1,
                                                  space="PSUM"))
            xs_cm = [pC.tile([128, TOK], bf16, name=f"xs_cm{i}")
                     for i in range(17)]
            for i in range(17):
                tmp = pC.tile([128, TOK], f32, name="ctmp", tag="ctmp",
                              bufs=3)
                nc.vector.tensor_scalar(tmp[:], xbc[i][:, 0:TOK],
                                        convw_s[:, i, 0:1], None,
                                        op0=OP.mult)
                for j in range(1, DCONV):
                    nc.vector.scalar_tensor_tensor(
                        tmp[:], xbc[i][:, j:j + TOK], convw_s[:, i, j:j + 1],
                        tmp[:], op0=OP.mult, op1=OP.add)
                nc.scalar.activation(xs_cm[i][:], tmp[:], AF.Silu,
                                     bias=convb_s[:, i:i + 1])
            C_cm = pC.tile([64, TOK], bf16, name="C_cm")
            nc.vector.tensor_copy(C_cm[:], xs_cm[16][64:128, :])
            dbg_dump("d_xs0", xs_cm[0][:], [128, TOK], mybir.dt.bfloat16)

            dt_f = pC.tile([NHM, TOK], f32, name="dt_f")
            Lc = pC.tile([NHM, TOK], f32, name="Lc")
            wdt = pC.tile([NHM, TOK], f32, name="wdt", tag="scr", bufs=2)
            u = pC.tile([NHM, TOK], f32, name="u")
            ex = pC.tile([NHM, TOK], f32, name="ex", tag="scr", bufs=2)
            nc.vector.tensor_scalar(u[:], dtraw[:], dtb_s[:], None,
                                    op0=OP.add)
            ab = pC.tile([NHM, TOK], f32, name="ab", tag="scr", bufs=2)
            nc.vector.tensor_scalar(ab[:], u[:], -1.0, None, op0=OP.mult)
            nc.vector.tensor_max(ab[:], ab[:], u[:])
            nc.scalar.activation(ex[:], ab[:], AF.Exp, scale=-1.0)
            nc.scalar.activation(ex[:], ex[:], AF.Ln, bias=1.0)
            nc.vector.tensor_scalar(dt_f[:], u[:], 0.0, None, op0=OP.max)
            nc.vector.tensor_add(dt_f[:], dt_f[:], ex[:])
            dta = u
            nc.vector.tensor_scalar(dta[:], dt_f[:], aneg_s[:], None,
                                    op0=OP.mult)
            for c in range(NCH):
                cs = slice(L * c, L * (c + 1))
                nc.vector.tensor_tensor_scan(Lc[:, cs], ones32[:],
                                             dta[:, cs], 0.0, op0=OP.mult,
                                             op1=OP.add)
                nc.scalar.activation(wdt[:, cs], Lc[:, cs], AF.Exp,
                                     scale=-1.0,
                                     bias=Lc[:, L * (c + 1) - 1:L * (c + 1)])
            nc.vector.tensor_mul(wdt[:], wdt[:], dt_f[:])

            dbg_dump("d_dt", dt_f[:], [NHM, TOK])
            dbg_dump("d_Lc", Lc[:], [NHM, TOK])

            eLcE = pC.tile([NHM, NCH], bf16, name="eLcE")
            lce = pC.tile([NHM, NCH], f32, name="lce")
            for c in range(NCH):
                nc.vector.tensor_copy(lce[:, c:c + 1],
                                      Lc[:, L * (c + 1) - 1:L * (c + 1)])
            nc.scalar.activation(eLcE[:], lce[:], AF.Exp)
            arep = [pC.tile([128, NCH], f32, name=f"arep{k}")
                    for k in range(16)]
            carep = [pC.tile([128, NCH], f32, name=f"carep{k}")
                     for k in range(16)]
            for k in range(16):
                pa = ps2p.tile([128, NCH], f32, name="pa", tag="pa", bufs=2)
                nc.tensor.matmul(pa[:], efull_s[:, 128 * k:128 * (k + 1)],
                                 eLcE[:], start=True, stop=True)
                nc.vector.tensor_copy(arep[k][:], pa[:])
                nc.vector.memset(carep[k][:, 0:1], 1.0)
                for c in range(1, NCH):
                    nc.vector.tensor_mul(carep[k][:, c:c + 1],
                                         carep[k][:, c - 1:c],
                                         arep[k][:, c - 1:c])

            tmv = [pC.tile([128, 96], f32, name=f"tmv{c}")
                   for c in range(NCH)]
            stk = pC.tile([96, TOK], f32, name="stk")
            nc.vector.tensor_scalar(stk[0:NHM, :], Lc[:], -1.0, None,
                                    op0=OP.mult)
            nc.vector.tensor_copy(stk[NHM:2 * NHM, :], dt_f[:])
            nc.vector.tensor_copy(stk[2 * NHM:3 * NHM, :], wdt[:])
            for c in range(NCH):
                pt = ps2p.tile([128, 96], f32, name="pt", tag="ptr", bufs=2)
                nc.tensor.transpose(pt[:], stk[:, L * c:L * (c + 1)],
                                    ident_f[0:96, 0:96])
                nc.vector.tensor_copy(tmv[c][:], pt[:])

            xs_tm = [pC.tile([128, 2176], bf16, name=f"xs_tm{c}")
                     for c in range(NCH)]
            for c in range(NCH):
                for i in range(17):
                    ptb = ps2p.tile([128, 128], bf16, name="ptb", tag="ptrb",
                                    bufs=3)
                    nc.tensor.transpose(ptb[:],
                                        xs_cm[i][:, L * c:L * (c + 1)],
                                        ident_b[:])
                    nc.vector.tensor_copy(
                        xs_tm[c][:, 128 * i:128 * (i + 1)], ptb[:])
            dbg_dump("d_xstm0", xs_tm[0][:], [128, 2176], mybir.dt.bfloat16)
            ps2.close()

        if PHASES >= 3:
            # ========== P3: scan (interleaved per chunk) ==========
            ps3 = stack()
            ps3p = ps3.enter_context(tc.tile_pool(name="ps3", bufs=1,
                                                  space="PSUM"))
            state = [pC.tile([128, PHD], f32, name=f"state{k}")
                     for k in range(16)]
            for k in range(16):
                nc.vector.memset(state[k][:], 0.0)
            stateb = [pC.tile([64, PHD], bf16, name=f"stateb{h}")
                      for h in range(NHM)]
            ycm = [pY.tile([128, TOK], bf16, name=f"ycm{k}")
                   for k in range(16)]

            def stage_bcast(lcf, hh, with_exp=True, channels=128):
                lba = pC.tile([channels, 16 * L], f32, name="lba",
                              tag="lball", bufs=1)
                nc.gpsimd.partition_broadcast(
                    lba[:], lcf[0:1, 16 * L * hh:16 * L * (hh + 1)],
                    channels=channels)
                eba = None
                if with_exp:
                    eba = pC.tile([64, 16 * L], bf16, name="eba",
                                  tag="eball", bufs=1)
                    nc.scalar.activation(eba[:], lba[0:64, :], AF.Exp)
                return lba, eba

            def make_cdec_dve(eba, h, cs):
                off = L * (h % 16)
                cd = pC.tile([64, L], bf16, name="cd", tag="cdec", bufs=4)
                nc.vector.tensor_mul(cd[:], C_cm[:, cs],
                                     eba[:, off:off + L])
                return cd

            def make_cdec(eba, h, cs):
                off = L * (h % 16)
                cd = pC.tile([64, L], bf16, name="cd", tag="cdec", bufs=4)
                nc.gpsimd.tensor_mul(cd[:], C_cm[:, cs],
                                     eba[:, off:off + L])
                return cd

            def stage_lc(c):
                t = pC.tile([1, NHM * L], f32, name=f"LcFc{c}",
                            tag="lcf", bufs=2)
                nc.sync.dma_start(t[0:1, :], Lc[:, L * c:L * (c + 1)])
                return t

            for c in range(NCH):
                cs = slice(L * c, L * (c + 1))
                lcf = stage_lc(c)
                if c > 0:
                    for h in range(NHM):
                        nc.gpsimd.tensor_copy(
                            stateb[h][:],
                            state[h // 2][64 * (h % 2):64 * (h % 2) + 64, :])
                pg = ps3p.tile([128, L], f32, name="pg", tag="pg", bufs=1)
                nc.tensor.matmul(pg[:], xs_cm[16][0:64, cs], C_cm[:, cs],
                                 start=True, stop=True)
                gts = pC.tile([128, L], bf16, name="gts", tag="gts", bufs=2)
                nc.vector.tensor_mul(gts[:], pg[:], tri01[:])
                lba = eba = None
                for h in range(NHM):
                    k = h // 2
                    rows = slice(64 * (h % 2), 64 * (h % 2) + 64)
                    if h % 16 == 0:
                        lba, eba = stage_bcast(lcf, h // 16,
                                               with_exp=(c > 0))
                    darg = pC.tile([128, L], f32, name="darg", tag="darg",
                                   bufs=4)
                    nc.vector.tensor_scalar(darg[:],
                                            lba[:, L * (h % 16):
                                                L * (h % 16) + L],
                                            tmv[c][:, h:h + 1], 0.0,
                                            op0=OP.add, op1=OP.min)
                    expd = pC.tile([128, L], f32, name="expd", tag="expd",
                                   bufs=4)
                    nc.scalar.activation(expd[:], darg[:], AF.Exp)
                    mt = pC.tile([128, L], bf16, name="mt", tag="mt", bufs=4)
                    nc.vector.scalar_tensor_tensor(
                        mt[:], gts[:], tmv[c][:, 32 + h:33 + h], expd[:],
                        op0=OP.mult, op1=OP.mult)
                    py = ps3p.tile([64, L], f32, name="py", tag="py", bufs=2)
                    nc.tensor.matmul(py[:],
                                     xs_tm[c][:, PHD * h:PHD * (h + 1)],
                                     mt[:], start=True, stop=(c == 0))
                    if c > 0:
                        cd = make_cdec(eba, h, cs)
                        nc.tensor.matmul(py[:], stateb[h][:], cd[:],
                                         start=False, stop=True)
                    nc.vector.scalar_tensor_tensor(
                        ycm[k][rows, cs], xs_cm[k][rows, cs],
                        drep_s[rows, k:k + 1], py[:], op0=OP.mult,
                        op1=OP.add)
                    bw = pC.tile([128, DS], bf16, name="bw", tag="bw",
                                 bufs=3)
                    nc.gpsimd.tensor_scalar(
                        bw[:], xs_tm[c][:, DIN:DIN + DS],
                        tmv[c][:, 64 + h:65 + h], None, op0=OP.mult)
                    psc = ps3p.tile([64, PHD], f32, name="psc", tag="psc",
                                    bufs=2)
                    nc.tensor.matmul(psc[:], bw[:],
                                     xs_tm[c][:, PHD * h:PHD * (h + 1)],
                                     start=True, stop=True)
                    nc.vector.scalar_tensor_tensor(
                        state[k][rows, :], state[k][rows, :],
                        arep[k][rows, c:c + 1], psc[:], op0=OP.mult,
                        op1=OP.add)

            b1_in = dram.tile([128, 16 * PHD], bf16, name="b1_in")
            b1_out = dram.tile([256, 16 * PHD], bf16, name="b1_out")
            steb = pC.tile([128, 16 * PHD], bf16, name="steb")
            for k in range(16):
                nc.vector.tensor_copy(steb[:, PHD * k:PHD * (k + 1)],
                                      state[k][:])
            nc.sync.dma_start(b1_in[:], steb[:])
            nc.gpsimd.collective_compute(
                "AllGather", OP.bypass, replica_groups=RG,
                ins=[b1_in.opt()], outs=[b1_out.opt()])
            dbg_dump("d_st0", state[0][:], [128, PHD])

            h0bf2 = [pC.tile([64, PHD], bf16, name=f"h0bf2{h}")
                     for h in range(NHM)]
            for k in range(16):
                rcv = pC.tile([128, PHD], bf16, name="rcv", tag="rcv",
                              bufs=2)
                nc.sync.dma_start(rcv[:],
                                  b1_out[0:128, PHD * k:PHD * (k + 1)])
                for j in (0, 1):
                    nc.vector.tensor_scalar(
                        h0bf2[2 * k + j][:], rcv[64 * j:64 * j + 64, :],
                        is_second[0:64, :], None, op0=OP.mult)
            for c in range(NCH):
                cs = slice(L * c, L * (c + 1))
                lcf2 = stage_lc(c)
                eba2 = None
                for h in range(NHM):
                    k = h // 2
                    rows = slice(64 * (h % 2), 64 * (h % 2) + 64)
                    if h % 16 == 0:
                        _, eba2 = stage_bcast(lcf2, h // 16)
                    cd = make_cdec(eba2, h, cs)
                    pyc = ps3p.tile([64, L], f32, name="pyc", tag="pyc",
                                    bufs=3)
                    nc.tensor.matmul(pyc[:], h0bf2[h][:], cd[:], start=True,
                                     stop=True)
                    # ycm += cumalpha * (h0^T @ Cdec)
                    nc.vector.scalar_tensor_tensor(
                        ycm[k][rows, cs], pyc[:],
                        carep[k][rows, c:c + 1], ycm[k][rows, cs],
                        op0=OP.mult, op1=OP.add)
            dbg_dump("d_y0", ycm[0][:], [128, TOK], mybir.dt.bfloat16)
            ps3.close()
            stC.close()
            stB.close()

        if PHASES >= 4:
            # ======== P4: gated norm + out_proj + x1 + rmsnorm2 ========
            st4 = stack()
            p4 = st4.enter_context(tc.tile_pool(name="p4", bufs=1))
            ps4s = stack()
            ps4 = ps4s.enter_context(tc.tile_pool(name="ps4", bufs=1,
                                                  space="PSUM"))
            g = [p4.tile([128, TOK], bf16, name=f"g{k}") for k in range(16)]
            for k in range(16):
                zs = p4.tile([128, TOK], bf16, name="zs", tag="zs", bufs=3)
                nc.sync.dma_start(zs[:], zsil_d[128 * k:128 * (k + 1), :])
                nc.vector.tensor_mul(g[k][:], ycm[k][:], zs[:])
            ssq = ps4.tile([1, TOK], f32, name="ssqg", tag="ssqg", bufs=1)
            for k in range(16):
                sq = p4.tile([128, TOK], bf16, name="gsq", tag="gsq", bufs=2)
                nc.vector.tensor_mul(sq[:], g[k][:], g[k][:])
                nc.tensor.matmul(ssq[:], onesb[:], sq[:], start=(k == 0),
                                 stop=(k == 15))
            rms = p4.tile([1, TOK], f32, name="grms")
            nc.scalar.activation(rms[:], ssq[:], AF.Sqrt,
                                 bias=eps_c[0:1, :], scale=1.0 / DIN)
            rinv = p4.tile([1, TOK], f32, name="grinv")
            nc.vector.reciprocal(rinv[:], rms[:])
            rbc = p4.tile([128, TOK], f32, name="grbc")
            nc.gpsimd.partition_broadcast(rbc[:], rinv[:], channels=128)
            for k in range(16):
                nc.vector.scalar_tensor_tensor(g[k][:], g[k][:],
                                               mnw_s[:, k:k + 1], rbc[:],
                                               op0=OP.mult, op1=OP.mult)
            dbg_dump("d_g0", g[0][:], [128, TOK], mybir.dt.bfloat16)

            x1 = [p4.tile([128, TOK], f32, name=f"x1_{i}") for i in range(8)]
            x1pb = pD.tile([128, 8], bf16, name="x1pb")
            for mb in range(8):
                sla = wslab(w_outproj, 128 * mb, 128, 8, f"wopa{mb}")
                slb = wslab(w_outproj, 128 * mb, 128, 8, f"wopb{mb}",
                            r0=1024)
                po = ps4.tile([128, TOK], f32, name="po", tag="pbig4",
                              bufs=3)
                for k in range(16):
                    sl_, kk = (sla, k) if k < 8 else (slb, k - 8)
                    nc.tensor.matmul(po[:], sl_[:, kk, :], g[k][:],
                                     start=(k == 0), stop=(k == 15))
                xre = p4.tile([128, TOK], f32, name="xre", tag="xre", bufs=2)
                nc.sync.dma_start(xre[:],
                                  xin[128 * mb:128 * (mb + 1), 3:TH])
                nc.vector.scalar_tensor_tensor(x1[mb][:], xre[:], 1.0,
                                               po[:], op0=OP.mult,
                                               op1=OP.add)
                nc.sync.dma_start(x1_d[128 * mb:128 * (mb + 1), :],
                                  x1[mb][:])
                nc.vector.tensor_copy(x1pb[:, mb:mb + 1],
                                      x1[mb][:, TOK - 1:TOK])
            # deferred rmsnorm2: qkv runs on raw x1 (bf16); the per-token
            # 1/rms scale commutes with the GEMM and lands in the evacs.
            x1b = [pD.tile([128, TOK], bf16, name=f"x1b{i}")
                   for i in range(8)]
            for i in range(8):
                nc.vector.tensor_copy(x1b[i][:], x1[i][:])
            ssq1 = ps4.tile([1, TOK], f32, name="ssq1", tag="ssq1", bufs=1)
            for i in range(8):
                sq1 = p4.tile([128, TOK], bf16, name="sq1", tag="sq1",
                              bufs=2)
                nc.vector.tensor_mul(sq1[:], x1b[i][:], x1b[i][:])
                nc.tensor.matmul(ssq1[:], onesb[:], sq1[:], start=(i == 0),
                                 stop=(i == 7))
            rms1 = p4.tile([1, TOK], f32, name="rms1")
            nc.scalar.activation(rms1[:], ssq1[:], AF.Sqrt,
                                 bias=eps_c[0:1, :], scale=1.0 / C_)
            rinv1 = pD.tile([1, TOK], f32, name="rinv1")
            nc.vector.reciprocal(rinv1[:], rms1[:])
            rinv1b = p4.tile([1, TOK], bf16, name="rinv1b")
            nc.vector.tensor_copy(rinv1b[:], rinv1[:])
            rbc1 = pD.tile([128, TOK], bf16, name="rbc1")
            nc.gpsimd.partition_broadcast(rbc1[:], rinv1b[:], channels=128)
            rinv1_tm = pD.tile([128, 4], f32, name="rinv1_tm")
            nc.sync.dma_start(rinv1_dd[:], rinv1[:])
            nc.sync.dma_start(
                rinv1_tm[:],
                rinv1_dd[0:1, :].rearrange("a (c p) -> (a p) c", p=128))
            dbg_dump("d_x1_0", x1[0][:], [128, TOK])
            ps4s.close()
            st4.close()
            stY.close()

        if PHASES >= 5:
            # ================= P5: attention =================
            st5 = stack()
            p5 = st5.enter_context(tc.tile_pool(name="p5", bufs=1))
            ps5s = stack()
            ps5 = ps5s.enter_context(tc.tile_pool(name="ps5", bufs=1,
                                                  space="PSUM"))
            amask = []
            for r in range(4):
                # keep when t >= s: f - p + (512*qb - 128*sb) >= 0,
                # variant j = sb - 4*qb in {0..3} -> base = -128*j
                m = p5.tile([128, 512], bf16, name=f"amask{r}")
                nc.vector.memset(m, 0.0)
                nc.gpsimd.affine_select(out=m, in_=m, compare_op=OP.is_ge,
                                        fill=NEG, base=-128 * r,
                                        channel_multiplier=-1,
                                        pattern=[[1, 512]])
                amask.append(m)
            qloc = [p5.tile([128, TOK], bf16, name=f"qloc{i}")
                    for i in range(8)]
            kloc = p5.tile([64, TOK], bf16, name="kloc")
            for mb in range(8):
                sl = wslab(w_att, 128 * mb, 128, 8, f"wq{mb}")
                pq = ps5.tile([128, TOK], f32, name="pq", tag="pbig5",
                              bufs=2)
                for k in range(8):
                    nc.tensor.matmul(pq[:], sl[:, k, :], x1b[k][:],
                                     start=(k == 0), stop=(k == 7))
                nc.vector.tensor_mul(qloc[mb][:], pq[:], rbc1[:])
            slk = wslab(w_att, 1024, 64, 8, "wkp")
            pk = ps5.tile([64, TOK], f32, name="pk", tag="psx", bufs=3)
            for k in range(8):
                nc.tensor.matmul(pk[:], slk[:, k, :], x1b[k][:],
                                 start=(k == 0), stop=(k == 7))
            nc.vector.tensor_mul(kloc[:], pk[:], rbc1[0:64, :])
            vloc = [p5.tile([128, 65], bf16, name=f"vloc{tb}")
                    for tb in range(4)]
            slv = wsl.tile([128, 8, 64], bf16, name="wvp", tag="wslab")
            nc.sync.dma_start(
                slv[:],
                w_att[:, 1088:1152].rearrange("(t p) m -> p t m", p=128))
            for tb in range(4):
                pv = ps5.tile([128, 64], f32, name="pv", tag="psx", bufs=3)
                for k in range(8):
                    nc.tensor.matmul(pv[:],
                                     x1b[k][:, 128 * tb:128 * (tb + 1)],
                                     slv[:, k, :], start=(k == 0),
                                     stop=(k == 7))
                nc.vector.tensor_scalar(vloc[tb][:, 0:64], pv[:],
                                        rinv1_tm[:, tb:tb + 1], None,
                                        op0=OP.mult)
                nc.vector.memset(vloc[tb][:, 64:65], 1.0)
            dbg_dump("d_q0", qloc[0][:], [128, TOK], mybir.dt.bfloat16)

            b2_in = dram.tile([652, TOK], bf16, name="b2_in")
            b2_out = dram.tile([1304, TOK], bf16, name="b2_out")
            for i in range(4):
                nc.sync.dma_start(b2_in[128 * i:128 * (i + 1), :],
                                  qloc[4 + i][:])
            nc.sync.dma_start(b2_in[512:576, :], kloc[:])
            for tb in range(4):
                nc.sync.dma_start(
                    b2_in[576:641, 128 * tb:128 * (tb + 1)]
                    .rearrange("r c -> c r"), vloc[tb][:])
            nc.sync.dma_start(
                b2_in[644:652, 0:128].rearrange("f p -> p f"), x1pb[:])
            nc.gpsimd.collective_compute(
                "AllGather", OP.bypass, replica_groups=RG,
                ins=[b2_in.opt()], outs=[b2_out.opt()])

            def masked2(dst, local_ap, recv_ap, local_is_first):
                # dst/recv must share a base partition; local may be shifted.
                P = local_ap.shape[0]
                ma = is_first if local_is_first else is_second
                mb_ = is_second if local_is_first else is_first
                nc.vector.tensor_scalar(dst, local_ap, ma[0:P, :], None,
                                        op0=OP.mult)
                nc.vector.scalar_tensor_tensor(dst, recv_ap, mb_[0:P, :],
                                               dst, op0=OP.mult, op1=OP.add)

            qall = [p5.tile([64, T_], bf16, name=f"qall{h}")
                    for h in range(8)]
            kall = p5.tile([64, T_], bf16, name="kall")
            vall = [p5.tile([128, 65], bf16, name=f"vall{gb}")
                    for gb in range(8)]
            for h in range(8):
                t = h // 2
                ro = 128 * t + 64 * (h % 2)
                rows = slice(64 * (h % 2), 64 * (h % 2) + 64)
                for half in (0, 1):
                    rcv = p5.tile([64, TOK], bf16, name="qr", tag="qrcv",
                                  bufs=2)
                    nc.sync.dma_start(
                        rcv[:],
                        b2_out[652 * half + ro:652 * half + ro + 64, :])
                    masked2(qall[h][:, TOK * half:TOK * (half + 1)],
                            qloc[t][rows, :], rcv[:],
                            local_is_first=(half == 0))
            for half in (0, 1):
                rcv = p5.tile([64, TOK], bf16, name="kr", tag="krcv", bufs=2)
                nc.sync.dma_start(
                    rcv[:], b2_out[652 * half + 512:652 * half + 576, :])
                masked2(kall[:, TOK * half:TOK * (half + 1)], kloc[:],
                        rcv[:], local_is_first=(half == 0))
            for gb in range(8):
                half, tb = gb // 4, gb % 4
                rcv = p5.tile([128, 65], bf16, name="vr", tag="vrcv", bufs=2)
                nc.sync.dma_start(
                    rcv[:], b2_out[652 * half + 576:652 * half + 641,
                                   128 * tb:128 * (tb + 1)]
                    .rearrange("r c -> c r"))
                masked2(vall[gb][:], vloc[tb][:], rcv[:],
                        local_is_first=(half == 0))
            x1p = p5.tile([128, 8], bf16, name="x1p")
            rx = p5.tile([128, 8], bf16, name="rx")
            nc.sync.dma_start(
                rx[:], b2_out[644:652, 0:128].rearrange("f p -> p f"))
            nc.vector.tensor_scalar(x1p[:], rx[:], is_second, None,
                                    op0=OP.mult)
            dbg_dump("d_qall0", qall[0][:], [64, T_], mybir.dt.bfloat16)
            dbg_dump("d_kall", kall[:], [64, T_], mybir.dt.bfloat16)

            yall = [p5.tile([64, T_], bf16, name=f"yall{h}")
                    for h in range(8)]
            for h in range(8):
                for qb in range(2):
                    qcols = slice(TOK * qb, TOK * (qb + 1))
                    pav = ps5.tile([65, TOK], f32, name="pav", tag="pav",
                                   bufs=2)
                    nsb = 4 * (qb + 1)
                    for sb in range(nsb):
                        psx = ps5.tile([128, TOK], f32, name="psx",
                                       tag="psx", bufs=3)
                        nc.tensor.matmul(psx[:],
                                         kall[:, 128 * sb:128 * (sb + 1)],
                                         qall[h][:, qcols], start=True,
                                         stop=True)
                        r = sb - 4 * qb
                        if 0 <= r <= 3:
                            nc.vector.tensor_add(psx[:], psx[:],
                                                 amask[r][:])
                        pexp = p5.tile([128, TOK], bf16, name="pexp",
                                       tag="pexp", bufs=4)
                        nc.scalar.activation(pexp[:], psx[:], AF.Exp)
                        nc.tensor.matmul(pav[:], vall[sb][:], pexp[:],
                                         start=(sb == 0),
                                         stop=(sb == nsb - 1))
                    rc = p5.tile([1, TOK], f32, name="rcs", tag="rcs",
                                 bufs=2)
                    nc.vector.reciprocal(rc[:], pav[64:65, :])
                    rcb = p5.tile([64, TOK], f32, name="rcb", tag="rcb",
                                  bufs=2)
                    nc.gpsimd.partition_broadcast(rcb[:], rc[:],
                                                  channels=64)
                    nc.vector.tensor_mul(yall[h][:, qcols], pav[0:64, :],
                                         rcb[:])
            dbg_dump("d_yall0", yall[0][:], [64, T_], mybir.dt.bfloat16)

            # exchange 3 + proj rhs assembly (per-head base-0 builds)
            wph = [p5.tile([64, TOK + 1], bf16, name=f"wph{h}", tag="wph",
                           bufs=8) for h in range(8)]
            yown = [p5.tile([128, TOK + 1], bf16, name=f"yown{t}")
                    for t in range(4)]
            for h in range(8):
                t = h // 2
                rows = slice(64 * (h % 2), 64 * (h % 2) + 64)
                nc.vector.tensor_scalar(wph[h][:, :],
                                        yall[h][:, TOK - 1:T_],
                                        is_first[0:64, :], None,
                                        op0=OP.mult)
                nc.vector.scalar_tensor_tensor(
                    wph[h][:, 1:TOK + 1], yall[h][:, 0:TOK],
                    is_second[0:64, :], wph[h][:, 1:TOK + 1],
                    op0=OP.mult, op1=OP.add)
                yoh = p5.tile([64, TOK + 1], bf16, name="yoh", tag="yoh",
                              bufs=2)
                nc.vector.tensor_scalar(yoh[:, :],
                                        yall[h][:, TOK - 1:T_],
                                        is_second[0:64, :], None,
                                        op0=OP.mult)
                nc.vector.scalar_tensor_tensor(
                    yoh[:, 1:TOK + 1], yall[h][:, 0:TOK],
                    is_first[0:64, :], yoh[:, 1:TOK + 1],
                    op0=OP.mult, op1=OP.add)
                nc.vector.tensor_copy(yown[t][rows, :], yoh[:])
            b3_in = dram.tile([512, TOK + 1], bf16, name="b3_in")
            b3_out = dram.tile([1024, TOK + 1], bf16, name="b3_out")
            for h in range(8):
                nc.sync.dma_start(b3_in[64 * h:64 * (h + 1), :], wph[h][:])
            nc.gpsimd.collective_compute(
                "AllGather", OP.bypass, replica_groups=RG,
                ins=[b3_in.opt()], outs=[b3_out.opt()])

            yfull = yown + [p5.tile([128, TOK + 1], bf16, name=f"yfp{t}")
                            for t in range(4)]
            for t in range(4):
                r0 = p5.tile([128, TOK + 1], bf16, name="yr0", tag="yr0",
                             bufs=2)
                r1 = p5.tile([128, TOK + 1], bf16, name="yr1", tag="yr1",
                             bufs=2)
                nc.sync.dma_start(r0[:], b3_out[128 * t:128 * (t + 1), :])
                nc.sync.dma_start(
                    r1[:], b3_out[512 + 128 * t:512 + 128 * (t + 1), :])
                nc.vector.tensor_scalar(yfull[4 + t][:], r0[:], is_second,
                                        None, op0=OP.mult)
                nc.vector.scalar_tensor_tensor(yfull[4 + t][:], r1[:],
                                               is_first, yfull[4 + t][:],
                                               op0=OP.mult, op1=OP.add)

            x2 = [p5.tile([128, TOK], f32, name=f"x2_{i}")
                  for i in range(8)]
            x2p = resid.tile([128, 8], f32, name="x2p")
            for mb in range(8):
                sl = wslab(w_proj, 128 * mb, 128, 8, f"wpj{mb}")
                pp = ps5.tile([128, TOK], f32, name="pp", tag="pbig5",
                              bufs=2)
                pp1 = ps5.tile([128, 1], f32, name="pp1", tag="pp1", bufs=1)
                for k in range(8):
                    nc.tensor.matmul(pp[:], sl[:, k, :],
                                     yfull[k][:, 1:TOK + 1],
                                     start=(k == 0), stop=(k == 7))
                    nc.tensor.matmul(pp1[:], sl[:, k, :], yfull[k][:, 0:1],
                                     start=(k == 0), stop=(k == 7))
                x1l = p5.tile([128, TOK], f32, name="x1l", tag="x1l",
                              bufs=2)
                nc.sync.dma_start(x1l[:], x1_d[128 * mb:128 * (mb + 1), :])
                nc.vector.scalar_tensor_tensor(x2[mb][:], x1l[:], 1.0,
                                               pp[:], op0=OP.mult,
                                               op1=OP.add)
                nc.sync.dma_start(x2_d[128 * mb:128 * (mb + 1), :],
                                  x2[mb][:])
                tpv = p5.tile([128, 1], f32, name="tpv", tag="tpv", bufs=2)
                nc.vector.tensor_add(tpv[:], x1p[:, mb:mb + 1], pp1[:])
                nc.vector.tensor_scalar(x2p[:, mb:mb + 1], tpv[:],
                                        is_second, None, op0=OP.mult)
            dbg_dump("d_x2_0", x2[0][:], [128, TOK])
            ps5s.close()
            st5.close()
            stD.close()

        if PHASES >= 6:
            # ================= P6: cmix =================
            st6 = stack()
            p6 = st6.enter_context(tc.tile_pool(name="p6", bufs=1))
            ps6s = stack()
            ps6 = ps6s.enter_context(tc.tile_pool(name="ps6", bufs=1,
                                                  space="PSUM"))
            x2l = [p6.tile([128, TOK], f32, name=f"x2l{i}")
                   for i in range(8)]
            for i in range(8):
                nc.sync.dma_start(x2l[i][:],
                                  x2_d[128 * i:128 * (i + 1), :])
            z3 = [p6.tile([128, TOK + 1], bf16, name=f"z3_{i}")
                  for i in range(8)]
            rmsnorm_cm([x2l[i][:] for i in range(8)],
                       [z3[i][:, 1:TOK + 1] for i in range(8)], TOK, p6,
                       ps6, C_, "n2")
            sqp = p6.tile([128, 8], bf16, name="sqp")
            nc.vector.tensor_mul(sqp[:], x2p[:], x2p[:])
            psp = ps6.tile([1, 8], f32, name="psp", tag="psp", bufs=1)
            nc.tensor.matmul(psp[:], onesb[:], sqp[:], start=True,
                             stop=True)
            ssp = p6.tile([1, 1], f32, name="ssp")
            nc.vector.tensor_reduce(ssp[:], psp[:],
                                    axis=mybir.AxisListType.X, op=OP.add)
            nc.scalar.activation(ssp[:], ssp[:], AF.Sqrt,
                                 bias=eps_c[0:1, :], scale=1.0 / C_)
            nc.vector.reciprocal(ssp[:], ssp[:])
            rpb = p6.tile([128, 1], f32, name="rpb")
            nc.gpsimd.partition_broadcast(rpb[:], ssp[:], channels=128)
            for i in range(8):
                nc.vector.scalar_tensor_tensor(z3[i][:, 0:1],
                                               x2p[:, i:i + 1], 1.0,
                                               rpb[:], op0=OP.mult,
                                               op1=OP.mult)
            dbg_dump("d_z3_0", z3[0][:], [128, TOK + 1], mybir.dt.bfloat16)

            xk = [p6.tile([128, TOK], bf16, name=f"xk{i}")
                  for i in range(8)]
            xr = [p6.tile([128, TOK], bf16, name=f"xr{i}")
                  for i in range(8)]
            for i in range(8):
                nc.vector.tensor_scalar(xk[i][:], z3[i][:, 1:TOK + 1],
                                        mk1_s[:, i:i + 1], None,
                                        op0=OP.mult)
                nc.vector.scalar_tensor_tensor(xk[i][:], z3[i][:, 0:TOK],
                                               mk_s[:, i:i + 1], xk[i][:],
                                               op0=OP.mult, op1=OP.add)
                nc.vector.tensor_scalar(xr[i][:], z3[i][:, 1:TOK + 1],
                                        mr1_s[:, i:i + 1], None,
                                        op0=OP.mult)
                nc.vector.scalar_tensor_tensor(xr[i][:], z3[i][:, 0:TOK],
                                               mr_s[:, i:i + 1], xr[i][:],
                                               op0=OP.mult, op1=OP.add)

            kE = [p6.tile([128, TOK], bf16, name=f"kE{i}")
                  for i in range(32)]
            for mb in range(32):
                sl = wslab(w_key, 128 * mb, 128, 8, f"wky{mb}")
                pky = ps6.tile([128, TOK], f32, name="pky", tag="pbig6",
                               bufs=3)
                for k in range(8):
                    nc.tensor.matmul(pky[:], sl[:, k, :], xk[k][:],
                                     start=(k == 0), stop=(k == 7))
                nc.scalar.activation(kE[mb][:], pky[:], AF.Erf,
                                     scale=1.0 / _DEN, bias=erfb_c[:, :])
            r_sb = [p6.tile([128, TOK], bf16, name=f"r_sb{i}")
                    for i in range(8)]
            for mb in range(8):
                sl = wslab(w_rec, 128 * mb, 128, 8, f"wrc{mb}")
                pr = ps6.tile([128, TOK], f32, name="pr", tag="pbig6",
                              bufs=3)
                for k in range(8):
                    nc.tensor.matmul(pr[:], sl[:, k, :], xr[k][:],
                                     start=(k == 0), stop=(k == 7))
                nc.scalar.activation(r_sb[mb][:], pr[:], AF.Sigmoid)
            dbg_dump("d_kE0", kE[0][:], [128, TOK], mybir.dt.bfloat16)
            dbg_dump("d_r0", r_sb[0][:], [128, TOK], mybir.dt.bfloat16)

            for mb in range(8):
                slab = wslab(w_val, 128 * mb, 128, 32, f"wvl{mb}", pool=p6,
                             tag="wslab_v", bufs=2)
                pvv = ps6.tile([128, TOK], f32, name="pvv", tag="pbig6",
                               bufs=3)
                for k in range(32):
                    nc.tensor.matmul(pvv[:], slab[:, k, :], kE[k][:],
                                     start=(k == 0), stop=(k == 31))
                tmpv = p6.tile([128, TOK], f32, name="tmpv", tag="tmpv",
                               bufs=2)
                nc.vector.tensor_scalar(tmpv[:], pvv[:],
                                        vbias_s[:, mb:mb + 1], None,
                                        op0=OP.add)
                nc.vector.tensor_mul(tmpv[:], tmpv[:], r_sb[mb][:])
                outt = p6.tile([128, TOK], f32, name="outt", tag="outt",
                               bufs=2)
                nc.vector.tensor_add(outt[:], x2l[mb][:], tmpv[:])
                nc.sync.dma_start(out_d[128 * mb:128 * (mb + 1), :],
                                  outt[:])
            ps6s.close()
            st6.close()

        for s in reversed(_open):
            s.close()
        whole.close()

    nc.compile()
    return nc, dbg_outs


# ================= host glue =================

def _prep_inputs(x, in_proj_w, conv_w, conv_b, dt_bias, A_log, D, mnorm_w,
                 out_proj_w, attn_w, proj_w, time_maa_k, time_maa_r, key_w,
                 recept_w, value_w):
    f32 = np.float32

    def b(a):
        return np.ascontiguousarray(np.asarray(a, f32).astype(BF16))

    x = np.asarray(x, f32)
    shared = {
        "w_inproj": b(in_proj_w),
        "convw": np.ascontiguousarray(
            np.asarray(conv_w, f32).reshape(17, 128, DCONV)
            .transpose(1, 0, 2)),
        "convb": np.ascontiguousarray(
            np.asarray(conv_b, f32).reshape(17, 128).T),
        "dtb": np.ascontiguousarray(
            np.asarray(dt_bias, f32).reshape(NHM, 1)),
        "aneg": np.ascontiguousarray(
            (-np.exp(np.asarray(A_log, f32))).reshape(NHM, 1)),
        # drep[p, k] = D[2k + (p >= 64)]
        "drep": np.ascontiguousarray(np.stack(
            [np.concatenate([np.full(64, D2[0]), np.full(64, D2[1])])
             for D2 in np.asarray(D, f32).reshape(16, 2)], axis=1)
            .astype(f32)),
        "mnw": np.ascontiguousarray(
            np.asarray(mnorm_w, f32).reshape(16, 128).T),
        "w_outproj": b(out_proj_w),
        "mk": np.ascontiguousarray(
            np.asarray(time_maa_k, f32).reshape(8, 128).T),
        "mk1": np.ascontiguousarray(
            (1.0 - np.asarray(time_maa_k, f32)).reshape(8, 128).T),
        "mr": np.ascontiguousarray(
            np.asarray(time_maa_r, f32).reshape(8, 128).T),
        "mr1": np.ascontiguousarray(
            (1.0 - np.asarray(time_maa_r, f32)).reshape(8, 128).T),
        "w_key": b(key_w),
        "w_val": b(0.5 * np.asarray(value_w, f32)),
        "vbias": np.ascontiguousarray(
            (0.5 * np.asarray(value_w, f32).sum(0)).reshape(8, 128).T),
        "w_rec": b(recept_w),
    }
    ef = np.zeros((NHM, DIN), f32)
    for k in range(16):
        ef[2 * k, 128 * k:128 * k + 64] = 1.0
        ef[2 * k + 1, 128 * k + 64:128 * k + 128] = 1.0
    shared["efull"] = ef

    attn_w = np.asarray(attn_w, f32)
    proj_w = np.asarray(proj_w, f32)
    scale = 1.0 / np.sqrt(np.float32(HD))
    in_maps = []
    for core in range(N_CORES):
        bi, half = core // 2, core % 2
        start = half * TOK
        xcm = x[bi].T
        xs = np.zeros((C_, TH), f32)
        xs[:, 3:] = xcm[:, start:start + TOK]
        if start >= 3:
            xs[:, 0:3] = xcm[:, start - 3:start]
        myh = np.arange(8 * half, 8 * half + 8)
        oth = np.arange(8 * (1 - half), 8 * (1 - half) + 8)
        qcols = attn_w[:, :C_].reshape(C_, NH, HD)
        wq_perm = np.concatenate(
            [qcols[:, myh].reshape(C_, 512),
             qcols[:, oth].reshape(C_, 512)], axis=1) * scale
        w_att_c = np.concatenate([wq_perm, attn_w[:, C_:]], axis=1)
        prows = proj_w.reshape(NH, HD, C_)
        w_proj_c = np.concatenate(
            [prows[myh].reshape(512, C_), prows[oth].reshape(512, C_)],
            axis=0)
        mskc = np.zeros((128, 2), f32)
        mskc[:, 0] = 1.0 - half
        mskc[:, 1] = half
        m = dict(shared)
        m["xin"] = np.ascontiguousarray(xs)
        m["w_att"] = np.ascontiguousarray(w_att_c.astype(BF16))
        m["w_proj"] = np.ascontiguousarray(w_proj_c.astype(BF16))
        m["msk"] = mskc
        in_maps.append(m)
    return in_maps


def kernel(**inputs):
    from concourse.bass_utils import run_bass_kernel_spmd

    if "nc" not in _CACHE:
        _CACHE["nc"], _CACHE["dbg"] = _build()
    nc = _CACHE["nc"]
    in_maps = _prep_inputs(**inputs)
    res = run_bass_kernel_spmd(nc, in_maps, core_ids=list(range(N_CORES)))
    _CACHE["results"] = res
    out = np.empty((B_, T_, C_), np.float32)
    for core in range(N_CORES):
        bi, half = core // 2, core % 2
        out[bi, half * TOK:(half + 1) * TOK, :] = \
            np.asarray(res.results[core]["out"], np.float32).T
    return out


# revision 28
# speedup vs baseline: 1.0876x; 1.0196x over previous
"""nn_Block_21062519619681 fully on-device: hybrid Mamba2 + MQA + RWKV-CMix
block as ONE Bass/Tile SPMD kernel on 8 trn2 NeuronCores.

Sharding: 8 cores = 4 batches x 2 token-halves (512 own tokens/core).
 - mamba: token-sharded; chunked-SSD scan (L=128); cross-half state carry via
   a pairwise AllGather applied as a linear correction pass.
 - attention: q-head-split (8 heads/core over ALL 1024 tokens; per-core
   permuted q/proj weights keep the SPMD graph rank-uniform); k/v + q halves
   exchanged via pairwise AllGather; softmax without max-subtraction (scores
   bounded); colsum ridden as a ones-column in the av matmul.
 - cmix: token-sharded, replicated weights, erf/sigmoid fused into PSUM evac.
All matmuls bf16 (weights pre-cast on host), fp32 PSUM accumulate, fp32
residual stream. Rank-dependent selection uses host-fed 0/1 masks (masked
sums) - the instruction graph is identical on all cores.
"""
import os
import sys

sys.path.insert(0, "/opt/trn_rl_repo")
import numpy as np
import ml_dtypes

B_, T_, C_ = 4, 1024, 1024
NH, HD = 16, 64
DS, DCONV, EXP, PHD = 64, 4, 2, 64
DIN = EXP * C_
NHM = DIN // PHD
CONVD = DIN + 2 * DS
FFN = 4 * C_
EPS = 1e-5
N_CORES = 8
TOK = 512
TH = TOK + 3
L = 128
NCH = TOK // L
NEG = -1e30

BF16 = ml_dtypes.bfloat16
DEBUG = bool(int(os.environ.get("BASSK_DEBUG", "0")))
PHASES = int(os.environ.get("BASSK_PHASES", "6"))

_CACHE = {}


def _build():
    import contextlib
    import concourse.mybir as mybir
    import concourse.bacc as bacc
    import concourse.tile as tile
    from concourse.masks import make_identity

    f32 = mybir.dt.float32
    bf16 = mybir.dt.bfloat16
    AF = mybir.ActivationFunctionType
    OP = mybir.AluOpType

    nc = bacc.Bacc("TRN2", target_bir_lowering=False, debug=False,
                   num_devices=N_CORES)

    def din(name, shape, dt=bf16):
        return nc.dram_tensor(name, shape, dt, kind="ExternalInput").ap()

    xin = din("xin", [C_, TH], f32)
    w_inproj = din("w_inproj", [C_, 4256])
    convw = din("convw", [128, 17, DCONV], f32)
    convb = din("convb", [128, 17], f32)
    dtb = din("dtb", [NHM, 1], f32)
    aneg = din("aneg", [NHM, 1], f32)
    drep = din("drep", [128, 16], f32)
    mnw = din("mnw", [128, 16], f32)
    w_outproj = din("w_outproj", [DIN, C_])
    w_att = din("w_att", [C_, 1024 + 128])
    w_proj = din("w_proj", [C_, C_])
    mk = din("mk", [128, 8], f32)
    mk1 = din("mk1", [128, 8], f32)
    mr = din("mr", [128, 8], f32)
    mr1 = din("mr1", [128, 8], f32)
    w_key = din("w_key", [C_, FFN])
    w_val = din("w_val", [FFN, C_])
    vbias = din("vbias", [128, 8], f32)
    w_rec = din("w_rec", [C_, C_])
    msk = din("msk", [128, 2], f32)
    efull = din("efull", [NHM, DIN])

    out_d = nc.dram_tensor("out", [C_, TOK], f32, kind="ExternalOutput").ap()

    dbg_outs = {}

    def dbg_dump(name, ap_or_tile, shape, dt=None):
        if not DEBUG:
            return
        d = nc.dram_tensor(name, shape, dt or mybir.dt.float32,
                           kind="ExternalOutput").ap()
        dbg_outs[name] = d
        nc.sync.dma_start(d, ap_or_tile)

    RG = [[0, 1], [2, 3], [4, 5], [6, 7]]

    with tile.TileContext(nc) as tc:
        _open = []

        def stack():
            s = contextlib.ExitStack()
            _open.append(s)
            return s

        whole = contextlib.ExitStack()
        consts = whole.enter_context(tc.tile_pool(name="consts", bufs=1))
        resid = whole.enter_context(tc.tile_pool(name="resid", bufs=1))
        wsl = whole.enter_context(tc.tile_pool(name="wsl", bufs=3))
        dram = whole.enter_context(tc.tile_pool(name="dram", bufs=1,
                                                space="DRAM"))

        # ---------------- constants ----------------
        ident_b = consts.tile([128, 128], bf16, name="ident_b")
        make_identity(nc, ident_b)
        ident_f = consts.tile([128, 128], f32, name="ident_f")
        make_identity(nc, ident_f)
        tri01 = consts.tile([128, 128], bf16, name="tri01")
        nc.vector.memset(tri01, 1.0)
        nc.gpsimd.affine_select(out=tri01, in_=tri01, compare_op=OP.is_ge,
                                fill=0.0, base=0, channel_multiplier=-1,
                                pattern=[[1, 128]])
        onesb = consts.tile([128, 1], bf16, name="onesb")
        nc.vector.memset(onesb, 1.0)
        onesf_r = consts.tile([1, 64], f32, name="onesf_r")
        nc.vector.memset(onesf_r, 1.0)
        ones32 = consts.tile([NHM, L], f32, name="ones32")
        nc.vector.memset(ones32, 1.0)
        eps_c = consts.tile([128, 1], f32, name="eps_c")
        nc.vector.memset(eps_c, EPS)
        _MU = float(np.sqrt(0.5))
        _DEN = float(np.sqrt(1.0 / (4.0 * np.pi)) * np.sqrt(2.0))
        erfb_c = consts.tile([128, 1], f32, name="erfb_c")
        nc.vector.memset(erfb_c, -_MU / _DEN)

        def cin(name, shape, src, dt=f32):
            t = consts.tile(list(shape), dt, name=name)
            nc.sync.dma_start(t[:], src)
            return t

        convw_s = cin("convw_s", [128, 17, DCONV], convw)
        convb_s = cin("convb_s", [128, 17], convb)
        dtb_s = cin("dtb_s", [NHM, 1], dtb)
        aneg_s = cin("aneg_s", [NHM, 1], aneg)
        drep_s = cin("drep_s", [128, 16], drep)
        mnw_s = cin("mnw_s", [128, 16], mnw)
        mk_s = cin("mk_s", [128, 8], mk)
        mk1_s = cin("mk1_s", [128, 8], mk1)
        mr_s = cin("mr_s", [128, 8], mr)
        mr1_s = cin("mr1_s", [128, 8], mr1)
        vbias_s = cin("vbias_s", [128, 8], vbias)
        msk_s = cin("msk_s", [128, 2], msk)
        efull_s = cin("efull_s", [NHM, DIN], efull, dt=bf16)
        is_first = msk_s[:, 0:1]
        is_second = msk_s[:, 1:2]

        zsil_d = dram.tile([DIN, TOK], bf16, name="zsil_d")
        rinv1_dd = dram.tile([1, TOK], f32, name="rinv1_dd")

        def wslab(wt, m0, mw, kt, name, pool=None, tag="wslab", bufs=None,
                  r0=0):
            s = (pool or wsl).tile([128, kt, mw], bf16, name=name, tag=tag,
                                   bufs=bufs)
            nc.sync.dma_start(
                s[:], wt[r0:r0 + 128 * kt, m0:m0 + mw]
                .rearrange("(t p) m -> p t m", p=128))
            return s

        def rmsnorm_cm(src_aps, dst_aps, width, pool, psp, nfeat, tag):
            ssq = psp.tile([1, width], f32, name=f"ssq_{tag}",
                           tag=f"ssq{tag}", bufs=1)
            n = len(src_aps)
            for i, sap in enumerate(src_aps):
                sq = pool.tile([128, width], bf16, name=f"sq_{tag}",
                               tag=f"sq{tag}", bufs=2)
                nc.vector.tensor_mul(sq[:], sap, sap)
                nc.tensor.matmul(ssq[:], onesb[:], sq[:], start=(i == 0),
                                 stop=(i == n - 1))
            rms = pool.tile([1, width], f32, name=f"rms_{tag}",
                            tag=f"rms{tag}", bufs=1)
            nc.scalar.activation(rms[:], ssq[:], AF.Sqrt,
                                 bias=eps_c[0:1, :], scale=1.0 / nfeat)
            rinv = pool.tile([1, width], f32, name=f"rinv_{tag}",
                             tag=f"rinv{tag}", bufs=1)
            nc.vector.reciprocal(rinv[:], rms[:])
            rbc = pool.tile([128, width], f32, name=f"rbc_{tag}",
                            tag=f"rbc{tag}", bufs=1)
            nc.gpsimd.partition_broadcast(rbc[:], rinv[:], channels=128)
            for i, sap in enumerate(src_aps):
                nc.vector.tensor_mul(dst_aps[i], sap, rbc[:])

        # pool nesting (open early -> close late):
        stY = stack()
        pY = stY.enter_context(tc.tile_pool(name="pY", bufs=1))   # ..P4
        stB = stack()
        pB = stB.enter_context(tc.tile_pool(name="pB", bufs=1))   # ..P3
        stC = stack()
        pC = stC.enter_context(tc.tile_pool(name="pC", bufs=1))   # ..P3

        # ================= P0 + P1: rmsnorm + in_proj =================
        st01 = stack()
        pA = st01.enter_context(tc.tile_pool(name="pA", bufs=1))
        ps01 = st01.enter_context(tc.tile_pool(name="ps01", bufs=1,
                                               space="PSUM"))
        xn = [pA.tile([128, TH], bf16, name=f"xn{i}") for i in range(8)]
        # streaming rmsnorm over x (full TH width, stats on own 512 cols)
        ssqx = ps01.tile([1, 512], f32, name="ssqx", tag="ssqx", bufs=1)
        ssqh = ps01.tile([1, 3], f32, name="ssqh", tag="ssqh", bufs=1)
        for i in range(8):
            xt = pA.tile([128, TH], f32, name="xt", tag="xt", bufs=3)
            nc.sync.dma_start(xt[:], xin[128 * i:128 * (i + 1), :])
            sqx = pA.tile([128, TH], bf16, name="sqx", tag="sqx", bufs=2)
            nc.vector.tensor_mul(sqx[:], xt[:], xt[:])
            nc.tensor.matmul(ssqx[:], onesb[:], sqx[:, 3:TH],
                             start=(i == 0), stop=(i == 7))
            nc.tensor.matmul(ssqh[:], onesb[:], sqx[:, 0:3],
                             start=(i == 0), stop=(i == 7))
        rmsx = pA.tile([1, TH], f32, name="rmsx")
        nc.scalar.activation(rmsx[:, 3:TH], ssqx[:], AF.Sqrt,
                             bias=eps_c[0:1, :], scale=1.0 / C_)
        nc.scalar.activation(rmsx[:, 0:3], ssqh[:], AF.Sqrt,
                             bias=eps_c[0:1, :], scale=1.0 / C_)
        rinvx = pA.tile([1, TH], f32, name="rinvx")
        nc.vector.reciprocal(rinvx[:], rmsx[:])
        rbcx = pA.tile([128, TH], f32, name="rbcx")
        nc.gpsimd.partition_broadcast(rbcx[:], rinvx[:], channels=128)
        for i in range(8):
            xt = pA.tile([128, TH], f32, name="xt", tag="xt", bufs=3)
            nc.sync.dma_start(xt[:], xin[128 * i:128 * (i + 1), :])
            nc.vector.tensor_mul(xn[i][:], xt[:], rbcx[:])
        dbg_dump("d_xn0", xn[0][:], [128, TH], mybir.dt.bfloat16)

        xbc = [pB.tile([128, TH], bf16, name=f"xbc{i}") for i in range(17)]
        dtraw = pB.tile([NHM, TOK], f32, name="dtraw")

        for mb in range(16):
            sl = wslab(w_inproj, 128 * mb, 128, 8, f"wz{mb}")
            pz = ps01.tile([128, TOK], f32, name="pz", tag="pbig", bufs=3)
            for k in range(8):
                nc.tensor.matmul(pz[:], sl[:, k, :], xn[k][:, 3:TH],
                                 start=(k == 0), stop=(k == 7))
            zst = pA.tile([128, TOK], bf16, name="zst", tag="zst", bufs=3)
            nc.scalar.activation(zst[:], pz[:], AF.Silu)
            nc.sync.dma_start(zsil_d[128 * mb:128 * (mb + 1), :], zst[:])
        for mb in range(17):
            sl = wslab(w_inproj, DIN + 128 * mb, 128, 8, f"wxbc{mb}")
            pb_ = ps01.tile([128, TOK], f32, name="pb", tag="pbig", bufs=3)
            ph = ps01.tile([128, 3], f32, name="ph", tag="phalo", bufs=2)
            for k in range(8):
                nc.tensor.matmul(pb_[:], sl[:, k, :], xn[k][:, 3:TH],
                                 start=(k == 0), stop=(k == 7))
                nc.tensor.matmul(ph[:], sl[:, k, :], xn[k][:, 0:3],
                                 start=(k == 0), stop=(k == 7))
            nc.scalar.copy(xbc[mb][:, 3:TH], pb_[:])
            nc.vector.tensor_copy(xbc[mb][:, 0:3], ph[:])
        sl = wslab(w_inproj, 4224, 32, 8, "wdtp")
        pdt = ps01.tile([NHM, TOK], f32, name="pdt", tag="pdt", bufs=1)
        for k in range(8):
            nc.tensor.matmul(pdt[:], sl[:, k, :], xn[k][:, 3:TH],
                             start=(k == 0), stop=(k == 7))
        nc.vector.tensor_copy(dtraw[:], pdt[:])
        dbg_dump("d_xbc0", xbc[0][:], [128, TH], mybir.dt.bfloat16)
        st01.close()

        if PHASES >= 2:
            # ============ P2: conv + dt pipeline + transposes ============
            ps2 = stack()
            ps2p = ps2.enter_context(tc.tile_pool(name="ps2", bufs=1,
                                                  space="PSUM"))
            xs_cm = [pC.tile([128, TOK], bf16, name=f"xs_cm{i}")
                     for i in range(17)]
            for i in range(17):
                tmp = pC.tile([128, TOK], f32, name="ctmp", tag="ctmp",
                              bufs=3)
                nc.vector.tensor_scalar(tmp[:], xbc[i][:, 0:TOK],
                                        convw_s[:, i, 0:1], None,
                                        op0=OP.mult)
                for j in range(1, DCONV):
                    nc.vector.scalar_tensor_tensor(
                        tmp[:], xbc[i][:, j:j + TOK], convw_s[:, i, j:j + 1],
                        tmp[:], op0=OP.mult, op1=OP.add)
                nc.scalar.activation(xs_cm[i][:], tmp[:], AF.Silu,
                                     bias=convb_s[:, i:i + 1])
            C_cm = pC.tile([64, TOK], bf16, name="C_cm")
            nc.vector.tensor_copy(C_cm[:], xs_cm[16][64:128, :])
            dbg_dump("d_xs0", xs_cm[0][:], [128, TOK], mybir.dt.bfloat16)

            dt_f = pC.tile([NHM, TOK], f32, name="dt_f")
            Lc = pC.tile([NHM, TOK], f32, name="Lc")
            wdt = pC.tile([NHM, TOK], f32, name="wdt", tag="scr", bufs=2)
            u = pC.tile([NHM, TOK], f32, name="u")
            ex = pC.tile([NHM, TOK], f32, name="ex", tag="scr", bufs=2)
            nc.vector.tensor_scalar(u[:], dtraw[:], dtb_s[:], None,
                                    op0=OP.add)
            ab = pC.tile([NHM, TOK], f32, name="ab", tag="scr", bufs=2)
            nc.vector.tensor_scalar(ab[:], u[:], -1.0, None, op0=OP.mult)
            nc.vector.tensor_max(ab[:], ab[:], u[:])
            nc.scalar.activation(ex[:], ab[:], AF.Exp, scale=-1.0)
            nc.scalar.activation(ex[:], ex[:], AF.Ln, bias=1.0)
            nc.vector.tensor_scalar(dt_f[:], u[:], 0.0, None, op0=OP.max)
            nc.vector.tensor_add(dt_f[:], dt_f[:], ex[:])
            dta = u
            nc.vector.tensor_scalar(dta[:], dt_f[:], aneg_s[:], None,
                                    op0=OP.mult)
            for c in range(NCH):
                cs = slice(L * c, L * (c + 1))
                nc.vector.tensor_tensor_scan(Lc[:, cs], ones32[:],
                                             dta[:, cs], 0.0, op0=OP.mult,
                                             op1=OP.add)
                nc.scalar.activation(wdt[:, cs], Lc[:, cs], AF.Exp,
                                     scale=-1.0,
                                     bias=Lc[:, L * (c + 1) - 1:L * (c + 1)])
            nc.vector.tensor_mul(wdt[:], wdt[:], dt_f[:])

            dbg_dump("d_dt", dt_f[:], [NHM, TOK])
            dbg_dump("d_Lc", Lc[:], [NHM, TOK])

            eLcE = pC.tile([NHM, NCH], bf16, name="eLcE")
            lce = pC.tile([NHM, NCH], f32, name="lce")
            for c in range(NCH):
                nc.vector.tensor_copy(lce[:, c:c + 1],
                                      Lc[:, L * (c + 1) - 1:L * (c + 1)])
            nc.scalar.activation(eLcE[:], lce[:], AF.Exp)
            arep = [pC.tile([128, NCH], f32, name=f"arep{k}")
                    for k in range(16)]
            carep = [pC.tile([128, NCH], f32, name=f"carep{k}")
                     for k in range(16)]
            for k in range(16):
                pa = ps2p.tile([128, NCH], f32, name="pa", tag="pa", bufs=2)
                nc.tensor.matmul(pa[:], efull_s[:, 128 * k:128 * (k + 1)],
                                 eLcE[:], start=True, stop=True)
                nc.vector.tensor_copy(arep[k][:], pa[:])
                nc.vector.memset(carep[k][:, 0:1], 1.0)
                for c in range(1, NCH):
                    nc.vector.tensor_mul(carep[k][:, c:c + 1],
                                         carep[k][:, c - 1:c],
                                         arep[k][:, c - 1:c])

            tmv = [pC.tile([128, 96], f32, name=f"tmv{c}")
                   for c in range(NCH)]
            stk = pC.tile([96, TOK], f32, name="stk")
            nc.vector.tensor_scalar(stk[0:NHM, :], Lc[:], -1.0, None,
                                    op0=OP.mult)
            nc.vector.tensor_copy(stk[NHM:2 * NHM, :], dt_f[:])
            nc.vector.tensor_copy(stk[2 * NHM:3 * NHM, :], wdt[:])
            for c in range(NCH):
                pt = ps2p.tile([128, 96], f32, name="pt", tag="ptr", bufs=2)
                nc.tensor.transpose(pt[:], stk[:, L * c:L * (c + 1)],
                                    ident_f[0:96, 0:96])
                nc.vector.tensor_copy(tmv[c][:], pt[:])

            xs_tm = [pC.tile([128, 2176], bf16, name=f"xs_tm{c}")
                     for c in range(NCH)]
            for c in range(NCH):
                for i in range(17):
                    ptb = ps2p.tile([128, 128], bf16, name="ptb", tag="ptrb",
                                    bufs=3)
                    nc.tensor.transpose(ptb[:],
                                        xs_cm[i][:, L * c:L * (c + 1)],
                                        ident_b[:])
                    nc.vector.tensor_copy(
                        xs_tm[c][:, 128 * i:128 * (i + 1)], ptb[:])
            dbg_dump("d_xstm0", xs_tm[0][:], [128, 2176], mybir.dt.bfloat16)
            ps2.close()

        if PHASES >= 3:
            # ========== P3: scan (interleaved per chunk) ==========
            ps3 = stack()
            ps3p = ps3.enter_context(tc.tile_pool(name="ps3", bufs=1,
                                                  space="PSUM"))
            state = [pC.tile([128, PHD], f32, name=f"state{k}")
                     for k in range(16)]
            for k in range(16):
                nc.vector.memset(state[k][:], 0.0)
            stateb = [pC.tile([64, PHD], bf16, name=f"stateb{h}")
                      for h in range(NHM)]
            ycm = [pY.tile([128, TOK], bf16, name=f"ycm{k}")
                   for k in range(16)]

            def stage_bcast(lcf, hh, with_exp=True, channels=128):
                lba = pC.tile([channels, 16 * L], f32, name="lba",
                              tag="lball", bufs=1)
                nc.gpsimd.partition_broadcast(
                    lba[:], lcf[0:1, 16 * L * hh:16 * L * (hh + 1)],
                    channels=channels)
                eba = None
                if with_exp:
                    eba = pC.tile([64, 16 * L], bf16, name="eba",
                                  tag="eball", bufs=1)
                    nc.scalar.activation(eba[:], lba[0:64, :], AF.Exp)
                return lba, eba

            def make_cdec_dve(eba, h, cs):
                off = L * (h % 16)
                cd = pC.tile([64, L], bf16, name="cd", tag="cdec", bufs=4)
                nc.vector.tensor_mul(cd[:], C_cm[:, cs],
                                     eba[:, off:off + L])
                return cd

            def make_cdec(eba, h, cs):
                off = L * (h % 16)
                cd = pC.tile([64, L], bf16, name="cd", tag="cdec", bufs=4)
                nc.gpsimd.tensor_mul(cd[:], C_cm[:, cs],
                                     eba[:, off:off + L])
                return cd

            def stage_lc(c):
                t = pC.tile([1, NHM * L], f32, name=f"LcFc{c}",
                            tag="lcf", bufs=2)
                nc.sync.dma_start(t[0:1, :], Lc[:, L * c:L * (c + 1)])
                return t

            for c in range(NCH):
                cs = slice(L * c, L * (c + 1))
                lcf = stage_lc(c)
                if c > 0:
                    for h in range(NHM):
                        nc.gpsimd.tensor_copy(
                            stateb[h][:],
                            state[h // 2][64 * (h % 2):64 * (h % 2) + 64, :])
                pg = ps3p.tile([128, L], f32, name="pg", tag="pg", bufs=1)
                nc.tensor.matmul(pg[:], xs_cm[16][0:64, cs], C_cm[:, cs],
                                 start=True, stop=True)
                gts = pC.tile([128, L], bf16, name="gts", tag="gts", bufs=2)
                nc.vector.tensor_mul(gts[:], pg[:], tri01[:])
                lba = eba = None
                for h in range(NHM):
                    k = h // 2
                    rows = slice(64 * (h % 2), 64 * (h % 2) + 64)
                    if h % 16 == 0:
                        lba, eba = stage_bcast(lcf, h // 16,
                                               with_exp=(c > 0))
                    darg = pC.tile([128, L], f32, name="darg", tag="darg",
                                   bufs=4)
                    nc.vector.tensor_scalar(darg[:],
                                            lba[:, L * (h % 16):
                                                L * (h % 16) + L],
                                            tmv[c][:, h:h + 1], 0.0,
                                            op0=OP.add, op1=OP.min)
                    expd = pC.tile([128, L], f32, name="expd", tag="expd",
                                   bufs=4)
                    nc.scalar.activation(expd[:], darg[:], AF.Exp)
                    mt = pC.tile([128, L], bf16, name="mt", tag="mt", bufs=4)
                    nc.vector.scalar_tensor_tensor(
                        mt[:], gts[:], tmv[c][:, 32 + h:33 + h], expd[:],
                        op0=OP.mult, op1=OP.mult)
                    py = ps3p.tile([64, L], f32, name="py", tag="py", bufs=2)
                    nc.tensor.matmul(py[:],
                                     xs_tm[c][:, PHD * h:PHD * (h + 1)],
                                     mt[:], start=True, stop=(c == 0))
                    if c > 0:
                        cd = make_cdec(eba, h, cs)
                        nc.tensor.matmul(py[:], stateb[h][:], cd[:],
                                         start=False, stop=True)
                    nc.vector.scalar_tensor_tensor(
                        ycm[k][rows, cs], xs_cm[k][rows, cs],
                        drep_s[rows, k:k + 1], py[:], op0=OP.mult,
                        op1=OP.add)
                    bw = pC.tile([128, DS], bf16, name="bw", tag="bw",
                                 bufs=3)
                    nc.gpsimd.tensor_scalar(
                        bw[:], xs_tm[c][:, DIN:DIN + DS],
                        tmv[c][:, 64 + h:65 + h], None, op0=OP.mult)
                    psc = ps3p.tile([64, PHD], f32, name="psc", tag="psc",
                                    bufs=2)
                    nc.tensor.matmul(psc[:], bw[:],
                                     xs_tm[c][:, PHD * h:PHD * (h + 1)],
                                     start=True, stop=True)
                    nc.vector.scalar_tensor_tensor(
                        state[k][rows, :], state[k][rows, :],
                        arep[k][rows, c:c + 1], psc[:], op0=OP.mult,
                        op1=OP.add)

            b1_in = dram.tile([128, 16 * PHD], bf16, name="b1_in")
            b1_out = dram.tile([256, 16 * PHD], bf16, name="b1_out")
            steb = pC.tile([128, 16 * PHD], bf16, name="steb")
            for k in range(16):
                nc.vector.tensor_copy(steb[:, PHD * k:PHD * (k + 1)],
                                      state[k][:])
            nc.sync.dma_start(b1_in[:], steb[:])
            nc.gpsimd.collective_compute(
                "AllGather", OP.bypass, replica_groups=RG,
                ins=[b1_in.opt()], outs=[b1_out.opt()])
            dbg_dump("d_st0", state[0][:], [128, PHD])

            h0bf2 = [pC.tile([64, PHD], bf16, name=f"h0bf2{h}")
                     for h in range(NHM)]
            for k in range(16):
                rcv = pC.tile([128, PHD], bf16, name="rcv", tag="rcv",
                              bufs=2)
                nc.sync.dma_start(rcv[:],
                                  b1_out[0:128, PHD * k:PHD * (k + 1)])
                for j in (0, 1):
                    nc.vector.tensor_scalar(
                        h0bf2[2 * k + j][:], rcv[64 * j:64 * j + 64, :],
                        is_second[0:64, :], None, op0=OP.mult)
            for c in range(NCH):
                cs = slice(L * c, L * (c + 1))
                lcf2 = stage_lc(c)
                eba2 = None
                for h in range(NHM):
                    k = h // 2
                    rows = slice(64 * (h % 2), 64 * (h % 2) + 64)
                    if h % 16 == 0:
                        _, eba2 = stage_bcast(lcf2, h // 16)
                    cd = make_cdec(eba2, h, cs)
                    pyc = ps3p.tile([64, L], f32, name="pyc", tag="pyc",
                                    bufs=3)
                    nc.tensor.matmul(pyc[:], h0bf2[h][:], cd[:], start=True,
                                     stop=True)
                    # ycm += cumalpha * (h0^T @ Cdec)
                    nc.vector.scalar_tensor_tensor(
                        ycm[k][rows, cs], pyc[:],
                        carep[k][rows, c:c + 1], ycm[k][rows, cs],
                        op0=OP.mult, op1=OP.add)
            dbg_dump("d_y0", ycm[0][:], [128, TOK], mybir.dt.bfloat16)
            ps3.close()
            stC.close()
            stB.close()

        if PHASES >= 4:
            # ======== P4: gated norm + out_proj + x1 + rmsnorm2 ========
            stE = stack()
            pE = stE.enter_context(tc.tile_pool(name="pE", bufs=1))  # ..P6
            stD = stack()
            pD = stD.enter_context(tc.tile_pool(name="pD", bufs=1))  # ..P5
            st4 = stack()
            p4 = st4.enter_context(tc.tile_pool(name="p4", bufs=1))
            ps4s = stack()
            ps4 = ps4s.enter_context(tc.tile_pool(name="ps4", bufs=1,
                                                  space="PSUM"))
            g = [p4.tile([128, TOK], bf16, name=f"g{k}") for k in range(16)]
            for k in range(16):
                zs = p4.tile([128, TOK], bf16, name="zs", tag="zs", bufs=3)
                nc.sync.dma_start(zs[:], zsil_d[128 * k:128 * (k + 1), :])
                nc.vector.tensor_mul(g[k][:], ycm[k][:], zs[:])
            ssq = ps4.tile([1, TOK], f32, name="ssqg", tag="ssqg", bufs=1)
            for k in range(16):
                sq = p4.tile([128, TOK], bf16, name="gsq", tag="gsq", bufs=2)
                nc.vector.tensor_mul(sq[:], g[k][:], g[k][:])
                nc.tensor.matmul(ssq[:], onesb[:], sq[:], start=(k == 0),
                                 stop=(k == 15))
            rms = p4.tile([1, TOK], f32, name="grms")
            nc.scalar.activation(rms[:], ssq[:], AF.Sqrt,
                                 bias=eps_c[0:1, :], scale=1.0 / DIN)
            rinv = p4.tile([1, TOK], f32, name="grinv")
            nc.vector.reciprocal(rinv[:], rms[:])
            rbc = p4.tile([128, TOK], f32, name="grbc")
            nc.gpsimd.partition_broadcast(rbc[:], rinv[:], channels=128)
            for k in range(16):
                nc.vector.scalar_tensor_tensor(g[k][:], g[k][:],
                                               mnw_s[:, k:k + 1], rbc[:],
                                               op0=OP.mult, op1=OP.mult)
            dbg_dump("d_g0", g[0][:], [128, TOK], mybir.dt.bfloat16)

            x1 = [pD.tile([128, TOK], f32, name=f"x1_{i}")
                  for i in range(8)]
            x1pb = pD.tile([128, 8], bf16, name="x1pb")
            for mb in range(8):
                sla = wslab(w_outproj, 128 * mb, 128, 8, f"wopa{mb}")
                slb = wslab(w_outproj, 128 * mb, 128, 8, f"wopb{mb}",
                            r0=1024)
                po = ps4.tile([128, TOK], f32, name="po", tag="pbig4",
                              bufs=3)
                for k in range(16):
                    sl_, kk = (sla, k) if k < 8 else (slb, k - 8)
                    nc.tensor.matmul(po[:], sl_[:, kk, :], g[k][:],
                                     start=(k == 0), stop=(k == 15))
                xre = p4.tile([128, TOK], f32, name="xre", tag="xre", bufs=2)
                nc.sync.dma_start(xre[:],
                                  xin[128 * mb:128 * (mb + 1), 3:TH])
                nc.vector.scalar_tensor_tensor(x1[mb][:], xre[:], 1.0,
                                               po[:], op0=OP.mult,
                                               op1=OP.add)
                nc.vector.tensor_copy(x1pb[:, mb:mb + 1],
                                      x1[mb][:, TOK - 1:TOK])
            # deferred rmsnorm2: qkv runs on raw x1 (bf16); the per-token
            # 1/rms scale commutes with the GEMM and lands in the evacs.
            x1b = [pD.tile([128, TOK], bf16, name=f"x1b{i}")
                   for i in range(8)]
            for i in range(8):
                nc.vector.tensor_copy(x1b[i][:], x1[i][:])
            ssq1 = ps4.tile([1, TOK], f32, name="ssq1", tag="ssq1", bufs=1)
            for i in range(8):
                sq1 = p4.tile([128, TOK], bf16, name="sq1", tag="sq1",
                              bufs=2)
                nc.vector.tensor_mul(sq1[:], x1b[i][:], x1b[i][:])
                nc.tensor.matmul(ssq1[:], onesb[:], sq1[:], start=(i == 0),
                                 stop=(i == 7))
            rms1 = p4.tile([1, TOK], f32, name="rms1")
            nc.scalar.activation(rms1[:], ssq1[:], AF.Sqrt,
                                 bias=eps_c[0:1, :], scale=1.0 / C_)
            rinv1 = pD.tile([1, TOK], f32, name="rinv1")
            nc.vector.reciprocal(rinv1[:], rms1[:])
            rinv1b = p4.tile([1, TOK], bf16, name="rinv1b")
            nc.vector.tensor_copy(rinv1b[:], rinv1[:])
            rbc1 = pD.tile([128, TOK], bf16, name="rbc1")
            nc.gpsimd.partition_broadcast(rbc1[:], rinv1b[:], channels=128)
            rinv1_tm = pD.tile([128, 4], f32, name="rinv1_tm")
            nc.sync.dma_start(rinv1_dd[:], rinv1[:])
            nc.sync.dma_start(
                rinv1_tm[:],
                rinv1_dd[0:1, :].rearrange("a (c p) -> (a p) c", p=128))
            dbg_dump("d_x1_0", x1[0][:], [128, TOK])
            ps4s.close()
            st4.close()

        if PHASES >= 5:
            # ================= P5: attention =================
            st5 = stack()
            p5 = st5.enter_context(tc.tile_pool(name="p5", bufs=1))
            ps5s = stack()
            ps5 = ps5s.enter_context(tc.tile_pool(name="ps5", bufs=1,
                                                  space="PSUM"))
            amask = []
            for r in range(4):
                # keep when t >= s: f - p + (512*qb - 128*sb) >= 0,
                # variant j = sb - 4*qb in {0..3} -> base = -128*j
                m = p5.tile([128, 512], bf16, name=f"amask{r}")
                nc.vector.memset(m, 0.0)
                nc.gpsimd.affine_select(out=m, in_=m, compare_op=OP.is_ge,
                                        fill=NEG, base=-128 * r,
                                        channel_multiplier=-1,
                                        pattern=[[1, 512]])
                amask.append(m)
            qloc = [p5.tile([128, TOK], bf16, name=f"qloc{i}")
                    for i in range(8)]
            kloc = p5.tile([64, TOK], bf16, name="kloc")
            for mb in range(8):
                sl = wslab(w_att, 128 * mb, 128, 8, f"wq{mb}")
                pq = ps5.tile([128, TOK], f32, name="pq", tag="pbig5",
                              bufs=2)
                for k in range(8):
                    nc.tensor.matmul(pq[:], sl[:, k, :], x1b[k][:],
                                     start=(k == 0), stop=(k == 7))
                nc.vector.tensor_mul(qloc[mb][:], pq[:], rbc1[:])
            slk = wslab(w_att, 1024, 64, 8, "wkp")
            pk = ps5.tile([64, TOK], f32, name="pk", tag="psx", bufs=3)
            for k in range(8):
                nc.tensor.matmul(pk[:], slk[:, k, :], x1b[k][:],
                                 start=(k == 0), stop=(k == 7))
            nc.vector.tensor_mul(kloc[:], pk[:], rbc1[0:64, :])
            vloc = [p5.tile([128, 65], bf16, name=f"vloc{tb}")
                    for tb in range(4)]
            slv = wsl.tile([128, 8, 64], bf16, name="wvp", tag="wslab")
            nc.sync.dma_start(
                slv[:],
                w_att[:, 1088:1152].rearrange("(t p) m -> p t m", p=128))
            for tb in range(4):
                pv = ps5.tile([128, 64], f32, name="pv", tag="psx", bufs=3)
                for k in range(8):
                    nc.tensor.matmul(pv[:],
                                     x1b[k][:, 128 * tb:128 * (tb + 1)],
                                     slv[:, k, :], start=(k == 0),
                                     stop=(k == 7))
                nc.vector.tensor_scalar(vloc[tb][:, 0:64], pv[:],
                                        rinv1_tm[:, tb:tb + 1], None,
                                        op0=OP.mult)
                nc.vector.memset(vloc[tb][:, 64:65], 1.0)
            dbg_dump("d_q0", qloc[0][:], [128, TOK], mybir.dt.bfloat16)

            b2_in = dram.tile([652, TOK], bf16, name="b2_in")
            b2_out = dram.tile([1304, TOK], bf16, name="b2_out")
            for i in range(4):
                nc.sync.dma_start(b2_in[128 * i:128 * (i + 1), :],
                                  qloc[4 + i][:])
            nc.sync.dma_start(b2_in[512:576, :], kloc[:])
            for tb in range(4):
                nc.sync.dma_start(
                    b2_in[576:641, 128 * tb:128 * (tb + 1)]
                    .rearrange("r c -> c r"), vloc[tb][:])
            nc.sync.dma_start(
                b2_in[644:652, 0:128].rearrange("f p -> p f"), x1pb[:])
            nc.gpsimd.collective_compute(
                "AllGather", OP.bypass, replica_groups=RG,
                ins=[b2_in.opt()], outs=[b2_out.opt()])

            def masked2(dst, local_ap, recv_ap, local_is_first):
                # dst/recv must share a base partition; local may be shifted.
                P = local_ap.shape[0]
                ma = is_first if local_is_first else is_second
                mb_ = is_second if local_is_first else is_first
                nc.vector.tensor_scalar(dst, local_ap, ma[0:P, :], None,
                                        op0=OP.mult)
                nc.vector.scalar_tensor_tensor(dst, recv_ap, mb_[0:P, :],
                                               dst, op0=OP.mult, op1=OP.add)

            qall = [p5.tile([64, T_], bf16, name=f"qall{h}")
                    for h in range(8)]
            kall = p5.tile([64, T_], bf16, name="kall")
            vall = [p5.tile([128, 65], bf16, name=f"vall{gb}")
                    for gb in range(8)]
            for h in range(8):
                t = h // 2
                ro = 128 * t + 64 * (h % 2)
                rows = slice(64 * (h % 2), 64 * (h % 2) + 64)
                for half in (0, 1):
                    rcv = p5.tile([64, TOK], bf16, name="qr", tag="qrcv",
                                  bufs=2)
                    nc.sync.dma_start(
                        rcv[:],
                        b2_out[652 * half + ro:652 * half + ro + 64, :])
                    masked2(qall[h][:, TOK * half:TOK * (half + 1)],
                            qloc[t][rows, :], rcv[:],
                            local_is_first=(half == 0))
            for half in (0, 1):
                rcv = p5.tile([64, TOK], bf16, name="kr", tag="krcv", bufs=2)
                nc.sync.dma_start(
                    rcv[:], b2_out[652 * half + 512:652 * half + 576, :])
                masked2(kall[:, TOK * half:TOK * (half + 1)], kloc[:],
                        rcv[:], local_is_first=(half == 0))
            for gb in range(8):
                half, tb = gb // 4, gb % 4
                rcv = p5.tile([128, 65], bf16, name="vr", tag="vrcv", bufs=2)
                nc.sync.dma_start(
                    rcv[:], b2_out[652 * half + 576:652 * half + 641,
                                   128 * tb:128 * (tb + 1)]
                    .rearrange("r c -> c r"))
                masked2(vall[gb][:], vloc[tb][:], rcv[:],
                        local_is_first=(half == 0))
            x1p = p5.tile([128, 8], bf16, name="x1p")
            rx = p5.tile([128, 8], bf16, name="rx")
            nc.sync.dma_start(
                rx[:], b2_out[644:652, 0:128].rearrange("f p -> p f"))
            nc.vector.tensor_scalar(x1p[:], rx[:], is_second, None,
                                    op0=OP.mult)
            dbg_dump("d_qall0", qall[0][:], [64, T_], mybir.dt.bfloat16)
            dbg_dump("d_kall", kall[:], [64, T_], mybir.dt.bfloat16)

            yall = [p5.tile([64, T_], bf16, name=f"yall{h}")
                    for h in range(8)]
            for h in range(8):
                for qb in range(2):
                    qcols = slice(TOK * qb, TOK * (qb + 1))
                    pav = ps5.tile([65, TOK], f32, name="pav", tag="pav",
                                   bufs=2)
                    nsb = 4 * (qb + 1)
                    for sb in range(nsb):
                        psx = ps5.tile([128, TOK], f32, name="psx",
                                       tag="psx", bufs=3)
                        nc.tensor.matmul(psx[:],
                                         kall[:, 128 * sb:128 * (sb + 1)],
                                         qall[h][:, qcols], start=True,
                                         stop=True)
                        r = sb - 4 * qb
                        if 0 <= r <= 3:
                            nc.vector.tensor_add(psx[:], psx[:],
                                                 amask[r][:])
                        pexp = p5.tile([128, TOK], bf16, name="pexp",
                                       tag="pexp", bufs=4)
                        nc.scalar.activation(pexp[:], psx[:], AF.Exp)
                        nc.tensor.matmul(pav[:], vall[sb][:], pexp[:],
                                         start=(sb == 0),
                                         stop=(sb == nsb - 1))
                    rc = p5.tile([1, TOK], f32, name="rcs", tag="rcs",
                                 bufs=2)
                    nc.vector.reciprocal(rc[:], pav[64:65, :])
                    rcb = p5.tile([64, TOK], f32, name="rcb", tag="rcb",
                                  bufs=2)
                    nc.gpsimd.partition_broadcast(rcb[:], rc[:],
                                                  channels=64)
                    nc.vector.tensor_mul(yall[h][:, qcols], pav[0:64, :],
                                         rcb[:])
            dbg_dump("d_yall0", yall[0][:], [64, T_], mybir.dt.bfloat16)

            # exchange 3 + proj rhs assembly (per-head base-0 builds)
            wph = [p5.tile([64, TOK + 1], bf16, name=f"wph{h}", tag="wph",
                           bufs=8) for h in range(8)]
            yown = [p5.tile([128, TOK + 1], bf16, name=f"yown{t}")
                    for t in range(4)]
            for h in range(8):
                t = h // 2
                rows = slice(64 * (h % 2), 64 * (h % 2) + 64)
                nc.vector.tensor_scalar(wph[h][:, :],
                                        yall[h][:, TOK - 1:T_],
                                        is_first[0:64, :], None,
                                        op0=OP.mult)
                nc.vector.scalar_tensor_tensor(
                    wph[h][:, 1:TOK + 1], yall[h][:, 0:TOK],
                    is_second[0:64, :], wph[h][:, 1:TOK + 1],
                    op0=OP.mult, op1=OP.add)
                yoh = p5.tile([64, TOK + 1], bf16, name="yoh", tag="yoh",
                              bufs=2)
                nc.vector.tensor_scalar(yoh[:, :],
                                        yall[h][:, TOK - 1:T_],
                                        is_second[0:64, :], None,
                                        op0=OP.mult)
                nc.vector.scalar_tensor_tensor(
                    yoh[:, 1:TOK + 1], yall[h][:, 0:TOK],
                    is_first[0:64, :], yoh[:, 1:TOK + 1],
                    op0=OP.mult, op1=OP.add)
                nc.vector.tensor_copy(yown[t][rows, :], yoh[:])
            b3_in = dram.tile([512, TOK + 1], bf16, name="b3_in")
            b3_out = dram.tile([1024, TOK + 1], bf16, name="b3_out")
            for h in range(8):
                nc.sync.dma_start(b3_in[64 * h:64 * (h + 1), :], wph[h][:])
            nc.gpsimd.collective_compute(
                "AllGather", OP.bypass, replica_groups=RG,
                ins=[b3_in.opt()], outs=[b3_out.opt()])

            yfull = yown + [p5.tile([128, TOK + 1], bf16, name=f"yfp{t}")
                            for t in range(4)]
            for t in range(4):
                r0 = p5.tile([128, TOK + 1], bf16, name="yr0", tag="yr0",
                             bufs=2)
                r1 = p5.tile([128, TOK + 1], bf16, name="yr1", tag="yr1",
                             bufs=2)
                nc.sync.dma_start(r0[:], b3_out[128 * t:128 * (t + 1), :])
                nc.sync.dma_start(
                    r1[:], b3_out[512 + 128 * t:512 + 128 * (t + 1), :])
                nc.vector.tensor_scalar(yfull[4 + t][:], r0[:], is_second,
                                        None, op0=OP.mult)
                nc.vector.scalar_tensor_tensor(yfull[4 + t][:], r1[:],
                                               is_first, yfull[4 + t][:],
                                               op0=OP.mult, op1=OP.add)

            x2 = [pE.tile([128, TOK], f32, name=f"x2_{i}")
                  for i in range(8)]
            x2p = resid.tile([128, 8], f32, name="x2p")
            for mb in range(8):
                sl = wslab(w_proj, 128 * mb, 128, 8, f"wpj{mb}")
                pp = ps5.tile([128, TOK], f32, name="pp", tag="pbig5",
                              bufs=2)
                pp1 = ps5.tile([128, 1], f32, name="pp1", tag="pp1", bufs=1)
                for k in range(8):
                    nc.tensor.matmul(pp[:], sl[:, k, :],
                                     yfull[k][:, 1:TOK + 1],
                                     start=(k == 0), stop=(k == 7))
                    nc.tensor.matmul(pp1[:], sl[:, k, :], yfull[k][:, 0:1],
                                     start=(k == 0), stop=(k == 7))
                nc.vector.scalar_tensor_tensor(x2[mb][:], x1[mb][:], 1.0,
                                               pp[:], op0=OP.mult,
                                               op1=OP.add)
                tpv = p5.tile([128, 1], f32, name="tpv", tag="tpv", bufs=2)
                nc.vector.tensor_add(tpv[:], x1p[:, mb:mb + 1], pp1[:])
                nc.vector.tensor_scalar(x2p[:, mb:mb + 1], tpv[:],
                                        is_second, None, op0=OP.mult)
            dbg_dump("d_x2_0", x2[0][:], [128, TOK])
            ps5s.close()
            st5.close()
            stD.close()

        if PHASES >= 6:
            # ================= P6: cmix =================
            st6 = stack()
            p6 = st6.enter_context(tc.tile_pool(name="p6", bufs=1))
            ps6s = stack()
            ps6 = ps6s.enter_context(tc.tile_pool(name="ps6", bufs=1,
                                                  space="PSUM"))
            z3 = [p6.tile([128, TOK + 1], bf16, name=f"z3_{i}")
                  for i in range(8)]
            rmsnorm_cm([x2[i][:] for i in range(8)],
                       [z3[i][:, 1:TOK + 1] for i in range(8)], TOK, p6,
                       ps6, C_, "n2")
            sqp = p6.tile([128, 8], bf16, name="sqp")
            nc.vector.tensor_mul(sqp[:], x2p[:], x2p[:])
            psp = ps6.tile([1, 8], f32, name="psp", tag="psp", bufs=1)
            nc.tensor.matmul(psp[:], onesb[:], sqp[:], start=True,
                             stop=True)
            ssp = p6.tile([1, 1], f32, name="ssp")
            nc.vector.tensor_reduce(ssp[:], psp[:],
                                    axis=mybir.AxisListType.X, op=OP.add)
            nc.scalar.activation(ssp[:], ssp[:], AF.Sqrt,
                                 bias=eps_c[0:1, :], scale=1.0 / C_)
            nc.vector.reciprocal(ssp[:], ssp[:])
            rpb = p6.tile([128, 1], f32, name="rpb")
            nc.gpsimd.partition_broadcast(rpb[:], ssp[:], channels=128)
            for i in range(8):
                nc.vector.scalar_tensor_tensor(z3[i][:, 0:1],
                                               x2p[:, i:i + 1], 1.0,
                                               rpb[:], op0=OP.mult,
                                               op1=OP.mult)
            dbg_dump("d_z3_0", z3[0][:], [128, TOK + 1], mybir.dt.bfloat16)

            xk = [p6.tile([128, TOK], bf16, name=f"xk{i}")
                  for i in range(8)]
            xr = [p6.tile([128, TOK], bf16, name=f"xr{i}")
                  for i in range(8)]
            for i in range(8):
                nc.vector.tensor_scalar(xk[i][:], z3[i][:, 1:TOK + 1],
                                        mk1_s[:, i:i + 1], None,
                                        op0=OP.mult)
                nc.vector.scalar_tensor_tensor(xk[i][:], z3[i][:, 0:TOK],
                                               mk_s[:, i:i + 1], xk[i][:],
                                               op0=OP.mult, op1=OP.add)
                nc.vector.tensor_scalar(xr[i][:], z3[i][:, 1:TOK + 1],
                                        mr1_s[:, i:i + 1], None,
                                        op0=OP.mult)
                nc.vector.scalar_tensor_tensor(xr[i][:], z3[i][:, 0:TOK],
                                               mr_s[:, i:i + 1], xr[i][:],
                                               op0=OP.mult, op1=OP.add)

            kE = [p6.tile([128, TOK], bf16, name=f"kE{i}")
                  for i in range(32)]
            for mb in range(32):
                sl = wslab(w_key, 128 * mb, 128, 8, f"wky{mb}")
                pky = ps6.tile([128, TOK], f32, name="pky", tag="pbig6",
                               bufs=3)
                for k in range(8):
                    nc.tensor.matmul(pky[:], sl[:, k, :], xk[k][:],
                                     start=(k == 0), stop=(k == 7))
                nc.scalar.activation(kE[mb][:], pky[:], AF.Erf,
                                     scale=1.0 / _DEN, bias=erfb_c[:, :])
            r_sb = [p6.tile([128, TOK], bf16, name=f"r_sb{i}")
                    for i in range(8)]
            for mb in range(8):
                sl = wslab(w_rec, 128 * mb, 128, 8, f"wrc{mb}")
                pr = ps6.tile([128, TOK], f32, name="pr", tag="pbig6",
                              bufs=3)
                for k in range(8):
                    nc.tensor.matmul(pr[:], sl[:, k, :], xr[k][:],
                                     start=(k == 0), stop=(k == 7))
                nc.scalar.activation(r_sb[mb][:], pr[:], AF.Sigmoid)
            dbg_dump("d_kE0", kE[0][:], [128, TOK], mybir.dt.bfloat16)
            dbg_dump("d_r0", r_sb[0][:], [128, TOK], mybir.dt.bfloat16)

            for mb in range(8):
                slab = wslab(w_val, 128 * mb, 128, 32, f"wvl{mb}", pool=p6,
                             tag="wslab_v", bufs=2)
                pvv = ps6.tile([128, TOK], f32, name="pvv", tag="pbig6",
                               bufs=3)
                for k in range(32):
                    nc.tensor.matmul(pvv[:], slab[:, k, :], kE[k][:],
                                     start=(k == 0), stop=(k == 31))
                tmpv = p6.tile([128, TOK], f32, name="tmpv", tag="tmpv",
                               bufs=2)
                nc.vector.tensor_scalar(tmpv[:], pvv[:],
                                        vbias_s[:, mb:mb + 1], None,
                                        op0=OP.add)
                nc.vector.tensor_mul(tmpv[:], tmpv[:], r_sb[mb][:])
                outt = p6.tile([128, TOK], f32, name="outt", tag="outt",
                               bufs=2)
                nc.vector.tensor_add(outt[:], x2[mb][:], tmpv[:])
                nc.sync.dma_start(out_d[128 * mb:128 * (mb + 1), :],
                                  outt[:])
            ps6s.close()
            st6.close()

        for s in reversed(_open):
            s.close()
        whole.close()

    nc.compile()
    return nc, dbg_outs


# ================= host glue =================

def _prep_inputs(x, in_proj_w, conv_w, conv_b, dt_bias, A_log, D, mnorm_w,
                 out_proj_w, attn_w, proj_w, time_maa_k, time_maa_r, key_w,
                 recept_w, value_w):
    f32 = np.float32

    def b(a):
        return np.ascontiguousarray(np.asarray(a, f32).astype(BF16))

    x = np.asarray(x, f32)
    shared = {
        "w_inproj": b(in_proj_w),
        "convw": np.ascontiguousarray(
            np.asarray(conv_w, f32).reshape(17, 128, DCONV)
            .transpose(1, 0, 2)),
        "convb": np.ascontiguousarray(
            np.asarray(conv_b, f32).reshape(17, 128).T),
        "dtb": np.ascontiguousarray(
            np.asarray(dt_bias, f32).reshape(NHM, 1)),
        "aneg": np.ascontiguousarray(
            (-np.exp(np.asarray(A_log, f32))).reshape(NHM, 1)),
        # drep[p, k] = D[2k + (p >= 64)]
        "drep": np.ascontiguousarray(np.stack(
            [np.concatenate([np.full(64, D2[0]), np.full(64, D2[1])])
             for D2 in np.asarray(D, f32).reshape(16, 2)], axis=1)
            .astype(f32)),
        "mnw": np.ascontiguousarray(
            np.asarray(mnorm_w, f32).reshape(16, 128).T),
        "w_outproj": b(out_proj_w),
        "mk": np.ascontiguousarray(
            np.asarray(time_maa_k, f32).reshape(8, 128).T),
        "mk1": np.ascontiguousarray(
            (1.0 - np.asarray(time_maa_k, f32)).reshape(8, 128).T),
        "mr": np.ascontiguousarray(
            np.asarray(time_maa_r, f32).reshape(8, 128).T),
        "mr1": np.ascontiguousarray(
            (1.0 - np.asarray(time_maa_r, f32)).reshape(8, 128).T),
        "w_key": b(key_w),
        "w_val": b(0.5 * np.asarray(value_w, f32)),
        "vbias": np.ascontiguousarray(
            (0.5 * np.asarray(value_w, f32).sum(0)).reshape(8, 128).T),
        "w_rec": b(recept_w),
    }
    ef = np.zeros((NHM, DIN), f32)
    for k in range(16):
        ef[2 * k, 128 * k:128 * k + 64] = 1.0
        ef[2 * k + 1, 128 * k + 64:128 * k + 128] = 1.0
    shared["efull"] = ef

    attn_w = np.asarray(attn_w, f32)
    proj_w = np.asarray(proj_w, f32)
    scale = 1.0 / np.sqrt(np.float32(HD))
    in_maps = []
    for core in range(N_CORES):
        bi, half = core // 2, core % 2
        start = half * TOK
        xcm = x[bi].T
        xs = np.zeros((C_, TH), f32)
        xs[:, 3:] = xcm[:, start:start + TOK]
        if start >= 3:
            xs[:, 0:3] = xcm[:, start - 3:start]
        myh = np.arange(8 * half, 8 * half + 8)
        oth = np.arange(8 * (1 - half), 8 * (1 - half) + 8)
        qcols = attn_w[:, :C_].reshape(C_, NH, HD)
        wq_perm = np.concatenate(
            [qcols[:, myh].reshape(C_, 512),
             qcols[:, oth].reshape(C_, 512)], axis=1) * scale
        w_att_c = np.concatenate([wq_perm, attn_w[:, C_:]], axis=1)
        prows = proj_w.reshape(NH, HD, C_)
        w_proj_c = np.concatenate(
            [prows[myh].reshape(512, C_), prows[oth].reshape(512, C_)],
            axis=0)
        mskc = np.zeros((128, 2), f32)
        mskc[:, 0] = 1.0 - half
        mskc[:, 1] = half
        m = dict(shared)
        m["xin"] = np.ascontiguousarray(xs)
        m["w_att"] = np.ascontiguousarray(w_att_c.astype(BF16))
        m["w_proj"] = np.ascontiguousarray(w_proj_c.astype(BF16))
        m["msk"] = mskc
        in_maps.append(m)
    return in_maps


def kernel(**inputs):
    from concourse.bass_utils import run_bass_kernel_spmd

    if "nc" not in _CACHE:
        _CACHE["nc"], _CACHE["dbg"] = _build()
    nc = _CACHE["nc"]
    in_maps = _prep_inputs(**inputs)
    res = run_bass_kernel_spmd(nc, in_maps, core_ids=list(range(N_CORES)))
    _CACHE["results"] = res
    out = np.empty((B_, T_, C_), np.float32)
    for core in range(N_CORES):
        bi, half = core // 2, core % 2
        out[bi, half * TOK:(half + 1) * TOK, :] = \
            np.asarray(res.results[core]["out"], np.float32).T
    return out


# revision 31
# speedup vs baseline: 1.1109x; 1.0214x over previous
"""nn_Block_21062519619681 fully on-device: hybrid Mamba2 + MQA + RWKV-CMix
block as ONE Bass/Tile SPMD kernel on 8 trn2 NeuronCores.

Sharding: 8 cores = 4 batches x 2 token-halves (512 own tokens/core).
 - mamba: token-sharded; chunked-SSD scan (L=128); cross-half state carry via
   a pairwise AllGather applied as a linear correction pass.
 - attention: q-head-split (8 heads/core over ALL 1024 tokens; per-core
   permuted q/proj weights keep the SPMD graph rank-uniform); k/v + q halves
   exchanged via pairwise AllGather; softmax without max-subtraction (scores
   bounded); colsum ridden as a ones-column in the av matmul.
 - cmix: token-sharded, replicated weights, erf/sigmoid fused into PSUM evac.
All matmuls bf16 (weights pre-cast on host), fp32 PSUM accumulate, fp32
residual stream. Rank-dependent selection uses host-fed 0/1 masks (masked
sums) - the instruction graph is identical on all cores.
"""
import os
import sys

sys.path.insert(0, "/opt/trn_rl_repo")
import numpy as np
import ml_dtypes

B_, T_, C_ = 4, 1024, 1024
NH, HD = 16, 64
DS, DCONV, EXP, PHD = 64, 4, 2, 64
DIN = EXP * C_
NHM = DIN // PHD
CONVD = DIN + 2 * DS
FFN = 4 * C_
EPS = 1e-5
N_CORES = 8
TOK = 512
TH = TOK + 3
L = 128
NCH = TOK // L
NEG = -1e30

BF16 = ml_dtypes.bfloat16
DEBUG = bool(int(os.environ.get("BASSK_DEBUG", "0")))
PHASES = int(os.environ.get("BASSK_PHASES", "6"))

_CACHE = {}


def _build():
    import contextlib
    import concourse.mybir as mybir
    import concourse.bacc as bacc
    import concourse.tile as tile
    from concourse.masks import make_identity

    f32 = mybir.dt.float32
    bf16 = mybir.dt.bfloat16
    AF = mybir.ActivationFunctionType
    OP = mybir.AluOpType

    nc = bacc.Bacc("TRN2", target_bir_lowering=False, debug=False,
                   num_devices=N_CORES)

    def din(name, shape, dt=bf16):
        return nc.dram_tensor(name, shape, dt, kind="ExternalInput").ap()

    xin = din("xin", [C_, TH], f32)
    w_inproj = din("w_inproj", [C_, 4256])
    convw = din("convw", [128, 17, DCONV], f32)
    convb = din("convb", [128, 17], f32)
    dtb = din("dtb", [NHM, 1], f32)
    aneg = din("aneg", [NHM, 1], f32)
    drep = din("drep", [128, 16], f32)
    mnw = din("mnw", [128, 16], f32)
    w_outproj = din("w_outproj", [DIN, C_])
    w_att = din("w_att", [C_, 1024 + 128])
    w_proj = din("w_proj", [C_, C_])
    mk = din("mk", [128, 8], f32)
    mk1 = din("mk1", [128, 8], f32)
    mr = din("mr", [128, 8], f32)
    mr1 = din("mr1", [128, 8], f32)
    w_key = din("w_key", [C_, FFN])
    w_val = din("w_val", [FFN, C_])
    vbias = din("vbias", [128, 8], f32)
    w_rec = din("w_rec", [C_, C_])
    msk = din("msk", [128, 2], f32)
    efull = din("efull", [NHM, DIN])

    out_d = nc.dram_tensor("out", [C_, TOK], f32, kind="ExternalOutput").ap()

    dbg_outs = {}

    def dbg_dump(name, ap_or_tile, shape, dt=None):
        if not DEBUG:
            return
        d = nc.dram_tensor(name, shape, dt or mybir.dt.float32,
                           kind="ExternalOutput").ap()
        dbg_outs[name] = d
        nc.sync.dma_start(d, ap_or_tile)

    RG = [[0, 1], [2, 3], [4, 5], [6, 7]]

    with tile.TileContext(nc) as tc:
        _open = []

        def stack():
            s = contextlib.ExitStack()
            _open.append(s)
            return s

        whole = contextlib.ExitStack()
        consts = whole.enter_context(tc.tile_pool(name="consts", bufs=1))
        resid = whole.enter_context(tc.tile_pool(name="resid", bufs=1))
        wsl = whole.enter_context(tc.tile_pool(name="wsl", bufs=6))
        dram = whole.enter_context(tc.tile_pool(name="dram", bufs=1,
                                                space="DRAM"))

        # ---------------- constants ----------------
        ident_b = consts.tile([128, 128], bf16, name="ident_b")
        make_identity(nc, ident_b)
        ident_f = consts.tile([128, 128], f32, name="ident_f")
        make_identity(nc, ident_f)
        tri01 = consts.tile([128, 128], bf16, name="tri01")
        nc.vector.memset(tri01, 1.0)
        nc.gpsimd.affine_select(out=tri01, in_=tri01, compare_op=OP.is_ge,
                                fill=0.0, base=0, channel_multiplier=-1,
                                pattern=[[1, 128]])
        onesb = consts.tile([128, 1], bf16, name="onesb")
        nc.vector.memset(onesb, 1.0)
        onesf_r = consts.tile([1, 64], f32, name="onesf_r")
        nc.vector.memset(onesf_r, 1.0)
        ones32 = consts.tile([NHM, L], f32, name="ones32")
        nc.vector.memset(ones32, 1.0)
        eps_c = consts.tile([128, 1], f32, name="eps_c")
        nc.vector.memset(eps_c, EPS)
        _MU = float(np.sqrt(0.5))
        _DEN = float(np.sqrt(1.0 / (4.0 * np.pi)) * np.sqrt(2.0))
        erfb_c = consts.tile([128, 1], f32, name="erfb_c")
        nc.vector.memset(erfb_c, -_MU / _DEN)

        def cin(name, shape, src, dt=f32):
            t = consts.tile(list(shape), dt, name=name)
            nc.sync.dma_start(t[:], src)
            return t

        convw_s = cin("convw_s", [128, 17, DCONV], convw)
        convb_s = cin("convb_s", [128, 17], convb)
        dtb_s = cin("dtb_s", [NHM, 1], dtb)
        aneg_s = cin("aneg_s", [NHM, 1], aneg)
        drep_s = cin("drep_s", [128, 16], drep)
        mnw_s = cin("mnw_s", [128, 16], mnw)
        mk_s = cin("mk_s", [128, 8], mk)
        mk1_s = cin("mk1_s", [128, 8], mk1)
        mr_s = cin("mr_s", [128, 8], mr)
        mr1_s = cin("mr1_s", [128, 8], mr1)
        vbias_s = cin("vbias_s", [128, 8], vbias)
        msk_s = cin("msk_s", [128, 2], msk)
        efull_s = cin("efull_s", [NHM, DIN], efull, dt=bf16)
        is_first = msk_s[:, 0:1]
        is_second = msk_s[:, 1:2]

        zsil_d = dram.tile([DIN, TOK], bf16, name="zsil_d")
        rinv1_dd = dram.tile([1, TOK], f32, name="rinv1_dd")

        def wslab(wt, m0, mw, kt, name, pool=None, tag="wslab", bufs=None,
                  r0=0):
            s = (pool or wsl).tile([128, kt, mw], bf16, name=name, tag=tag,
                                   bufs=bufs)
            nc.sync.dma_start(
                s[:], wt[r0:r0 + 128 * kt, m0:m0 + mw]
                .rearrange("(t p) m -> p t m", p=128))
            return s

        def rmsnorm_cm(src_aps, dst_aps, width, pool, psp, nfeat, tag):
            ssq = psp.tile([1, width], f32, name=f"ssq_{tag}",
                           tag=f"ssq{tag}", bufs=1)
            n = len(src_aps)
            for i, sap in enumerate(src_aps):
                sq = pool.tile([128, width], bf16, name=f"sq_{tag}",
                               tag=f"sq{tag}", bufs=2)
                nc.vector.tensor_mul(sq[:], sap, sap)
                nc.tensor.matmul(ssq[:], onesb[:], sq[:], start=(i == 0),
                                 stop=(i == n - 1))
            rms = pool.tile([1, width], f32, name=f"rms_{tag}",
                            tag=f"rms{tag}", bufs=1)
            nc.scalar.activation(rms[:], ssq[:], AF.Sqrt,
                                 bias=eps_c[0:1, :], scale=1.0 / nfeat)
            rinv = pool.tile([1, width], f32, name=f"rinv_{tag}",
                             tag=f"rinv{tag}", bufs=1)
            nc.vector.reciprocal(rinv[:], rms[:])
            rbc = pool.tile([128, width], f32, name=f"rbc_{tag}",
                            tag=f"rbc{tag}", bufs=1)
            nc.gpsimd.partition_broadcast(rbc[:], rinv[:], channels=128)
            for i, sap in enumerate(src_aps):
                nc.vector.tensor_mul(dst_aps[i], sap, rbc[:])

        # pool nesting (open early -> close late):
        stY = stack()
        pY = stY.enter_context(tc.tile_pool(name="pY", bufs=1))   # ..P4
        stB = stack()
        pB = stB.enter_context(tc.tile_pool(name="pB", bufs=1))   # ..P3
        stC = stack()
        pC = stC.enter_context(tc.tile_pool(name="pC", bufs=1))   # ..P3

        # ================= P0 + P1: rmsnorm + in_proj =================
        st01 = stack()
        pA = st01.enter_context(tc.tile_pool(name="pA", bufs=1))
        ps01 = st01.enter_context(tc.tile_pool(name="ps01", bufs=1,
                                               space="PSUM"))
        xn = [pA.tile([128, TH], bf16, name=f"xn{i}") for i in range(8)]
        # streaming rmsnorm over x (full TH width, stats on own 512 cols)
        ssqx = ps01.tile([1, 512], f32, name="ssqx", tag="ssqx", bufs=1)
        ssqh = ps01.tile([1, 3], f32, name="ssqh", tag="ssqh", bufs=1)
        for i in range(8):
            xt = pA.tile([128, TH], f32, name="xt", tag="xt", bufs=3)
            nc.sync.dma_start(xt[:], xin[128 * i:128 * (i + 1), :])
            sqx = pA.tile([128, TH], bf16, name="sqx", tag="sqx", bufs=2)
            nc.vector.tensor_mul(sqx[:], xt[:], xt[:])
            nc.tensor.matmul(ssqx[:], onesb[:], sqx[:, 3:TH],
                             start=(i == 0), stop=(i == 7))
            nc.tensor.matmul(ssqh[:], onesb[:], sqx[:, 0:3],
                             start=(i == 0), stop=(i == 7))
        rmsx = pA.tile([1, TH], f32, name="rmsx")
        nc.scalar.activation(rmsx[:, 3:TH], ssqx[:], AF.Sqrt,
                             bias=eps_c[0:1, :], scale=1.0 / C_)
        nc.scalar.activation(rmsx[:, 0:3], ssqh[:], AF.Sqrt,
                             bias=eps_c[0:1, :], scale=1.0 / C_)
        rinvx = pA.tile([1, TH], f32, name="rinvx")
        nc.vector.reciprocal(rinvx[:], rmsx[:])
        rbcx = pA.tile([128, TH], f32, name="rbcx")
        nc.gpsimd.partition_broadcast(rbcx[:], rinvx[:], channels=128)
        for i in range(8):
            xt = pA.tile([128, TH], f32, name="xt", tag="xt", bufs=3)
            nc.sync.dma_start(xt[:], xin[128 * i:128 * (i + 1), :])
            nc.vector.tensor_mul(xn[i][:], xt[:], rbcx[:])
        dbg_dump("d_xn0", xn[0][:], [128, TH], mybir.dt.bfloat16)

        xbc = [pB.tile([128, TH], bf16, name=f"xbc{i}") for i in range(17)]
        dtraw = pB.tile([NHM, TOK], f32, name="dtraw")

        for mb in range(16):
            sl = wslab(w_inproj, 128 * mb, 128, 8, f"wz{mb}")
            pz = ps01.tile([128, TOK], f32, name="pz", tag="pbig", bufs=4)
            for k in range(8):
                nc.tensor.matmul(pz[:], sl[:, k, :], xn[k][:, 3:TH],
                                 start=(k == 0), stop=(k == 7))
            zst = pA.tile([128, TOK], bf16, name="zst", tag="zst", bufs=3)
            nc.scalar.activation(zst[:], pz[:], AF.Silu)
            nc.sync.dma_start(zsil_d[128 * mb:128 * (mb + 1), :], zst[:])
        for mb in range(17):
            sl = wslab(w_inproj, DIN + 128 * mb, 128, 8, f"wxbc{mb}")
            pb_ = ps01.tile([128, TOK], f32, name="pb", tag="pbig", bufs=4)
            ph = ps01.tile([128, 3], f32, name="ph", tag="phalo", bufs=1)
            for k in range(8):
                nc.tensor.matmul(pb_[:], sl[:, k, :], xn[k][:, 3:TH],
                                 start=(k == 0), stop=(k == 7))
                nc.tensor.matmul(ph[:], sl[:, k, :], xn[k][:, 0:3],
                                 start=(k == 0), stop=(k == 7))
            nc.scalar.copy(xbc[mb][:, 3:TH], pb_[:])
            nc.vector.tensor_copy(xbc[mb][:, 0:3], ph[:])
        sl = wslab(w_inproj, 4224, 32, 8, "wdtp")
        pdt = ps01.tile([NHM, TOK], f32, name="pdt", tag="pdt", bufs=1)
        for k in range(8):
            nc.tensor.matmul(pdt[:], sl[:, k, :], xn[k][:, 3:TH],
                             start=(k == 0), stop=(k == 7))
        nc.vector.tensor_copy(dtraw[:], pdt[:])
        dbg_dump("d_xbc0", xbc[0][:], [128, TH], mybir.dt.bfloat16)
        st01.close()

        if PHASES >= 2:
            # ============ P2: conv + dt pipeline + transposes ============
            ps2 = stack()
            ps2p = ps2.enter_context(tc.tile_pool(name="ps2", bufs=1,
                                                  space="PSUM"))
            xs_cm = [pC.tile([128, TOK], bf16, name=f"xs_cm{i}")
                     for i in range(17)]
            for i in range(17):
                tmp = pC.tile([128, TOK], f32, name="ctmp", tag="ctmp",
                              bufs=3)
                nc.vector.tensor_scalar(tmp[:], xbc[i][:, 0:TOK],
                                        convw_s[:, i, 0:1], None,
                                        op0=OP.mult)
                for j in range(1, DCONV):
                    nc.vector.scalar_tensor_tensor(
                        tmp[:], xbc[i][:, j:j + TOK], convw_s[:, i, j:j + 1],
                        tmp[:], op0=OP.mult, op1=OP.add)
                nc.scalar.activation(xs_cm[i][:], tmp[:], AF.Silu,
                                     bias=convb_s[:, i:i + 1])
            C_cm = pC.tile([64, TOK], bf16, name="C_cm")
            nc.vector.tensor_copy(C_cm[:], xs_cm[16][64:128, :])
            dbg_dump("d_xs0", xs_cm[0][:], [128, TOK], mybir.dt.bfloat16)

            dt_f = pC.tile([NHM, TOK], f32, name="dt_f")
            Lc = pC.tile([NHM, TOK], f32, name="Lc")
            wdt = pC.tile([NHM, TOK], f32, name="wdt", tag="scr", bufs=2)
            u = pC.tile([NHM, TOK], f32, name="u")
            ex = pC.tile([NHM, TOK], f32, name="ex", tag="scr", bufs=2)
            nc.vector.tensor_scalar(u[:], dtraw[:], dtb_s[:], None,
                                    op0=OP.add)
            ab = pC.tile([NHM, TOK], f32, name="ab", tag="scr", bufs=2)
            nc.vector.tensor_scalar(ab[:], u[:], -1.0, None, op0=OP.mult)
            nc.vector.tensor_max(ab[:], ab[:], u[:])
            nc.scalar.activation(ex[:], ab[:], AF.Exp, scale=-1.0)
            nc.scalar.activation(ex[:], ex[:], AF.Ln, bias=1.0)
            nc.vector.tensor_scalar(dt_f[:], u[:], 0.0, None, op0=OP.max)
            nc.vector.tensor_add(dt_f[:], dt_f[:], ex[:])
            dta = u
            nc.vector.tensor_scalar(dta[:], dt_f[:], aneg_s[:], None,
                                    op0=OP.mult)
            for c in range(NCH):
                cs = slice(L * c, L * (c + 1))
                nc.vector.tensor_tensor_scan(Lc[:, cs], ones32[:],
                                             dta[:, cs], 0.0, op0=OP.mult,
                                             op1=OP.add)
                nc.scalar.activation(wdt[:, cs], Lc[:, cs], AF.Exp,
                                     scale=-1.0,
                                     bias=Lc[:, L * (c + 1) - 1:L * (c + 1)])
            nc.vector.tensor_mul(wdt[:], wdt[:], dt_f[:])

            dbg_dump("d_dt", dt_f[:], [NHM, TOK])
            dbg_dump("d_Lc", Lc[:], [NHM, TOK])

            eLcE = pC.tile([NHM, NCH], bf16, name="eLcE")
            lce = pC.tile([NHM, NCH], f32, name="lce")
            for c in range(NCH):
                nc.vector.tensor_copy(lce[:, c:c + 1],
                                      Lc[:, L * (c + 1) - 1:L * (c + 1)])
            nc.scalar.activation(eLcE[:], lce[:], AF.Exp)
            arep = [pC.tile([128, NCH], f32, name=f"arep{k}")
                    for k in range(16)]
            carep = [pC.tile([128, NCH], f32, name=f"carep{k}")
                     for k in range(16)]
            for k in range(16):
                pa = ps2p.tile([128, NCH], f32, name="pa", tag="pa", bufs=2)
                nc.tensor.matmul(pa[:], efull_s[:, 128 * k:128 * (k + 1)],
                                 eLcE[:], start=True, stop=True)
                nc.vector.tensor_copy(arep[k][:], pa[:])
                nc.vector.memset(carep[k][:, 0:1], 1.0)
                for c in range(1, NCH):
                    nc.vector.tensor_mul(carep[k][:, c:c + 1],
                                         carep[k][:, c - 1:c],
                                         arep[k][:, c - 1:c])

            tmv = [pC.tile([128, 96], f32, name=f"tmv{c}")
                   for c in range(NCH)]
            stk = pC.tile([96, TOK], f32, name="stk")
            nc.vector.tensor_scalar(stk[0:NHM, :], Lc[:], -1.0, None,
                                    op0=OP.mult)
            nc.vector.tensor_copy(stk[NHM:2 * NHM, :], dt_f[:])
            nc.vector.tensor_copy(stk[2 * NHM:3 * NHM, :], wdt[:])
            for c in range(NCH):
                pt = ps2p.tile([128, 96], f32, name="pt", tag="ptr", bufs=2)
                nc.tensor.transpose(pt[:], stk[:, L * c:L * (c + 1)],
                                    ident_f[0:96, 0:96])
                nc.vector.tensor_copy(tmv[c][:], pt[:])

            xs_tm = [pC.tile([128, 2176], bf16, name=f"xs_tm{c}")
                     for c in range(NCH)]
            for c in range(NCH):
                for i in range(17):
                    ptb = ps2p.tile([128, 128], bf16, name="ptb", tag="ptrb",
                                    bufs=3)
                    nc.tensor.transpose(ptb[:],
                                        xs_cm[i][:, L * c:L * (c + 1)],
                                        ident_b[:])
                    nc.vector.tensor_copy(
                        xs_tm[c][:, 128 * i:128 * (i + 1)], ptb[:])
            dbg_dump("d_xstm0", xs_tm[0][:], [128, 2176], mybir.dt.bfloat16)
            ps2.close()

        if PHASES >= 3:
            # ========== P3: scan (interleaved per chunk) ==========
            ps3 = stack()
            ps3p = ps3.enter_context(tc.tile_pool(name="ps3", bufs=1,
                                                  space="PSUM"))
            state = [pC.tile([128, PHD], f32, name=f"state{k}")
                     for k in range(16)]
            for k in range(16):
                nc.vector.memset(state[k][:], 0.0)
            stateb = [pC.tile([64, PHD], bf16, name=f"stateb{h}")
                      for h in range(NHM)]
            ycm = [pY.tile([128, TOK], bf16, name=f"ycm{k}")
                   for k in range(16)]

            def stage_bcast(lcf, hh, with_exp=True, channels=128):
                lba = pC.tile([channels, 16 * L], f32, name="lba",
                              tag="lball", bufs=1)
                nc.gpsimd.partition_broadcast(
                    lba[:], lcf[0:1, 16 * L * hh:16 * L * (hh + 1)],
                    channels=channels)
                eba = None
                if with_exp:
                    eba = pC.tile([64, 16 * L], bf16, name="eba",
                                  tag="eball", bufs=1)
                    nc.scalar.activation(eba[:], lba[0:64, :], AF.Exp)
                return lba, eba

            def make_cdec_dve(eba, h, cs):
                off = L * (h % 16)
                cd = pC.tile([64, L], bf16, name="cd", tag="cdec", bufs=4)
                nc.vector.tensor_mul(cd[:], C_cm[:, cs],
                                     eba[:, off:off + L])
                return cd

            def make_cdec(eba, h, cs):
                off = L * (h % 16)
                cd = pC.tile([64, L], bf16, name="cd", tag="cdec", bufs=4)
                nc.gpsimd.tensor_mul(cd[:], C_cm[:, cs],
                                     eba[:, off:off + L])
                return cd

            def stage_lc(c):
                t = pC.tile([1, NHM * L], f32, name=f"LcFc{c}",
                            tag="lcf", bufs=2)
                nc.sync.dma_start(t[0:1, :], Lc[:, L * c:L * (c + 1)])
                return t

            for c in range(NCH):
                cs = slice(L * c, L * (c + 1))
                lcf = stage_lc(c)
                if c > 0:
                    for h in range(NHM):
                        nc.gpsimd.tensor_copy(
                            stateb[h][:],
                            state[h // 2][64 * (h % 2):64 * (h % 2) + 64, :])
                pg = ps3p.tile([128, L], f32, name="pg", tag="pg", bufs=1)
                nc.tensor.matmul(pg[:], xs_cm[16][0:64, cs], C_cm[:, cs],
                                 start=True, stop=True)
                gts = pC.tile([128, L], bf16, name="gts", tag="gts", bufs=2)
                nc.vector.tensor_mul(gts[:], pg[:], tri01[:])
                lba = eba = None
                for h in range(NHM):
                    k = h // 2
                    rows = slice(64 * (h % 2), 64 * (h % 2) + 64)
                    if h % 16 == 0:
                        lba, eba = stage_bcast(lcf, h // 16,
                                               with_exp=(c > 0))
                    darg = pC.tile([128, L], f32, name="darg", tag="darg",
                                   bufs=4)
                    nc.vector.tensor_scalar(darg[:],
                                            lba[:, L * (h % 16):
                                                L * (h % 16) + L],
                                            tmv[c][:, h:h + 1], 0.0,
                                            op0=OP.add, op1=OP.min)
                    expd = pC.tile([128, L], f32, name="expd", tag="expd",
                                   bufs=4)
                    nc.scalar.activation(expd[:], darg[:], AF.Exp)
                    mt = pC.tile([128, L], bf16, name="mt", tag="mt", bufs=4)
                    nc.vector.scalar_tensor_tensor(
                        mt[:], gts[:], tmv[c][:, 32 + h:33 + h], expd[:],
                        op0=OP.mult, op1=OP.mult)
                    py = ps3p.tile([64, L], f32, name="py", tag="py", bufs=2)
                    nc.tensor.matmul(py[:],
                                     xs_tm[c][:, PHD * h:PHD * (h + 1)],
                                     mt[:], start=True, stop=(c == 0))
                    if c > 0:
                        cd = make_cdec(eba, h, cs)
                        nc.tensor.matmul(py[:], stateb[h][:], cd[:],
                                         start=False, stop=True)
                    nc.vector.scalar_tensor_tensor(
                        ycm[k][rows, cs], xs_cm[k][rows, cs],
                        drep_s[rows, k:k + 1], py[:], op0=OP.mult,
                        op1=OP.add)
                    bw = pC.tile([128, DS], bf16, name="bw", tag="bw",
                                 bufs=3)
                    nc.gpsimd.tensor_scalar(
                        bw[:], xs_tm[c][:, DIN:DIN + DS],
                        tmv[c][:, 64 + h:65 + h], None, op0=OP.mult)
                    psc = ps3p.tile([64, PHD], f32, name="psc", tag="psc",
                                    bufs=2)
                    nc.tensor.matmul(psc[:], bw[:],
                                     xs_tm[c][:, PHD * h:PHD * (h + 1)],
                                     start=True, stop=True)
                    nc.vector.scalar_tensor_tensor(
                        state[k][rows, :], state[k][rows, :],
                        arep[k][rows, c:c + 1], psc[:], op0=OP.mult,
                        op1=OP.add)

            b1_in = dram.tile([128, 16 * PHD], bf16, name="b1_in")
            b1_out = dram.tile([256, 16 * PHD], bf16, name="b1_out")
            steb = pC.tile([128, 16 * PHD], bf16, name="steb")
            for k in range(16):
                nc.vector.tensor_copy(steb[:, PHD * k:PHD * (k + 1)],
                                      state[k][:])
            nc.sync.dma_start(b1_in[:], steb[:])
            nc.gpsimd.collective_compute(
                "AllGather", OP.bypass, replica_groups=RG,
                ins=[b1_in.opt()], outs=[b1_out.opt()])
            dbg_dump("d_st0", state[0][:], [128, PHD])

            h0bf2 = [pC.tile([64, PHD], bf16, name=f"h0bf2{h}")
                     for h in range(NHM)]
            for k in range(16):
                rcv = pC.tile([128, PHD], bf16, name="rcv", tag="rcv",
                              bufs=2)
                nc.sync.dma_start(rcv[:],
                                  b1_out[0:128, PHD * k:PHD * (k + 1)])
                for j in (0, 1):
                    nc.vector.tensor_scalar(
                        h0bf2[2 * k + j][:], rcv[64 * j:64 * j + 64, :],
                        is_second[0:64, :], None, op0=OP.mult)
            for c in range(NCH):
                cs = slice(L * c, L * (c + 1))
                lcf2 = stage_lc(c)
                eba2 = None
                for h in range(NHM):
                    k = h // 2
                    rows = slice(64 * (h % 2), 64 * (h % 2) + 64)
                    if h % 16 == 0:
                        _, eba2 = stage_bcast(lcf2, h // 16)
                    cd = make_cdec(eba2, h, cs)
                    pyc = ps3p.tile([64, L], f32, name="pyc", tag="pyc",
                                    bufs=3)
                    nc.tensor.matmul(pyc[:], h0bf2[h][:], cd[:], start=True,
                                     stop=True)
                    # ycm += cumalpha * (h0^T @ Cdec)
                    nc.vector.scalar_tensor_tensor(
                        ycm[k][rows, cs], pyc[:],
                        carep[k][rows, c:c + 1], ycm[k][rows, cs],
                        op0=OP.mult, op1=OP.add)
            dbg_dump("d_y0", ycm[0][:], [128, TOK], mybir.dt.bfloat16)
            ps3.close()
            stC.close()
            stB.close()

        if PHASES >= 4:
            # ======== P4: gated norm + out_proj + x1 + rmsnorm2 ========
            stE = stack()
            pE = stE.enter_context(tc.tile_pool(name="pE", bufs=1))  # ..P6
            stD = stack()
            pD = stD.enter_context(tc.tile_pool(name="pD", bufs=1))  # ..P5
            st4 = stack()
            p4 = st4.enter_context(tc.tile_pool(name="p4", bufs=1))
            ps4s = stack()
            ps4 = ps4s.enter_context(tc.tile_pool(name="ps4", bufs=1,
                                                  space="PSUM"))
            g = [p4.tile([128, TOK], bf16, name=f"g{k}") for k in range(16)]
            for k in range(16):
                zs = p4.tile([128, TOK], bf16, name="zs", tag="zs", bufs=3)
                nc.sync.dma_start(zs[:], zsil_d[128 * k:128 * (k + 1), :])
                nc.vector.tensor_mul(g[k][:], ycm[k][:], zs[:])
            ssq = ps4.tile([1, TOK], f32, name="ssqg", tag="ssqg", bufs=1)
            for k in range(16):
                sq = p4.tile([128, TOK], bf16, name="gsq", tag="gsq", bufs=2)
                nc.vector.tensor_mul(sq[:], g[k][:], g[k][:])
                nc.tensor.matmul(ssq[:], onesb[:], sq[:], start=(k == 0),
                                 stop=(k == 15))
            rms = p4.tile([1, TOK], f32, name="grms")
            nc.scalar.activation(rms[:], ssq[:], AF.Sqrt,
                                 bias=eps_c[0:1, :], scale=1.0 / DIN)
            rinv = p4.tile([1, TOK], f32, name="grinv")
            nc.vector.reciprocal(rinv[:], rms[:])
            rbc = p4.tile([128, TOK], f32, name="grbc")
            nc.gpsimd.partition_broadcast(rbc[:], rinv[:], channels=128)
            for k in range(16):
                nc.vector.scalar_tensor_tensor(g[k][:], g[k][:],
                                               mnw_s[:, k:k + 1], rbc[:],
                                               op0=OP.mult, op1=OP.mult)
            dbg_dump("d_g0", g[0][:], [128, TOK], mybir.dt.bfloat16)

            x1 = [pD.tile([128, TOK], f32, name=f"x1_{i}")
                  for i in range(8)]
            x1pb = pD.tile([128, 8], bf16, name="x1pb")
            for mb in range(8):
                sla = wslab(w_outproj, 128 * mb, 128, 8, f"wopa{mb}")
                slb = wslab(w_outproj, 128 * mb, 128, 8, f"wopb{mb}",
                            r0=1024)
                po = ps4.tile([128, TOK], f32, name="po", tag="pbig4",
                              bufs=5)
                for k in range(16):
                    sl_, kk = (sla, k) if k < 8 else (slb, k - 8)
                    nc.tensor.matmul(po[:], sl_[:, kk, :], g[k][:],
                                     start=(k == 0), stop=(k == 15))
                xre = p4.tile([128, TOK], f32, name="xre", tag="xre", bufs=2)
                nc.sync.dma_start(xre[:],
                                  xin[128 * mb:128 * (mb + 1), 3:TH])
                nc.vector.scalar_tensor_tensor(x1[mb][:], xre[:], 1.0,
                                               po[:], op0=OP.mult,
                                               op1=OP.add)
                nc.vector.tensor_copy(x1pb[:, mb:mb + 1],
                                      x1[mb][:, TOK - 1:TOK])
            # deferred rmsnorm2: qkv runs on raw x1 (bf16); the per-token
            # 1/rms scale commutes with the GEMM and lands in the evacs.
            x1b = [pD.tile([128, TOK], bf16, name=f"x1b{i}")
                   for i in range(8)]
            for i in range(8):
                nc.vector.tensor_copy(x1b[i][:], x1[i][:])
            ssq1 = ps4.tile([1, TOK], f32, name="ssq1", tag="ssq1", bufs=1)
            for i in range(8):
                sq1 = p4.tile([128, TOK], bf16, name="sq1", tag="sq1",
                              bufs=2)
                nc.vector.tensor_mul(sq1[:], x1b[i][:], x1b[i][:])
                nc.tensor.matmul(ssq1[:], onesb[:], sq1[:], start=(i == 0),
                                 stop=(i == 7))
            rms1 = p4.tile([1, TOK], f32, name="rms1")
            nc.scalar.activation(rms1[:], ssq1[:], AF.Sqrt,
                                 bias=eps_c[0:1, :], scale=1.0 / C_)
            rinv1 = pD.tile([1, TOK], f32, name="rinv1")
            nc.vector.reciprocal(rinv1[:], rms1[:])
            rinv1b = p4.tile([1, TOK], bf16, name="rinv1b")
            nc.vector.tensor_copy(rinv1b[:], rinv1[:])
            rbc1 = pD.tile([128, TOK], bf16, name="rbc1")
            nc.gpsimd.partition_broadcast(rbc1[:], rinv1b[:], channels=128)
            rinv1_tm = pD.tile([128, 4], f32, name="rinv1_tm")
            nc.sync.dma_start(rinv1_dd[:], rinv1[:])
            nc.sync.dma_start(
                rinv1_tm[:],
                rinv1_dd[0:1, :].rearrange("a (c p) -> (a p) c", p=128))
            dbg_dump("d_x1_0", x1[0][:], [128, TOK])
            ps4s.close()
            st4.close()

        if PHASES >= 5:
            # ================= P5: attention =================
            st5 = stack()
            p5 = st5.enter_context(tc.tile_pool(name="p5", bufs=1))
            ps5s = stack()
            ps5 = ps5s.enter_context(tc.tile_pool(name="ps5", bufs=1,
                                                  space="PSUM"))
            amask = []
            for r in range(4):
                # keep when t >= s: f - p + (512*qb - 128*sb) >= 0,
                # variant j = sb - 4*qb in {0..3} -> base = -128*j
                m = p5.tile([128, 512], bf16, name=f"amask{r}")
                nc.vector.memset(m, 0.0)
                nc.gpsimd.affine_select(out=m, in_=m, compare_op=OP.is_ge,
                                        fill=NEG, base=-128 * r,
                                        channel_multiplier=-1,
                                        pattern=[[1, 512]])
                amask.append(m)
            qloc = [p5.tile([128, TOK], bf16, name=f"qloc{i}")
                    for i in range(8)]
            kloc = p5.tile([64, TOK], bf16, name="kloc")
            for mb in range(8):
                sl = wslab(w_att, 128 * mb, 128, 8, f"wq{mb}")
                pq = ps5.tile([128, TOK], f32, name="pq", tag="pbig5",
                              bufs=2)
                for k in range(8):
                    nc.tensor.matmul(pq[:], sl[:, k, :], x1b[k][:],
                                     start=(k == 0), stop=(k == 7))
                nc.vector.tensor_mul(qloc[mb][:], pq[:], rbc1[:])
            slk = wslab(w_att, 1024, 64, 8, "wkp")
            pk = ps5.tile([64, TOK], f32, name="pk", tag="psx", bufs=3)
            for k in range(8):
                nc.tensor.matmul(pk[:], slk[:, k, :], x1b[k][:],
                                 start=(k == 0), stop=(k == 7))
            nc.vector.tensor_mul(kloc[:], pk[:], rbc1[0:64, :])
            vloc = [p5.tile([128, 65], bf16, name=f"vloc{tb}")
                    for tb in range(4)]
            slv = wsl.tile([128, 8, 64], bf16, name="wvp", tag="wslab")
            nc.sync.dma_start(
                slv[:],
                w_att[:, 1088:1152].rearrange("(t p) m -> p t m", p=128))
            for tb in range(4):
                pv = ps5.tile([128, 64], f32, name="pv", tag="psx", bufs=3)
                for k in range(8):
                    nc.tensor.matmul(pv[:],
                                     x1b[k][:, 128 * tb:128 * (tb + 1)],
                                     slv[:, k, :], start=(k == 0),
                                     stop=(k == 7))
                nc.vector.tensor_scalar(vloc[tb][:, 0:64], pv[:],
                                        rinv1_tm[:, tb:tb + 1], None,
                                        op0=OP.mult)
                nc.vector.memset(vloc[tb][:, 64:65], 1.0)
            dbg_dump("d_q0", qloc[0][:], [128, TOK], mybir.dt.bfloat16)

            b2_in = dram.tile([652, TOK], bf16, name="b2_in")
            b2_out = dram.tile([1304, TOK], bf16, name="b2_out")
            for i in range(4):
                nc.sync.dma_start(b2_in[128 * i:128 * (i + 1), :],
                                  qloc[4 + i][:])
            nc.sync.dma_start(b2_in[512:576, :], kloc[:])
            for tb in range(4):
                nc.sync.dma_start(
                    b2_in[576:641, 128 * tb:128 * (tb + 1)]
                    .rearrange("r c -> c r"), vloc[tb][:])
            nc.sync.dma_start(
                b2_in[644:652, 0:128].rearrange("f p -> p f"), x1pb[:])
            nc.gpsimd.collective_compute(
                "AllGather", OP.bypass, replica_groups=RG,
                ins=[b2_in.opt()], outs=[b2_out.opt()])

            def masked2(dst, local_ap, recv_ap, local_is_first):
                # dst/recv must share a base partition; local may be shifted.
                P = local_ap.shape[0]
                ma = is_first if local_is_first else is_second
                mb_ = is_second if local_is_first else is_first
                nc.vector.tensor_scalar(dst, local_ap, ma[0:P, :], None,
                                        op0=OP.mult)
                nc.vector.scalar_tensor_tensor(dst, recv_ap, mb_[0:P, :],
                                               dst, op0=OP.mult, op1=OP.add)

            qall = [p5.tile([64, T_], bf16, name=f"qall{h}")
                    for h in range(8)]
            kall = p5.tile([64, T_], bf16, name="kall")
            vall = [p5.tile([128, 65], bf16, name=f"vall{gb}")
                    for gb in range(8)]
            for h in range(8):
                t = h // 2
                ro = 128 * t + 64 * (h % 2)
                rows = slice(64 * (h % 2), 64 * (h % 2) + 64)
                for half in (0, 1):
                    rcv = p5.tile([64, TOK], bf16, name="qr", tag="qrcv",
                                  bufs=2)
                    nc.sync.dma_start(
                        rcv[:],
                        b2_out[652 * half + ro:652 * half + ro + 64, :])
                    masked2(qall[h][:, TOK * half:TOK * (half + 1)],
                            qloc[t][rows, :], rcv[:],
                            local_is_first=(half == 0))
            for half in (0, 1):
                rcv = p5.tile([64, TOK], bf16, name="kr", tag="krcv", bufs=2)
                nc.sync.dma_start(
                    rcv[:], b2_out[652 * half + 512:652 * half + 576, :])
                masked2(kall[:, TOK * half:TOK * (half + 1)], kloc[:],
                        rcv[:], local_is_first=(half == 0))
            for gb in range(8):
                half, tb = gb // 4, gb % 4
                rcv = p5.tile([128, 65], bf16, name="vr", tag="vrcv", bufs=2)
                nc.sync.dma_start(
                    rcv[:], b2_out[652 * half + 576:652 * half + 641,
                                   128 * tb:128 * (tb + 1)]
                    .rearrange("r c -> c r"))
                masked2(vall[gb][:], vloc[tb][:], rcv[:],
                        local_is_first=(half == 0))
            x1p = p5.tile([128, 8], bf16, name="x1p")
            rx = p5.tile([128, 8], bf16, name="rx")
            nc.sync.dma_start(
                rx[:], b2_out[644:652, 0:128].rearrange("f p -> p f"))
            nc.vector.tensor_scalar(x1p[:], rx[:], is_second, None,
                                    op0=OP.mult)
            dbg_dump("d_qall0", qall[0][:], [64, T_], mybir.dt.bfloat16)
            dbg_dump("d_kall", kall[:], [64, T_], mybir.dt.bfloat16)

            yall = [p5.tile([64, T_], bf16, name=f"yall{h}")
                    for h in range(8)]
            for h in range(8):
                for qb in range(2):
                    qcols = slice(TOK * qb, TOK * (qb + 1))
                    pav = ps5.tile([65, TOK], f32, name="pav", tag="pav",
                                   bufs=2)
                    nsb = 4 * (qb + 1)
                    for sb in range(nsb):
                        psx = ps5.tile([128, TOK], f32, name="psx",
                                       tag="psx", bufs=3)
                        nc.tensor.matmul(psx[:],
                                         kall[:, 128 * sb:128 * (sb + 1)],
                                         qall[h][:, qcols], start=True,
                                         stop=True)
                        r = sb - 4 * qb
                        if 0 <= r <= 3:
                            nc.vector.tensor_add(psx[:], psx[:],
                                                 amask[r][:])
                        pexp = p5.tile([128, TOK], bf16, name="pexp",
                                       tag="pexp", bufs=4)
                        nc.scalar.activation(pexp[:], psx[:], AF.Exp)
                        nc.tensor.matmul(pav[:], vall[sb][:], pexp[:],
                                         start=(sb == 0),
                                         stop=(sb == nsb - 1))
                    rc = p5.tile([1, TOK], f32, name="rcs", tag="rcs",
                                 bufs=2)
                    nc.vector.reciprocal(rc[:], pav[64:65, :])
                    rcb = p5.tile([64, TOK], f32, name="rcb", tag="rcb",
                                  bufs=2)
                    nc.gpsimd.partition_broadcast(rcb[:], rc[:],
                                                  channels=64)
                    nc.vector.tensor_mul(yall[h][:, qcols], pav[0:64, :],
                                         rcb[:])
            dbg_dump("d_yall0", yall[0][:], [64, T_], mybir.dt.bfloat16)

            # exchange 3 + proj rhs assembly (per-head base-0 builds)
            wph = [p5.tile([64, TOK + 1], bf16, name=f"wph{h}", tag="wph",
                           bufs=8) for h in range(8)]
            yown = [p5.tile([128, TOK + 1], bf16, name=f"yown{t}")
                    for t in range(4)]
            for h in range(8):
                t = h // 2
                rows = slice(64 * (h % 2), 64 * (h % 2) + 64)
                nc.vector.tensor_scalar(wph[h][:, :],
                                        yall[h][:, TOK - 1:T_],
                                        is_first[0:64, :], None,
                                        op0=OP.mult)
                nc.vector.scalar_tensor_tensor(
                    wph[h][:, 1:TOK + 1], yall[h][:, 0:TOK],
                    is_second[0:64, :], wph[h][:, 1:TOK + 1],
                    op0=OP.mult, op1=OP.add)
                yoh = p5.tile([64, TOK + 1], bf16, name="yoh", tag="yoh",
                              bufs=2)
                nc.vector.tensor_scalar(yoh[:, :],
                                        yall[h][:, TOK - 1:T_],
                                        is_second[0:64, :], None,
                                        op0=OP.mult)
                nc.vector.scalar_tensor_tensor(
                    yoh[:, 1:TOK + 1], yall[h][:, 0:TOK],
                    is_first[0:64, :], yoh[:, 1:TOK + 1],
                    op0=OP.mult, op1=OP.add)
                nc.vector.tensor_copy(yown[t][rows, :], yoh[:])
            b3_in = dram.tile([512, TOK + 1], bf16, name="b3_in")
            b3_out = dram.tile([1024, TOK + 1], bf16, name="b3_out")
            for h in range(8):
                nc.sync.dma_start(b3_in[64 * h:64 * (h + 1), :], wph[h][:])
            nc.gpsimd.collective_compute(
                "AllGather", OP.bypass, replica_groups=RG,
                ins=[b3_in.opt()], outs=[b3_out.opt()])

            yfull = yown + [p5.tile([128, TOK + 1], bf16, name=f"yfp{t}")
                            for t in range(4)]
            for t in range(4):
                r0 = p5.tile([128, TOK + 1], bf16, name="yr0", tag="yr0",
                             bufs=2)
                r1 = p5.tile([128, TOK + 1], bf16, name="yr1", tag="yr1",
                             bufs=2)
                nc.sync.dma_start(r0[:], b3_out[128 * t:128 * (t + 1), :])
                nc.sync.dma_start(
                    r1[:], b3_out[512 + 128 * t:512 + 128 * (t + 1), :])
                nc.vector.tensor_scalar(yfull[4 + t][:], r0[:], is_second,
                                        None, op0=OP.mult)
                nc.vector.scalar_tensor_tensor(yfull[4 + t][:], r1[:],
                                               is_first, yfull[4 + t][:],
                                               op0=OP.mult, op1=OP.add)

            x2 = [pE.tile([128, TOK], f32, name=f"x2_{i}")
                  for i in range(8)]
            x2p = resid.tile([128, 8], f32, name="x2p")
            for mb in range(8):
                sl = wslab(w_proj, 128 * mb, 128, 8, f"wpj{mb}")
                pp = ps5.tile([128, TOK], f32, name="pp", tag="pbig5",
                              bufs=2)
                pp1 = ps5.tile([128, 1], f32, name="pp1", tag="pp1", bufs=1)
                for k in range(8):
                    nc.tensor.matmul(pp[:], sl[:, k, :],
                                     yfull[k][:, 1:TOK + 1],
                                     start=(k == 0), stop=(k == 7))
                    nc.tensor.matmul(pp1[:], sl[:, k, :], yfull[k][:, 0:1],
                                     start=(k == 0), stop=(k == 7))
                nc.vector.scalar_tensor_tensor(x2[mb][:], x1[mb][:], 1.0,
                                               pp[:], op0=OP.mult,
                                               op1=OP.add)
                tpv = p5.tile([128, 1], f32, name="tpv", tag="tpv", bufs=2)
                nc.vector.tensor_add(tpv[:], x1p[:, mb:mb + 1], pp1[:])
                nc.vector.tensor_scalar(x2p[:, mb:mb + 1], tpv[:],
                                        is_second, None, op0=OP.mult)
            dbg_dump("d_x2_0", x2[0][:], [128, TOK])
            ps5s.close()
            st5.close()
            stD.close()

        if PHASES >= 6:
            # ================= P6: cmix =================
            st6 = stack()
            p6 = st6.enter_context(tc.tile_pool(name="p6", bufs=1))
            ps6s = stack()
            ps6 = ps6s.enter_context(tc.tile_pool(name="ps6", bufs=1,
                                                  space="PSUM"))
            z3 = [p6.tile([128, TOK + 1], bf16, name=f"z3_{i}")
                  for i in range(8)]
            rmsnorm_cm([x2[i][:] for i in range(8)],
                       [z3[i][:, 1:TOK + 1] for i in range(8)], TOK, p6,
                       ps6, C_, "n2")
            sqp = p6.tile([128, 8], bf16, name="sqp")
            nc.vector.tensor_mul(sqp[:], x2p[:], x2p[:])
            psp = ps6.tile([1, 8], f32, name="psp", tag="psp", bufs=1)
            nc.tensor.matmul(psp[:], onesb[:], sqp[:], start=True,
                             stop=True)
            ssp = p6.tile([1, 1], f32, name="ssp")
            nc.vector.tensor_reduce(ssp[:], psp[:],
                                    axis=mybir.AxisListType.X, op=OP.add)
            nc.scalar.activation(ssp[:], ssp[:], AF.Sqrt,
                                 bias=eps_c[0:1, :], scale=1.0 / C_)
            nc.vector.reciprocal(ssp[:], ssp[:])
            rpb = p6.tile([128, 1], f32, name="rpb")
            nc.gpsimd.partition_broadcast(rpb[:], ssp[:], channels=128)
            for i in range(8):
                nc.vector.scalar_tensor_tensor(z3[i][:, 0:1],
                                               x2p[:, i:i + 1], 1.0,
                                               rpb[:], op0=OP.mult,
                                               op1=OP.mult)
            dbg_dump("d_z3_0", z3[0][:], [128, TOK + 1], mybir.dt.bfloat16)

            xk = [p6.tile([128, TOK], bf16, name=f"xk{i}")
                  for i in range(8)]
            xr = [p6.tile([128, TOK], bf16, name=f"xr{i}")
                  for i in range(8)]
            for i in range(8):
                nc.vector.tensor_scalar(xk[i][:], z3[i][:, 1:TOK + 1],
                                        mk1_s[:, i:i + 1], None,
                                        op0=OP.mult)
                nc.vector.scalar_tensor_tensor(xk[i][:], z3[i][:, 0:TOK],
                                               mk_s[:, i:i + 1], xk[i][:],
                                               op0=OP.mult, op1=OP.add)
                nc.vector.tensor_scalar(xr[i][:], z3[i][:, 1:TOK + 1],
                                        mr1_s[:, i:i + 1], None,
                                        op0=OP.mult)
                nc.vector.scalar_tensor_tensor(xr[i][:], z3[i][:, 0:TOK],
                                               mr_s[:, i:i + 1], xr[i][:],
                                               op0=OP.mult, op1=OP.add)

            kE = [p6.tile([128, TOK], bf16, name=f"kE{i}")
                  for i in range(32)]
            for mb in range(32):
                sl = wslab(w_key, 128 * mb, 128, 8, f"wky{mb}")
                pky = ps6.tile([128, TOK], f32, name="pky", tag="pbig6",
                               bufs=6)
                for k in range(8):
                    nc.tensor.matmul(pky[:], sl[:, k, :], xk[k][:],
                                     start=(k == 0), stop=(k == 7))
                nc.scalar.activation(kE[mb][:], pky[:], AF.Erf,
                                     scale=1.0 / _DEN, bias=erfb_c[:, :])
            r_sb = [p6.tile([128, TOK], bf16, name=f"r_sb{i}")
                    for i in range(8)]
            for mb in range(8):
                sl = wslab(w_rec, 128 * mb, 128, 8, f"wrc{mb}")
                pr = ps6.tile([128, TOK], f32, name="pr", tag="pbig6",
                              bufs=6)
                for k in range(8):
                    nc.tensor.matmul(pr[:], sl[:, k, :], xr[k][:],
                                     start=(k == 0), stop=(k == 7))
                nc.scalar.activation(r_sb[mb][:], pr[:], AF.Sigmoid)
            dbg_dump("d_kE0", kE[0][:], [128, TOK], mybir.dt.bfloat16)
            dbg_dump("d_r0", r_sb[0][:], [128, TOK], mybir.dt.bfloat16)

            for mb in range(8):
                slab = wslab(w_val, 128 * mb, 128, 32, f"wvl{mb}", pool=p6,
                             tag="wslab_v", bufs=2)
                pvv = ps6.tile([128, TOK], f32, name="pvv", tag="pbig6",
                               bufs=6)
                for k in range(32):
                    nc.tensor.matmul(pvv[:], slab[:, k, :], kE[k][:],
                                     start=(k == 0), stop=(k == 31))
                tmpv = p6.tile([128, TOK], f32, name="tmpv", tag="tmpv",
                               bufs=2)
                nc.vector.tensor_scalar(tmpv[:], pvv[:],
                                        vbias_s[:, mb:mb + 1], None,
                                        op0=OP.add)
                nc.vector.tensor_mul(tmpv[:], tmpv[:], r_sb[mb][:])
                outt = p6.tile([128, TOK], f32, name="outt", tag="outt",
                               bufs=2)
                nc.vector.tensor_add(outt[:], x2[mb][:], tmpv[:])
                nc.sync.dma_start(out_d[128 * mb:128 * (mb + 1), :],
                                  outt[:])
            ps6s.close()
            st6.close()

        for s in reversed(_open):
            s.close()
        whole.close()

    nc.compile()
    return nc, dbg_outs


# ================= host glue =================

def _prep_inputs(x, in_proj_w, conv_w, conv_b, dt_bias, A_log, D, mnorm_w,
                 out_proj_w, attn_w, proj_w, time_maa_k, time_maa_r, key_w,
                 recept_w, value_w):
    f32 = np.float32

    def b(a):
        return np.ascontiguousarray(np.asarray(a, f32).astype(BF16))

    x = np.asarray(x, f32)
    shared = {
        "w_inproj": b(in_proj_w),
        "convw": np.ascontiguousarray(
            np.asarray(conv_w, f32).reshape(17, 128, DCONV)
            .transpose(1, 0, 2)),
        "convb": np.ascontiguousarray(
            np.asarray(conv_b, f32).reshape(17, 128).T),
        "dtb": np.ascontiguousarray(
            np.asarray(dt_bias, f32).reshape(NHM, 1)),
        "aneg": np.ascontiguousarray(
            (-np.exp(np.asarray(A_log, f32))).reshape(NHM, 1)),
        # drep[p, k] = D[2k + (p >= 64)]
        "drep": np.ascontiguousarray(np.stack(
            [np.concatenate([np.full(64, D2[0]), np.full(64, D2[1])])
             for D2 in np.asarray(D, f32).reshape(16, 2)], axis=1)
            .astype(f32)),
        "mnw": np.ascontiguousarray(
            np.asarray(mnorm_w, f32).reshape(16, 128).T),
        "w_outproj": b(out_proj_w),
        "mk": np.ascontiguousarray(
            np.asarray(time_maa_k, f32).reshape(8, 128).T),
        "mk1": np.ascontiguousarray(
            (1.0 - np.asarray(time_maa_k, f32)).reshape(8, 128).T),
        "mr": np.ascontiguousarray(
            np.asarray(time_maa_r, f32).reshape(8, 128).T),
        "mr1": np.ascontiguousarray(
            (1.0 - np.asarray(time_maa_r, f32)).reshape(8, 128).T),
        "w_key": b(key_w),
        "w_val": b(0.5 * np.asarray(value_w, f32)),
        "vbias": np.ascontiguousarray(
            (0.5 * np.asarray(value_w, f32).sum(0)).reshape(8, 128).T),
        "w_rec": b(recept_w),
    }
    ef = np.zeros((NHM, DIN), f32)
    for k in range(16):
        ef[2 * k, 128 * k:128 * k + 64] = 1.0
        ef[2 * k + 1, 128 * k + 64:128 * k + 128] = 1.0
    shared["efull"] = ef

    attn_w = np.asarray(attn_w, f32)
    proj_w = np.asarray(proj_w, f32)
    scale = 1.0 / np.sqrt(np.float32(HD))
    in_maps = []
    for core in range(N_CORES):
        bi, half = core // 2, core % 2
        start = half * TOK
        xcm = x[bi].T
        xs = np.zeros((C_, TH), f32)
        xs[:, 3:] = xcm[:, start:start + TOK]
        if start >= 3:
            xs[:, 0:3] = xcm[:, start - 3:start]
        myh = np.arange(8 * half, 8 * half + 8)
        oth = np.arange(8 * (1 - half), 8 * (1 - half) + 8)
        qcols = attn_w[:, :C_].reshape(C_, NH, HD)
        wq_perm = np.concatenate(
            [qcols[:, myh].reshape(C_, 512),
             qcols[:, oth].reshape(C_, 512)], axis=1) * scale
        w_att_c = np.concatenate([wq_perm, attn_w[:, C_:]], axis=1)
        prows = proj_w.reshape(NH, HD, C_)
        w_proj_c = np.concatenate(
            [prows[myh].reshape(512, C_), prows[oth].reshape(512, C_)],
            axis=0)
        mskc = np.zeros((128, 2), f32)
        mskc[:, 0] = 1.0 - half
        mskc[:, 1] = half
        m = dict(shared)
        m["xin"] = np.ascontiguousarray(xs)
        m["w_att"] = np.ascontiguousarray(w_att_c.astype(BF16))
        m["w_proj"] = np.ascontiguousarray(w_proj_c.astype(BF16))
        m["msk"] = mskc
        in_maps.append(m)
    return in_maps


def kernel(**inputs):
    from concourse.bass_utils import run_bass_kernel_spmd

    if "nc" not in _CACHE:
        _CACHE["nc"], _CACHE["dbg"] = _build()
    nc = _CACHE["nc"]
    in_maps = _prep_inputs(**inputs)
    res = run_bass_kernel_spmd(nc, in_maps, core_ids=list(range(N_CORES)))
    _CACHE["results"] = res
    out = np.empty((B_, T_, C_), np.float32)
    for core in range(N_CORES):
        bi, half = core // 2, core % 2
        out[bi, half * TOK:(half + 1) * TOK, :] = \
            np.asarray(res.results[core]["out"], np.float32).T
    return out


# revision 32
# speedup vs baseline: 1.1178x; 1.0062x over previous
"""nn_Block_21062519619681 fully on-device: hybrid Mamba2 + MQA + RWKV-CMix
block as ONE Bass/Tile SPMD kernel on 8 trn2 NeuronCores.

Sharding: 8 cores = 4 batches x 2 token-halves (512 own tokens/core).
 - mamba: token-sharded; chunked-SSD scan (L=128); cross-half state carry via
   a pairwise AllGather applied as a linear correction pass.
 - attention: q-head-split (8 heads/core over ALL 1024 tokens; per-core
   permuted q/proj weights keep the SPMD graph rank-uniform); k/v + q halves
   exchanged via pairwise AllGather; softmax without max-subtraction (scores
   bounded); colsum ridden as a ones-column in the av matmul.
 - cmix: token-sharded, replicated weights, erf/sigmoid fused into PSUM evac.
All matmuls bf16 (weights pre-cast on host), fp32 PSUM accumulate, fp32
residual stream. Rank-dependent selection uses host-fed 0/1 masks (masked
sums) - the instruction graph is identical on all cores.
"""
import os
import sys

sys.path.insert(0, "/opt/trn_rl_repo")
import numpy as np
import ml_dtypes

B_, T_, C_ = 4, 1024, 1024
NH, HD = 16, 64
DS, DCONV, EXP, PHD = 64, 4, 2, 64
DIN = EXP * C_
NHM = DIN // PHD
CONVD = DIN + 2 * DS
FFN = 4 * C_
EPS = 1e-5
N_CORES = 8
TOK = 512
TH = TOK + 3
L = 128
NCH = TOK // L
NEG = -1e30

BF16 = ml_dtypes.bfloat16
DEBUG = bool(int(os.environ.get("BASSK_DEBUG", "0")))
PHASES = int(os.environ.get("BASSK_PHASES", "6"))

_CACHE = {}


def _build():
    import contextlib
    import concourse.mybir as mybir
    import concourse.bacc as bacc
    import concourse.tile as tile
    from concourse.masks import make_identity

    f32 = mybir.dt.float32
    bf16 = mybir.dt.bfloat16
    AF = mybir.ActivationFunctionType
    OP = mybir.AluOpType

    nc = bacc.Bacc("TRN2", target_bir_lowering=False, debug=False,
                   num_devices=N_CORES)

    def din(name, shape, dt=bf16):
        return nc.dram_tensor(name, shape, dt, kind="ExternalInput").ap()

    xin = din("xin", [C_, TH], f32)
    w_inproj = din("w_inproj", [C_, 4256])
    convw = din("convw", [128, 17, DCONV], f32)
    convb = din("convb", [128, 17], f32)
    dtb = din("dtb", [NHM, 1], f32)
    aneg = din("aneg", [NHM, 1], f32)
    drep = din("drep", [128, 16], f32)
    mnw = din("mnw", [128, 16], f32)
    w_outproj = din("w_outproj", [DIN, C_])
    w_att = din("w_att", [C_, 1024 + 128])
    w_proj = din("w_proj", [C_, C_])
    mk = din("mk", [128, 8], f32)
    mk1 = din("mk1", [128, 8], f32)
    mr = din("mr", [128, 8], f32)
    mr1 = din("mr1", [128, 8], f32)
    w_key = din("w_key", [C_, FFN])
    w_val = din("w_val", [FFN, C_])
    vbias = din("vbias", [128, 8], f32)
    w_rec = din("w_rec", [C_, C_])
    msk = din("msk", [128, 2], f32)
    efull = din("efull", [NHM, DIN])

    out_d = nc.dram_tensor("out", [C_, TOK], f32, kind="ExternalOutput").ap()

    dbg_outs = {}

    def dbg_dump(name, ap_or_tile, shape, dt=None):
        if not DEBUG:
            return
        d = nc.dram_tensor(name, shape, dt or mybir.dt.float32,
                           kind="ExternalOutput").ap()
        dbg_outs[name] = d
        nc.sync.dma_start(d, ap_or_tile)

    RG = [[0, 1], [2, 3], [4, 5], [6, 7]]

    with tile.TileContext(nc) as tc:
        _open = []

        def stack():
            s = contextlib.ExitStack()
            _open.append(s)
            return s

        whole = contextlib.ExitStack()
        consts = whole.enter_context(tc.tile_pool(name="consts", bufs=1))
        resid = whole.enter_context(tc.tile_pool(name="resid", bufs=1))
        wsl = whole.enter_context(tc.tile_pool(name="wsl", bufs=6))
        dram = whole.enter_context(tc.tile_pool(name="dram", bufs=1,
                                                space="DRAM"))

        # ---------------- constants ----------------
        ident_b = consts.tile([128, 128], bf16, name="ident_b")
        make_identity(nc, ident_b)
        ident_f = consts.tile([128, 128], f32, name="ident_f")
        make_identity(nc, ident_f)
        tri01 = consts.tile([128, 128], bf16, name="tri01")
        nc.vector.memset(tri01, 1.0)
        nc.gpsimd.affine_select(out=tri01, in_=tri01, compare_op=OP.is_ge,
                                fill=0.0, base=0, channel_multiplier=-1,
                                pattern=[[1, 128]])
        onesb = consts.tile([128, 1], bf16, name="onesb")
        nc.vector.memset(onesb, 1.0)
        onesf_r = consts.tile([1, 64], f32, name="onesf_r")
        nc.vector.memset(onesf_r, 1.0)
        ones32 = consts.tile([NHM, L], f32, name="ones32")
        nc.vector.memset(ones32, 1.0)
        eps_c = consts.tile([128, 1], f32, name="eps_c")
        nc.vector.memset(eps_c, EPS)
        _MU = float(np.sqrt(0.5))
        _DEN = float(np.sqrt(1.0 / (4.0 * np.pi)) * np.sqrt(2.0))
        erfb_c = consts.tile([128, 1], f32, name="erfb_c")
        nc.vector.memset(erfb_c, -_MU / _DEN)

        def cin(name, shape, src, dt=f32):
            t = consts.tile(list(shape), dt, name=name)
            nc.sync.dma_start(t[:], src)
            return t

        convw_s = cin("convw_s", [128, 17, DCONV], convw)
        convb_s = cin("convb_s", [128, 17], convb)
        dtb_s = cin("dtb_s", [NHM, 1], dtb)
        aneg_s = cin("aneg_s", [NHM, 1], aneg)
        drep_s = cin("drep_s", [128, 16], drep)
        mnw_s = cin("mnw_s", [128, 16], mnw)
        mk_s = cin("mk_s", [128, 8], mk)
        mk1_s = cin("mk1_s", [128, 8], mk1)
        mr_s = cin("mr_s", [128, 8], mr)
        mr1_s = cin("mr1_s", [128, 8], mr1)
        vbias_s = cin("vbias_s", [128, 8], vbias)
        msk_s = cin("msk_s", [128, 2], msk)
        efull_s = cin("efull_s", [NHM, DIN], efull, dt=bf16)
        is_first = msk_s[:, 0:1]
        is_second = msk_s[:, 1:2]

        zsil_d = dram.tile([DIN, TOK], bf16, name="zsil_d")
        rinv1_dd = dram.tile([1, TOK], f32, name="rinv1_dd")

        def wslab(wt, m0, mw, kt, name, pool=None, tag="wslab", bufs=None,
                  r0=0):
            s = (pool or wsl).tile([128, kt, mw], bf16, name=name, tag=tag,
                                   bufs=bufs)
            nc.sync.dma_start(
                s[:], wt[r0:r0 + 128 * kt, m0:m0 + mw]
                .rearrange("(t p) m -> p t m", p=128))
            return s

        def rmsnorm_cm(src_aps, dst_aps, width, pool, psp, nfeat, tag):
            ssq = psp.tile([1, width], f32, name=f"ssq_{tag}",
                           tag=f"ssq{tag}", bufs=1)
            n = len(src_aps)
            for i, sap in enumerate(src_aps):
                sq = pool.tile([128, width], bf16, name=f"sq_{tag}",
                               tag=f"sq{tag}", bufs=2)
                nc.vector.tensor_mul(sq[:], sap, sap)
                nc.tensor.matmul(ssq[:], onesb[:], sq[:], start=(i == 0),
                                 stop=(i == n - 1))
            rms = pool.tile([1, width], f32, name=f"rms_{tag}",
                            tag=f"rms{tag}", bufs=1)
            nc.scalar.activation(rms[:], ssq[:], AF.Sqrt,
                                 bias=eps_c[0:1, :], scale=1.0 / nfeat)
            rinv = pool.tile([1, width], f32, name=f"rinv_{tag}",
                             tag=f"rinv{tag}", bufs=1)
            nc.vector.reciprocal(rinv[:], rms[:])
            rbc = pool.tile([128, width], f32, name=f"rbc_{tag}",
                            tag=f"rbc{tag}", bufs=1)
            nc.gpsimd.partition_broadcast(rbc[:], rinv[:], channels=128)
            for i, sap in enumerate(src_aps):
                nc.vector.tensor_mul(dst_aps[i], sap, rbc[:])

        # pool nesting (open early -> close late):
        stY = stack()
        pY = stY.enter_context(tc.tile_pool(name="pY", bufs=1))   # ..P4
        stB = stack()
        pB = stB.enter_context(tc.tile_pool(name="pB", bufs=1))   # ..P3
        stC = stack()
        pC = stC.enter_context(tc.tile_pool(name="pC", bufs=1))   # ..P3

        # ================= P0 + P1: rmsnorm + in_proj =================
        st01 = stack()
        pA = st01.enter_context(tc.tile_pool(name="pA", bufs=1))
        ps01 = st01.enter_context(tc.tile_pool(name="ps01", bufs=1,
                                               space="PSUM"))
        xn = [pA.tile([128, TH], bf16, name=f"xn{i}") for i in range(8)]
        # streaming rmsnorm over x (full TH width, stats on own 512 cols)
        ssqx = ps01.tile([1, 512], f32, name="ssqx", tag="ssqx", bufs=1)
        ssqh = ps01.tile([1, 3], f32, name="ssqh", tag="ssqh", bufs=1)
        for i in range(8):
            xt = pA.tile([128, TH], f32, name="xt", tag="xt", bufs=4)
            nc.sync.dma_start(xt[:], xin[128 * i:128 * (i + 1), :])
            sqx = pA.tile([128, TH], bf16, name="sqx", tag="sqx", bufs=2)
            nc.vector.tensor_mul(sqx[:], xt[:], xt[:])
            nc.tensor.matmul(ssqx[:], onesb[:], sqx[:, 3:TH],
                             start=(i == 0), stop=(i == 7))
            nc.tensor.matmul(ssqh[:], onesb[:], sqx[:, 0:3],
                             start=(i == 0), stop=(i == 7))
        rmsx = pA.tile([1, TH], f32, name="rmsx")
        nc.scalar.activation(rmsx[:, 3:TH], ssqx[:], AF.Sqrt,
                             bias=eps_c[0:1, :], scale=1.0 / C_)
        nc.scalar.activation(rmsx[:, 0:3], ssqh[:], AF.Sqrt,
                             bias=eps_c[0:1, :], scale=1.0 / C_)
        rinvx = pA.tile([1, TH], f32, name="rinvx")
        nc.vector.reciprocal(rinvx[:], rmsx[:])
        rbcx = pA.tile([128, TH], f32, name="rbcx")
        nc.gpsimd.partition_broadcast(rbcx[:], rinvx[:], channels=128)
        for i in range(8):
            xt = pA.tile([128, TH], f32, name="xt", tag="xt", bufs=4)
            nc.sync.dma_start(xt[:], xin[128 * i:128 * (i + 1), :])
            nc.vector.tensor_mul(xn[i][:], xt[:], rbcx[:])
        dbg_dump("d_xn0", xn[0][:], [128, TH], mybir.dt.bfloat16)

        xbc = [pB.tile([128, TH], bf16, name=f"xbc{i}") for i in range(17)]
        dtraw = pB.tile([NHM, TOK], f32, name="dtraw")

        for mb in range(16):
            sl = wslab(w_inproj, 128 * mb, 128, 8, f"wz{mb}")
            pz = ps01.tile([128, TOK], f32, name="pz", tag="pbig", bufs=4)
            for k in range(8):
                nc.tensor.matmul(pz[:], sl[:, k, :], xn[k][:, 3:TH],
                                 start=(k == 0), stop=(k == 7))
            zst = pA.tile([128, TOK], bf16, name="zst", tag="zst", bufs=3)
            nc.scalar.activation(zst[:], pz[:], AF.Silu)
            nc.sync.dma_start(zsil_d[128 * mb:128 * (mb + 1), :], zst[:])
        for mb in range(17):
            sl = wslab(w_inproj, DIN + 128 * mb, 128, 8, f"wxbc{mb}")
            pb_ = ps01.tile([128, TOK], f32, name="pb", tag="pbig", bufs=4)
            ph = ps01.tile([128, 3], f32, name="ph", tag="phalo", bufs=1)
            for k in range(8):
                nc.tensor.matmul(pb_[:], sl[:, k, :], xn[k][:, 3:TH],
                                 start=(k == 0), stop=(k == 7))
                nc.tensor.matmul(ph[:], sl[:, k, :], xn[k][:, 0:3],
                                 start=(k == 0), stop=(k == 7))
            nc.scalar.copy(xbc[mb][:, 3:TH], pb_[:])
            nc.vector.tensor_copy(xbc[mb][:, 0:3], ph[:])
        sl = wslab(w_inproj, 4224, 32, 8, "wdtp")
        pdt = ps01.tile([NHM, TOK], f32, name="pdt", tag="pdt", bufs=1)
        for k in range(8):
            nc.tensor.matmul(pdt[:], sl[:, k, :], xn[k][:, 3:TH],
                             start=(k == 0), stop=(k == 7))
        nc.vector.tensor_copy(dtraw[:], pdt[:])
        dbg_dump("d_xbc0", xbc[0][:], [128, TH], mybir.dt.bfloat16)
        st01.close()

        if PHASES >= 2:
            # ============ P2: conv + dt pipeline + transposes ============
            ps2 = stack()
            ps2p = ps2.enter_context(tc.tile_pool(name="ps2", bufs=1,
                                                  space="PSUM"))
            xs_cm = [pC.tile([128, TOK], bf16, name=f"xs_cm{i}")
                     for i in range(17)]
            for i in range(17):
                tmp = pC.tile([128, TOK], f32, name="ctmp", tag="ctmp",
                              bufs=4)
                nc.vector.tensor_scalar(tmp[:], xbc[i][:, 0:TOK],
                                        convw_s[:, i, 0:1], None,
                                        op0=OP.mult)
                for j in range(1, DCONV):
                    nc.vector.scalar_tensor_tensor(
                        tmp[:], xbc[i][:, j:j + TOK], convw_s[:, i, j:j + 1],
                        tmp[:], op0=OP.mult, op1=OP.add)
                nc.scalar.activation(xs_cm[i][:], tmp[:], AF.Silu,
                                     bias=convb_s[:, i:i + 1])
            C_cm = pC.tile([64, TOK], bf16, name="C_cm")
            nc.vector.tensor_copy(C_cm[:], xs_cm[16][64:128, :])
            dbg_dump("d_xs0", xs_cm[0][:], [128, TOK], mybir.dt.bfloat16)

            dt_f = pC.tile([NHM, TOK], f32, name="dt_f")
            Lc = pC.tile([NHM, TOK], f32, name="Lc")
            wdt = pC.tile([NHM, TOK], f32, name="wdt", tag="scr", bufs=2)
            u = pC.tile([NHM, TOK], f32, name="u")
            ex = pC.tile([NHM, TOK], f32, name="ex", tag="scr", bufs=2)
            nc.vector.tensor_scalar(u[:], dtraw[:], dtb_s[:], None,
                                    op0=OP.add)
            ab = pC.tile([NHM, TOK], f32, name="ab", tag="scr", bufs=2)
            nc.vector.tensor_scalar(ab[:], u[:], -1.0, None, op0=OP.mult)
            nc.vector.tensor_max(ab[:], ab[:], u[:])
            nc.scalar.activation(ex[:], ab[:], AF.Exp, scale=-1.0)
            nc.scalar.activation(ex[:], ex[:], AF.Ln, bias=1.0)
            nc.vector.tensor_scalar(dt_f[:], u[:], 0.0, None, op0=OP.max)
            nc.vector.tensor_add(dt_f[:], dt_f[:], ex[:])
            dta = u
            nc.vector.tensor_scalar(dta[:], dt_f[:], aneg_s[:], None,
                                    op0=OP.mult)
            for c in range(NCH):
                cs = slice(L * c, L * (c + 1))
                nc.vector.tensor_tensor_scan(Lc[:, cs], ones32[:],
                                             dta[:, cs], 0.0, op0=OP.mult,
                                             op1=OP.add)
                nc.scalar.activation(wdt[:, cs], Lc[:, cs], AF.Exp,
                                     scale=-1.0,
                                     bias=Lc[:, L * (c + 1) - 1:L * (c + 1)])
            nc.vector.tensor_mul(wdt[:], wdt[:], dt_f[:])

            dbg_dump("d_dt", dt_f[:], [NHM, TOK])
            dbg_dump("d_Lc", Lc[:], [NHM, TOK])

            eLcE = pC.tile([NHM, NCH], bf16, name="eLcE")
            lce = pC.tile([NHM, NCH], f32, name="lce")
            for c in range(NCH):
                nc.vector.tensor_copy(lce[:, c:c + 1],
                                      Lc[:, L * (c + 1) - 1:L * (c + 1)])
            nc.scalar.activation(eLcE[:], lce[:], AF.Exp)
            arep = [pC.tile([128, NCH], f32, name=f"arep{k}")
                    for k in range(16)]
            carep = [pC.tile([128, NCH], f32, name=f"carep{k}")
                     for k in range(16)]
            for k in range(16):
                pa = ps2p.tile([128, NCH], f32, name="pa", tag="pa", bufs=2)
                nc.tensor.matmul(pa[:], efull_s[:, 128 * k:128 * (k + 1)],
                                 eLcE[:], start=True, stop=True)
                nc.vector.tensor_copy(arep[k][:], pa[:])
                nc.vector.memset(carep[k][:, 0:1], 1.0)
                for c in range(1, NCH):
                    nc.vector.tensor_mul(carep[k][:, c:c + 1],
                                         carep[k][:, c - 1:c],
                                         arep[k][:, c - 1:c])

            tmv = [pC.tile([128, 96], f32, name=f"tmv{c}")
                   for c in range(NCH)]
            stk = pC.tile([96, TOK], f32, name="stk")
            nc.vector.tensor_scalar(stk[0:NHM, :], Lc[:], -1.0, None,
                                    op0=OP.mult)
            nc.vector.tensor_copy(stk[NHM:2 * NHM, :], dt_f[:])
            nc.vector.tensor_copy(stk[2 * NHM:3 * NHM, :], wdt[:])
            for c in range(NCH):
                pt = ps2p.tile([128, 96], f32, name="pt", tag="ptr", bufs=2)
                nc.tensor.transpose(pt[:], stk[:, L * c:L * (c + 1)],
                                    ident_f[0:96, 0:96])
                nc.vector.tensor_copy(tmv[c][:], pt[:])

            xs_tm = [pC.tile([128, 2176], bf16, name=f"xs_tm{c}")
                     for c in range(NCH)]
            for c in range(NCH):
                for i in range(17):
                    ptb = ps2p.tile([128, 128], bf16, name="ptb", tag="ptrb",
                                    bufs=3)
                    nc.tensor.transpose(ptb[:],
                                        xs_cm[i][:, L * c:L * (c + 1)],
                                        ident_b[:])
                    nc.vector.tensor_copy(
                        xs_tm[c][:, 128 * i:128 * (i + 1)], ptb[:])
            dbg_dump("d_xstm0", xs_tm[0][:], [128, 2176], mybir.dt.bfloat16)
            ps2.close()

        if PHASES >= 3:
            # ========== P3: scan (interleaved per chunk) ==========
            ps3 = stack()
            ps3p = ps3.enter_context(tc.tile_pool(name="ps3", bufs=1,
                                                  space="PSUM"))
            state = [pC.tile([128, PHD], f32, name=f"state{k}")
                     for k in range(16)]
            for k in range(16):
                nc.vector.memset(state[k][:], 0.0)
            stateb = [pC.tile([64, PHD], bf16, name=f"stateb{h}")
                      for h in range(NHM)]
            ycm = [pY.tile([128, TOK], bf16, name=f"ycm{k}")
                   for k in range(16)]

            def stage_bcast(lcf, hh, with_exp=True, channels=128):
                lba = pC.tile([channels, 16 * L], f32, name="lba",
                              tag="lball", bufs=1)
                nc.gpsimd.partition_broadcast(
                    lba[:], lcf[0:1, 16 * L * hh:16 * L * (hh + 1)],
                    channels=channels)
                eba = None
                if with_exp:
                    eba = pC.tile([64, 16 * L], bf16, name="eba",
                                  tag="eball", bufs=1)
                    nc.scalar.activation(eba[:], lba[0:64, :], AF.Exp)
                return lba, eba

            def make_cdec_dve(eba, h, cs):
                off = L * (h % 16)
                cd = pC.tile([64, L], bf16, name="cd", tag="cdec", bufs=6)
                nc.vector.tensor_mul(cd[:], C_cm[:, cs],
                                     eba[:, off:off + L])
                return cd

            def make_cdec(eba, h, cs):
                off = L * (h % 16)
                cd = pC.tile([64, L], bf16, name="cd", tag="cdec", bufs=6)
                nc.gpsimd.tensor_mul(cd[:], C_cm[:, cs],
                                     eba[:, off:off + L])
                return cd

            def stage_lc(c):
                t = pC.tile([1, NHM * L], f32, name=f"LcFc{c}",
                            tag="lcf", bufs=2)
                nc.sync.dma_start(t[0:1, :], Lc[:, L * c:L * (c + 1)])
                return t

            for c in range(NCH):
                cs = slice(L * c, L * (c + 1))
                lcf = stage_lc(c)
                if c > 0:
                    for h in range(NHM):
                        nc.gpsimd.tensor_copy(
                            stateb[h][:],
                            state[h // 2][64 * (h % 2):64 * (h % 2) + 64, :])
                pg = ps3p.tile([128, L], f32, name="pg", tag="pg", bufs=1)
                nc.tensor.matmul(pg[:], xs_cm[16][0:64, cs], C_cm[:, cs],
                                 start=True, stop=True)
                gts = pC.tile([128, L], bf16, name="gts", tag="gts", bufs=2)
                nc.vector.tensor_mul(gts[:], pg[:], tri01[:])
                lba = eba = None
                for h in range(NHM):
                    k = h // 2
                    rows = slice(64 * (h % 2), 64 * (h % 2) + 64)
                    if h % 16 == 0:
                        lba, eba = stage_bcast(lcf, h // 16,
                                               with_exp=(c > 0))
                    darg = pC.tile([128, L], f32, name="darg", tag="darg",
                                   bufs=4)
                    nc.vector.tensor_scalar(darg[:],
                                            lba[:, L * (h % 16):
                                                L * (h % 16) + L],
                                            tmv[c][:, h:h + 1], 0.0,
                                            op0=OP.add, op1=OP.min)
                    expd = pC.tile([128, L], f32, name="expd", tag="expd",
                                   bufs=4)
                    nc.scalar.activation(expd[:], darg[:], AF.Exp)
                    mt = pC.tile([128, L], bf16, name="mt", tag="mt", bufs=4)
                    nc.vector.scalar_tensor_tensor(
                        mt[:], gts[:], tmv[c][:, 32 + h:33 + h], expd[:],
                        op0=OP.mult, op1=OP.mult)
                    py = ps3p.tile([64, L], f32, name="py", tag="py", bufs=2)
                    nc.tensor.matmul(py[:],
                                     xs_tm[c][:, PHD * h:PHD * (h + 1)],
                                     mt[:], start=True, stop=(c == 0))
                    if c > 0:
                        cd = make_cdec(eba, h, cs)
                        nc.tensor.matmul(py[:], stateb[h][:], cd[:],
                                         start=False, stop=True)
                    nc.vector.scalar_tensor_tensor(
                        ycm[k][rows, cs], xs_cm[k][rows, cs],
                        drep_s[rows, k:k + 1], py[:], op0=OP.mult,
                        op1=OP.add)
                    bw = pC.tile([128, DS], bf16, name="bw", tag="bw",
                                 bufs=3)
                    nc.gpsimd.tensor_scalar(
                        bw[:], xs_tm[c][:, DIN:DIN + DS],
                        tmv[c][:, 64 + h:65 + h], None, op0=OP.mult)
                    psc = ps3p.tile([64, PHD], f32, name="psc", tag="psc",
                                    bufs=2)
                    nc.tensor.matmul(psc[:], bw[:],
                                     xs_tm[c][:, PHD * h:PHD * (h + 1)],
                                     start=True, stop=True)
                    nc.vector.scalar_tensor_tensor(
                        state[k][rows, :], state[k][rows, :],
                        arep[k][rows, c:c + 1], psc[:], op0=OP.mult,
                        op1=OP.add)

            b1_in = dram.tile([128, 16 * PHD], bf16, name="b1_in")
            b1_out = dram.tile([256, 16 * PHD], bf16, name="b1_out")
            steb = pC.tile([128, 16 * PHD], bf16, name="steb")
            for k in range(16):
                nc.vector.tensor_copy(steb[:, PHD * k:PHD * (k + 1)],
                                      state[k][:])
            nc.sync.dma_start(b1_in[:], steb[:])
            nc.gpsimd.collective_compute(
                "AllGather", OP.bypass, replica_groups=RG,
                ins=[b1_in.opt()], outs=[b1_out.opt()])
            dbg_dump("d_st0", state[0][:], [128, PHD])

            h0bf2 = [pC.tile([64, PHD], bf16, name=f"h0bf2{h}")
                     for h in range(NHM)]
            for k in range(16):
                rcv = pC.tile([128, PHD], bf16, name="rcv", tag="rcv",
                              bufs=2)
                nc.sync.dma_start(rcv[:],
                                  b1_out[0:128, PHD * k:PHD * (k + 1)])
                for j in (0, 1):
                    nc.vector.tensor_scalar(
                        h0bf2[2 * k + j][:], rcv[64 * j:64 * j + 64, :],
                        is_second[0:64, :], None, op0=OP.mult)
            for c in range(NCH):
                cs = slice(L * c, L * (c + 1))
                lcf2 = stage_lc(c)
                eba2 = None
                for h in range(NHM):
                    k = h // 2
                    rows = slice(64 * (h % 2), 64 * (h % 2) + 64)
                    if h % 16 == 0:
                        _, eba2 = stage_bcast(lcf2, h // 16)
                    cd = make_cdec(eba2, h, cs)
                    pyc = ps3p.tile([64, L], f32, name="pyc", tag="pyc",
                                    bufs=3)
                    nc.tensor.matmul(pyc[:], h0bf2[h][:], cd[:], start=True,
                                     stop=True)
                    # ycm += cumalpha * (h0^T @ Cdec)
                    nc.vector.scalar_tensor_tensor(
                        ycm[k][rows, cs], pyc[:],
                        carep[k][rows, c:c + 1], ycm[k][rows, cs],
                        op0=OP.mult, op1=OP.add)
            dbg_dump("d_y0", ycm[0][:], [128, TOK], mybir.dt.bfloat16)
            ps3.close()
            stC.close()
            stB.close()

        if PHASES >= 4:
            # ======== P4: gated norm + out_proj + x1 + rmsnorm2 ========
            stE = stack()
            pE = stE.enter_context(tc.tile_pool(name="pE", bufs=1))  # ..P6
            stD = stack()
            pD = stD.enter_context(tc.tile_pool(name="pD", bufs=1))  # ..P5
            st4 = stack()
            p4 = st4.enter_context(tc.tile_pool(name="p4", bufs=1))
            ps4s = stack()
            ps4 = ps4s.enter_context(tc.tile_pool(name="ps4", bufs=1,
                                                  space="PSUM"))
            g = [p4.tile([128, TOK], bf16, name=f"g{k}") for k in range(16)]
            for k in range(16):
                zs = p4.tile([128, TOK], bf16, name="zs", tag="zs", bufs=4)
                nc.sync.dma_start(zs[:], zsil_d[128 * k:128 * (k + 1), :])
                nc.vector.tensor_mul(g[k][:], ycm[k][:], zs[:])
            ssq = ps4.tile([1, TOK], f32, name="ssqg", tag="ssqg", bufs=1)
            for k in range(16):
                sq = p4.tile([128, TOK], bf16, name="gsq", tag="gsq", bufs=3)
                nc.vector.tensor_mul(sq[:], g[k][:], g[k][:])
                nc.tensor.matmul(ssq[:], onesb[:], sq[:], start=(k == 0),
                                 stop=(k == 15))
            rms = p4.tile([1, TOK], f32, name="grms")
            nc.scalar.activation(rms[:], ssq[:], AF.Sqrt,
                                 bias=eps_c[0:1, :], scale=1.0 / DIN)
            rinv = p4.tile([1, TOK], f32, name="grinv")
            nc.vector.reciprocal(rinv[:], rms[:])
            rbc = p4.tile([128, TOK], f32, name="grbc")
            nc.gpsimd.partition_broadcast(rbc[:], rinv[:], channels=128)
            for k in range(16):
                nc.vector.scalar_tensor_tensor(g[k][:], g[k][:],
                                               mnw_s[:, k:k + 1], rbc[:],
                                               op0=OP.mult, op1=OP.mult)
            dbg_dump("d_g0", g[0][:], [128, TOK], mybir.dt.bfloat16)

            x1 = [pD.tile([128, TOK], f32, name=f"x1_{i}")
                  for i in range(8)]
            x1pb = pD.tile([128, 8], bf16, name="x1pb")
            for mb in range(8):
                sla = wslab(w_outproj, 128 * mb, 128, 8, f"wopa{mb}")
                slb = wslab(w_outproj, 128 * mb, 128, 8, f"wopb{mb}",
                            r0=1024)
                po = ps4.tile([128, TOK], f32, name="po", tag="pbig4",
                              bufs=5)
                for k in range(16):
                    sl_, kk = (sla, k) if k < 8 else (slb, k - 8)
                    nc.tensor.matmul(po[:], sl_[:, kk, :], g[k][:],
                                     start=(k == 0), stop=(k == 15))
                xre = p4.tile([128, TOK], f32, name="xre", tag="xre", bufs=2)
                nc.sync.dma_start(xre[:],
                                  xin[128 * mb:128 * (mb + 1), 3:TH])
                nc.vector.scalar_tensor_tensor(x1[mb][:], xre[:], 1.0,
                                               po[:], op0=OP.mult,
                                               op1=OP.add)
                nc.vector.tensor_copy(x1pb[:, mb:mb + 1],
                                      x1[mb][:, TOK - 1:TOK])
            # deferred rmsnorm2: qkv runs on raw x1 (bf16); the per-token
            # 1/rms scale commutes with the GEMM and lands in the evacs.
            x1b = [pD.tile([128, TOK], bf16, name=f"x1b{i}")
                   for i in range(8)]
            for i in range(8):
                nc.vector.tensor_copy(x1b[i][:], x1[i][:])
            ssq1 = ps4.tile([1, TOK], f32, name="ssq1", tag="ssq1", bufs=1)
            for i in range(8):
                sq1 = p4.tile([128, TOK], bf16, name="sq1", tag="sq1",
                              bufs=3)
                nc.vector.tensor_mul(sq1[:], x1b[i][:], x1b[i][:])
                nc.tensor.matmul(ssq1[:], onesb[:], sq1[:], start=(i == 0),
                                 stop=(i == 7))
            rms1 = p4.tile([1, TOK], f32, name="rms1")
            nc.scalar.activation(rms1[:], ssq1[:], AF.Sqrt,
                                 bias=eps_c[0:1, :], scale=1.0 / C_)
            rinv1 = pD.tile([1, TOK], f32, name="rinv1")
            nc.vector.reciprocal(rinv1[:], rms1[:])
            rinv1b = p4.tile([1, TOK], bf16, name="rinv1b")
            nc.vector.tensor_copy(rinv1b[:], rinv1[:])
            rbc1 = pD.tile([128, TOK], bf16, name="rbc1")
            nc.gpsimd.partition_broadcast(rbc1[:], rinv1b[:], channels=128)
            rinv1_tm = pD.tile([128, 4], f32, name="rinv1_tm")
            nc.sync.dma_start(rinv1_dd[:], rinv1[:])
            nc.sync.dma_start(
                rinv1_tm[:],
                rinv1_dd[0:1, :].rearrange("a (c p) -> (a p) c", p=128))
            dbg_dump("d_x1_0", x1[0][:], [128, TOK])
            ps4s.close()
            st4.close()

        if PHASES >= 5:
            # ================= P5: attention =================
            st5 = stack()
            p5 = st5.enter_context(tc.tile_pool(name="p5", bufs=1))
            ps5s = stack()
            ps5 = ps5s.enter_context(tc.tile_pool(name="ps5", bufs=1,
                                                  space="PSUM"))
            amask = []
            for r in range(4):
                # keep when t >= s: f - p + (512*qb - 128*sb) >= 0,
                # variant j = sb - 4*qb in {0..3} -> base = -128*j
                m = p5.tile([128, 512], bf16, name=f"amask{r}")
                nc.vector.memset(m, 0.0)
                nc.gpsimd.affine_select(out=m, in_=m, compare_op=OP.is_ge,
                                        fill=NEG, base=-128 * r,
                                        channel_multiplier=-1,
                                        pattern=[[1, 512]])
                amask.append(m)
            qloc = [p5.tile([128, TOK], bf16, name=f"qloc{i}")
                    for i in range(8)]
            kloc = p5.tile([64, TOK], bf16, name="kloc")
            for mb in range(8):
                sl = wslab(w_att, 128 * mb, 128, 8, f"wq{mb}")
                pq = ps5.tile([128, TOK], f32, name="pq", tag="pbig5",
                              bufs=2)
                for k in range(8):
                    nc.tensor.matmul(pq[:], sl[:, k, :], x1b[k][:],
                                     start=(k == 0), stop=(k == 7))
                nc.vector.tensor_mul(qloc[mb][:], pq[:], rbc1[:])
            slk = wslab(w_att, 1024, 64, 8, "wkp")
            pk = ps5.tile([64, TOK], f32, name="pk", tag="psx", bufs=3)
            for k in range(8):
                nc.tensor.matmul(pk[:], slk[:, k, :], x1b[k][:],
                                 start=(k == 0), stop=(k == 7))
            nc.vector.tensor_mul(kloc[:], pk[:], rbc1[0:64, :])
            vloc = [p5.tile([128, 65], bf16, name=f"vloc{tb}")
                    for tb in range(4)]
            slv = wsl.tile([128, 8, 64], bf16, name="wvp", tag="wslab")
            nc.sync.dma_start(
                slv[:],
                w_att[:, 1088:1152].rearrange("(t p) m -> p t m", p=128))
            for tb in range(4):
                pv = ps5.tile([128, 64], f32, name="pv", tag="psx", bufs=3)
                for k in range(8):
                    nc.tensor.matmul(pv[:],
                                     x1b[k][:, 128 * tb:128 * (tb + 1)],
                                     slv[:, k, :], start=(k == 0),
                                     stop=(k == 7))
                nc.vector.tensor_scalar(vloc[tb][:, 0:64], pv[:],
                                        rinv1_tm[:, tb:tb + 1], None,
                                        op0=OP.mult)
                nc.vector.memset(vloc[tb][:, 64:65], 1.0)
            dbg_dump("d_q0", qloc[0][:], [128, TOK], mybir.dt.bfloat16)

            b2_in = dram.tile([652, TOK], bf16, name="b2_in")
            b2_out = dram.tile([1304, TOK], bf16, name="b2_out")
            for i in range(4):
                nc.sync.dma_start(b2_in[128 * i:128 * (i + 1), :],
                                  qloc[4 + i][:])
            nc.sync.dma_start(b2_in[512:576, :], kloc[:])
            for tb in range(4):
                nc.sync.dma_start(
                    b2_in[576:641, 128 * tb:128 * (tb + 1)]
                    .rearrange("r c -> c r"), vloc[tb][:])
            nc.sync.dma_start(
                b2_in[644:652, 0:128].rearrange("f p -> p f"), x1pb[:])
            nc.gpsimd.collective_compute(
                "AllGather", OP.bypass, replica_groups=RG,
                ins=[b2_in.opt()], outs=[b2_out.opt()])

            def masked2(dst, local_ap, recv_ap, local_is_first):
                # dst/recv must share a base partition; local may be shifted.
                P = local_ap.shape[0]
                ma = is_first if local_is_first else is_second
                mb_ = is_second if local_is_first else is_first
                nc.vector.tensor_scalar(dst, local_ap, ma[0:P, :], None,
                                        op0=OP.mult)
                nc.vector.scalar_tensor_tensor(dst, recv_ap, mb_[0:P, :],
                                               dst, op0=OP.mult, op1=OP.add)

            qall = [p5.tile([64, T_], bf16, name=f"qall{h}")
                    for h in range(8)]
            kall = p5.tile([64, T_], bf16, name="kall")
            vall = [p5.tile([128, 65], bf16, name=f"vall{gb}")
                    for gb in range(8)]
            for h in range(8):
                t = h // 2
                ro = 128 * t + 64 * (h % 2)
                rows = slice(64 * (h % 2), 64 * (h % 2) + 64)
                for half in (0, 1):
                    rcv = p5.tile([64, TOK], bf16, name="qr", tag="qrcv",
                                  bufs=3)
                    nc.sync.dma_start(
                        rcv[:],
                        b2_out[652 * half + ro:652 * half + ro + 64, :])
                    masked2(qall[h][:, TOK * half:TOK * (half + 1)],
                            qloc[t][rows, :], rcv[:],
                            local_is_first=(half == 0))
            for half in (0, 1):
                rcv = p5.tile([64, TOK], bf16, name="kr", tag="krcv", bufs=3)
                nc.sync.dma_start(
                    rcv[:], b2_out[652 * half + 512:652 * half + 576, :])
                masked2(kall[:, TOK * half:TOK * (half + 1)], kloc[:],
                        rcv[:], local_is_first=(half == 0))
            for gb in range(8):
                half, tb = gb // 4, gb % 4
                rcv = p5.tile([128, 65], bf16, name="vr", tag="vrcv", bufs=3)
                nc.sync.dma_start(
                    rcv[:], b2_out[652 * half + 576:652 * half + 641,
                                   128 * tb:128 * (tb + 1)]
                    .rearrange("r c -> c r"))
                masked2(vall[gb][:], vloc[tb][:], rcv[:],
                        local_is_first=(half == 0))
            x1p = p5.tile([128, 8], bf16, name="x1p")
            rx = p5.tile([128, 8], bf16, name="rx")
            nc.sync.dma_start(
                rx[:], b2_out[644:652, 0:128].rearrange("f p -> p f"))
            nc.vector.tensor_scalar(x1p[:], rx[:], is_second, None,
                                    op0=OP.mult)
            dbg_dump("d_qall0", qall[0][:], [64, T_], mybir.dt.bfloat16)
            dbg_dump("d_kall", kall[:], [64, T_], mybir.dt.bfloat16)

            yall = [p5.tile([64, T_], bf16, name=f"yall{h}")
                    for h in range(8)]
            for h in range(8):
                for qb in range(2):
                    qcols = slice(TOK * qb, TOK * (qb + 1))
                    pav = ps5.tile([65, TOK], f32, name="pav", tag="pav",
                                   bufs=2)
                    nsb = 4 * (qb + 1)
                    for sb in range(nsb):
                        psx = ps5.tile([128, TOK], f32, name="psx",
                                       tag="psx", bufs=3)
                        nc.tensor.matmul(psx[:],
                                         kall[:, 128 * sb:128 * (sb + 1)],
                                         qall[h][:, qcols], start=True,
                                         stop=True)
                        r = sb - 4 * qb
                        if 0 <= r <= 3:
                            nc.vector.tensor_add(psx[:], psx[:],
                                                 amask[r][:])
                        pexp = p5.tile([128, TOK], bf16, name="pexp",
                                       tag="pexp", bufs=4)
                        nc.scalar.activation(pexp[:], psx[:], AF.Exp)
                        nc.tensor.matmul(pav[:], vall[sb][:], pexp[:],
                                         start=(sb == 0),
                                         stop=(sb == nsb - 1))
                    rc = p5.tile([1, TOK], f32, name="rcs", tag="rcs",
                                 bufs=2)
                    nc.vector.reciprocal(rc[:], pav[64:65, :])
                    rcb = p5.tile([64, TOK], f32, name="rcb", tag="rcb",
                                  bufs=2)
                    nc.gpsimd.partition_broadcast(rcb[:], rc[:],
                                                  channels=64)
                    nc.vector.tensor_mul(yall[h][:, qcols], pav[0:64, :],
                                         rcb[:])
            dbg_dump("d_yall0", yall[0][:], [64, T_], mybir.dt.bfloat16)

            # exchange 3 + proj rhs assembly (per-head base-0 builds)
            wph = [p5.tile([64, TOK + 1], bf16, name=f"wph{h}", tag="wph",
                           bufs=8) for h in range(8)]
            yown = [p5.tile([128, TOK + 1], bf16, name=f"yown{t}")
                    for t in range(4)]
            for h in range(8):
                t = h // 2
                rows = slice(64 * (h % 2), 64 * (h % 2) + 64)
                nc.vector.tensor_scalar(wph[h][:, :],
                                        yall[h][:, TOK - 1:T_],
                                        is_first[0:64, :], None,
                                        op0=OP.mult)
                nc.vector.scalar_tensor_tensor(
                    wph[h][:, 1:TOK + 1], yall[h][:, 0:TOK],
                    is_second[0:64, :], wph[h][:, 1:TOK + 1],
                    op0=OP.mult, op1=OP.add)
                yoh = p5.tile([64, TOK + 1], bf16, name="yoh", tag="yoh",
                              bufs=2)
                nc.vector.tensor_scalar(yoh[:, :],
                                        yall[h][:, TOK - 1:T_],
                                        is_second[0:64, :], None,
                                        op0=OP.mult)
                nc.vector.scalar_tensor_tensor(
                    yoh[:, 1:TOK + 1], yall[h][:, 0:TOK],
                    is_first[0:64, :], yoh[:, 1:TOK + 1],
                    op0=OP.mult, op1=OP.add)
                nc.vector.tensor_copy(yown[t][rows, :], yoh[:])
            b3_in = dram.tile([512, TOK + 1], bf16, name="b3_in")
            b3_out = dram.tile([1024, TOK + 1], bf16, name="b3_out")
            for h in range(8):
                nc.sync.dma_start(b3_in[64 * h:64 * (h + 1), :], wph[h][:])
            nc.gpsimd.collective_compute(
                "AllGather", OP.bypass, replica_groups=RG,
                ins=[b3_in.opt()], outs=[b3_out.opt()])

            yfull = yown + [p5.tile([128, TOK + 1], bf16, name=f"yfp{t}")
                            for t in range(4)]
            for t in range(4):
                r0 = p5.tile([128, TOK + 1], bf16, name="yr0", tag="yr0",
                             bufs=2)
                r1 = p5.tile([128, TOK + 1], bf16, name="yr1", tag="yr1",
                             bufs=2)
                nc.sync.dma_start(r0[:], b3_out[128 * t:128 * (t + 1), :])
                nc.sync.dma_start(
                    r1[:], b3_out[512 + 128 * t:512 + 128 * (t + 1), :])
                nc.vector.tensor_scalar(yfull[4 + t][:], r0[:], is_second,
                                        None, op0=OP.mult)
                nc.vector.scalar_tensor_tensor(yfull[4 + t][:], r1[:],
                                               is_first, yfull[4 + t][:],
                                               op0=OP.mult, op1=OP.add)

            x2 = [pE.tile([128, TOK], f32, name=f"x2_{i}")
                  for i in range(8)]
            x2p = resid.tile([128, 8], f32, name="x2p")
            for mb in range(8):
                sl = wslab(w_proj, 128 * mb, 128, 8, f"wpj{mb}")
                pp = ps5.tile([128, TOK], f32, name="pp", tag="pbig5",
                              bufs=2)
                pp1 = ps5.tile([128, 1], f32, name="pp1", tag="pp1", bufs=1)
                for k in range(8):
                    nc.tensor.matmul(pp[:], sl[:, k, :],
                                     yfull[k][:, 1:TOK + 1],
                                     start=(k == 0), stop=(k == 7))
                    nc.tensor.matmul(pp1[:], sl[:, k, :], yfull[k][:, 0:1],
                                     start=(k == 0), stop=(k == 7))
                nc.vector.scalar_tensor_tensor(x2[mb][:], x1[mb][:], 1.0,
                                               pp[:], op0=OP.mult,
                                               op1=OP.add)
                tpv = p5.tile([128, 1], f32, name="tpv", tag="tpv", bufs=2)
                nc.vector.tensor_add(tpv[:], x1p[:, mb:mb + 1], pp1[:])
                nc.vector.tensor_scalar(x2p[:, mb:mb + 1], tpv[:],
                                        is_second, None, op0=OP.mult)
            dbg_dump("d_x2_0", x2[0][:], [128, TOK])
            ps5s.close()
            st5.close()
            stD.close()

        if PHASES >= 6:
            # ================= P6: cmix =================
            st6 = stack()
            p6 = st6.enter_context(tc.tile_pool(name="p6", bufs=1))
            ps6s = stack()
            ps6 = ps6s.enter_context(tc.tile_pool(name="ps6", bufs=1,
                                                  space="PSUM"))
            z3 = [p6.tile([128, TOK + 1], bf16, name=f"z3_{i}")
                  for i in range(8)]
            rmsnorm_cm([x2[i][:] for i in range(8)],
                       [z3[i][:, 1:TOK + 1] for i in range(8)], TOK, p6,
                       ps6, C_, "n2")
            sqp = p6.tile([128, 8], bf16, name="sqp")
            nc.vector.tensor_mul(sqp[:], x2p[:], x2p[:])
            psp = ps6.tile([1, 8], f32, name="psp", tag="psp", bufs=1)
            nc.tensor.matmul(psp[:], onesb[:], sqp[:], start=True,
                             stop=True)
            ssp = p6.tile([1, 1], f32, name="ssp")
            nc.vector.tensor_reduce(ssp[:], psp[:],
                                    axis=mybir.AxisListType.X, op=OP.add)
            nc.scalar.activation(ssp[:], ssp[:], AF.Sqrt,
                                 bias=eps_c[0:1, :], scale=1.0 / C_)
            nc.vector.reciprocal(ssp[:], ssp[:])
            rpb = p6.tile([128, 1], f32, name="rpb")
            nc.gpsimd.partition_broadcast(rpb[:], ssp[:], channels=128)
            for i in range(8):
                nc.vector.scalar_tensor_tensor(z3[i][:, 0:1],
                                               x2p[:, i:i + 1], 1.0,
                                               rpb[:], op0=OP.mult,
                                               op1=OP.mult)
            dbg_dump("d_z3_0", z3[0][:], [128, TOK + 1], mybir.dt.bfloat16)

            xk = [p6.tile([128, TOK], bf16, name=f"xk{i}")
                  for i in range(8)]
            xr = [p6.tile([128, TOK], bf16, name=f"xr{i}")
                  for i in range(8)]
            for i in range(8):
                nc.vector.tensor_scalar(xk[i][:], z3[i][:, 1:TOK + 1],
                                        mk1_s[:, i:i + 1], None,
                                        op0=OP.mult)
                nc.vector.scalar_tensor_tensor(xk[i][:], z3[i][:, 0:TOK],
                                               mk_s[:, i:i + 1], xk[i][:],
                                               op0=OP.mult, op1=OP.add)
                nc.vector.tensor_scalar(xr[i][:], z3[i][:, 1:TOK + 1],
                                        mr1_s[:, i:i + 1], None,
                                        op0=OP.mult)
                nc.vector.scalar_tensor_tensor(xr[i][:], z3[i][:, 0:TOK],
                                               mr_s[:, i:i + 1], xr[i][:],
                                               op0=OP.mult, op1=OP.add)

            kE = [p6.tile([128, TOK], bf16, name=f"kE{i}")
                  for i in range(32)]
            for mb in range(32):
                sl = wslab(w_key, 128 * mb, 128, 8, f"wky{mb}")
                pky = ps6.tile([128, TOK], f32, name="pky", tag="pbig6",
                               bufs=6)
                for k in range(8):
                    nc.tensor.matmul(pky[:], sl[:, k, :], xk[k][:],
                                     start=(k == 0), stop=(k == 7))
                nc.scalar.activation(kE[mb][:], pky[:], AF.Erf,
                                     scale=1.0 / _DEN, bias=erfb_c[:, :])
            r_sb = [p6.tile([128, TOK], bf16, name=f"r_sb{i}")
                    for i in range(8)]
            for mb in range(8):
                sl = wslab(w_rec, 128 * mb, 128, 8, f"wrc{mb}")
                pr = ps6.tile([128, TOK], f32, name="pr", tag="pbig6",
                              bufs=6)
                for k in range(8):
                    nc.tensor.matmul(pr[:], sl[:, k, :], xr[k][:],
                                     start=(k == 0), stop=(k == 7))
                nc.scalar.activation(r_sb[mb][:], pr[:], AF.Sigmoid)
            dbg_dump("d_kE0", kE[0][:], [128, TOK], mybir.dt.bfloat16)
            dbg_dump("d_r0", r_sb[0][:], [128, TOK], mybir.dt.bfloat16)

            for mb in range(8):
                slab = wslab(w_val, 128 * mb, 128, 32, f"wvl{mb}", pool=p6,
                             tag="wslab_v", bufs=2)
                pvv = ps6.tile([128, TOK], f32, name="pvv", tag="pbig6",
                               bufs=6)
                for k in range(32):
                    nc.tensor.matmul(pvv[:], slab[:, k, :], kE[k][:],
                                     start=(k == 0), stop=(k == 31))
                tmpv = p6.tile([128, TOK], f32, name="tmpv", tag="tmpv",
                               bufs=2)
                nc.vector.tensor_scalar(tmpv[:], pvv[:],
                                        vbias_s[:, mb:mb + 1], None,
                                        op0=OP.add)
                nc.vector.tensor_mul(tmpv[:], tmpv[:], r_sb[mb][:])
                outt = p6.tile([128, TOK], f32, name="outt", tag="outt",
                               bufs=2)
                nc.vector.tensor_add(outt[:], x2[mb][:], tmpv[:])
                nc.sync.dma_start(out_d[128 * mb:128 * (mb + 1), :],
                                  outt[:])
            ps6s.close()
            st6.close()

        for s in reversed(_open):
            s.close()
        whole.close()

    nc.compile()
    return nc, dbg_outs


# ================= host glue =================

def _prep_inputs(x, in_proj_w, conv_w, conv_b, dt_bias, A_log, D, mnorm_w,
                 out_proj_w, attn_w, proj_w, time_maa_k, time_maa_r, key_w,
                 recept_w, value_w):
    f32 = np.float32

    def b(a):
        return np.ascontiguousarray(np.asarray(a, f32).astype(BF16))

    x = np.asarray(x, f32)
    shared = {
        "w_inproj": b(in_proj_w),
        "convw": np.ascontiguousarray(
            np.asarray(conv_w, f32).reshape(17, 128, DCONV)
            .transpose(1, 0, 2)),
        "convb": np.ascontiguousarray(
            np.asarray(conv_b, f32).reshape(17, 128).T),
        "dtb": np.ascontiguousarray(
            np.asarray(dt_bias, f32).reshape(NHM, 1)),
        "aneg": np.ascontiguousarray(
            (-np.exp(np.asarray(A_log, f32))).reshape(NHM, 1)),
        # drep[p, k] = D[2k + (p >= 64)]
        "drep": np.ascontiguousarray(np.stack(
            [np.concatenate([np.full(64, D2[0]), np.full(64, D2[1])])
             for D2 in np.asarray(D, f32).reshape(16, 2)], axis=1)
            .astype(f32)),
        "mnw": np.ascontiguousarray(
            np.asarray(mnorm_w, f32).reshape(16, 128).T),
        "w_outproj": b(out_proj_w),
        "mk": np.ascontiguousarray(
            np.asarray(time_maa_k, f32).reshape(8, 128).T),
        "mk1": np.ascontiguousarray(
            (1.0 - np.asarray(time_maa_k, f32)).reshape(8, 128).T),
        "mr": np.ascontiguousarray(
            np.asarray(time_maa_r, f32).reshape(8, 128).T),
        "mr1": np.ascontiguousarray(
            (1.0 - np.asarray(time_maa_r, f32)).reshape(8, 128).T),
        "w_key": b(key_w),
        "w_val": b(0.5 * np.asarray(value_w, f32)),
        "vbias": np.ascontiguousarray(
            (0.5 * np.asarray(value_w, f32).sum(0)).reshape(8, 128).T),
        "w_rec": b(recept_w),
    }
    ef = np.zeros((NHM, DIN), f32)
    for k in range(16):
        ef[2 * k, 128 * k:128 * k + 64] = 1.0
        ef[2 * k + 1, 128 * k + 64:128 * k + 128] = 1.0
    shared["efull"] = ef

    attn_w = np.asarray(attn_w, f32)
    proj_w = np.asarray(proj_w, f32)
    scale = 1.0 / np.sqrt(np.float32(HD))
    in_maps = []
    for core in range(N_CORES):
        bi, half = core // 2, core % 2
        start = half * TOK
        xcm = x[bi].T
        xs = np.zeros((C_, TH), f32)
        xs[:, 3:] = xcm[:, start:start + TOK]
        if start >= 3:
            xs[:, 0:3] = xcm[:, start - 3:start]
        myh = np.arange(8 * half, 8 * half + 8)
        oth = np.arange(8 * (1 - half), 8 * (1 - half) + 8)
        qcols = attn_w[:, :C_].reshape(C_, NH, HD)
        wq_perm = np.concatenate(
            [qcols[:, myh].reshape(C_, 512),
             qcols[:, oth].reshape(C_, 512)], axis=1) * scale
        w_att_c = np.concatenate([wq_perm, attn_w[:, C_:]], axis=1)
        prows = proj_w.reshape(NH, HD, C_)
        w_proj_c = np.concatenate(
            [prows[myh].reshape(512, C_), prows[oth].reshape(512, C_)],
            axis=0)
        mskc = np.zeros((128, 2), f32)
        mskc[:, 0] = 1.0 - half
        mskc[:, 1] = half
        m = dict(shared)
        m["xin"] = np.ascontiguousarray(xs)
        m["w_att"] = np.ascontiguousarray(w_att_c.astype(BF16))
        m["w_proj"] = np.ascontiguousarray(w_proj_c.astype(BF16))
        m["msk"] = mskc
        in_maps.append(m)
    return in_maps


def kernel(**inputs):
    from concourse.bass_utils import run_bass_kernel_spmd

    if "nc" not in _CACHE:
        _CACHE["nc"], _CACHE["dbg"] = _build()
    nc = _CACHE["nc"]
    in_maps = _prep_inputs(**inputs)
    res = run_bass_kernel_spmd(nc, in_maps, core_ids=list(range(N_CORES)))
    _CACHE["results"] = res
    out = np.empty((B_, T_, C_), np.float32)
    for core in range(N_CORES):
        bi, half = core // 2, core % 2
        out[bi, half * TOK:(half + 1) * TOK, :] = \
            np.asarray(res.results[core]["out"], np.float32).T
    return out
